# revision 2
# baseline (speedup 1.0000x reference)
"""Trainium2 Bass kernel for nn_CapsuleNeuralNetworkV2 (8 cores, data-parallel).

Math (per sample, 8 capsule iterations then decoder):
  v = h.reshape(4, 196)
  q = v @ W1.T + b1 ; k = v @ W2.T + b2 ; u = v @ W3.T + b3
  scores[t,s] = q_t . k_s  ->  softmax over s -> h'_t = sum_s P[t,s] u_s
  dec = relu(h Wd1.T + bd1) Wd2.T + bd2 ; out = softmax(dec Wo.T + bo)

Host-side algebra:
  scores[t,s] = v_t . z_s + r_s,  z_s = G v_s + c, r_s = a.v_s + d,
  G = W1.T W2, a = W2.T b1, c = W1.T b2, d = b1.b2; biases fused via an
  augmented ones row so one matmul emits [z | r | u] per slot.

v3 layout/schedule changes vs v2:
  - h slot is 197 wide ([v(196) | 1]); zu slot is 393 ([z | r | u]); the
    dots run over 197 elems and pick up r via the ones column.
  - zu matmuls write two 2-slot PSUM tiles; each is evacuated by ONE Act
    copy (f32->bf16), replacing four per-slot copies.
  - vt transposes write one bf16 PSUM tile; two DVE 2x copies evacuate.
  - dots are 16 STT+accum ops, combine is 4 chains of (seed + 3 MACs);
    both are spread across DVE/Act/Pool by a static table tuned against
    the instruction cost model.
  - probs = e * (1/sum) via one broadcast tensor_tensor.
"""

import numpy as np
import ml_dtypes

import concourse.bass as bass
import concourse.tile as tile
from concourse import bacc, mybir
from concourse.bass import ds
from concourse.bass_utils import run_bass_kernel_spmd
from concourse.masks import make_identity

FR = mybir.dt.float32r
BF = mybir.dt.bfloat16
F32 = mybir.dt.float32
AF = mybir.ActivationFunctionType
ALU = mybir.AluOpType

B = 32768
NCORES = 8
P = 128
T = 4
FV = 196
FEAT = 784
SLOT = FV + 1  # 197: 196 data + ones col
ZSLOT = 2 * FV + 1  # 393: z(196) | r(1) | u(196)

# --- static engine tables (v=DVE, a=Act, p=Pool), tuned vs cost model ---
# dots[t][s] (Act cannot run STT)
DOTS_ENG = [
    "vp..",
    "pp..",
    "pp..",
    "pp..",
]
# combine: SEED_ENG[t] + MAC_ENG[t] (3 chained MACs; MACs only v or p)
# combine mul engine per (t, s); adds are two batched DVE tensor_tensor
MUL_ENG = [
    "aapp",
    "aapp",
    "aapp",
    "appp",
]
# vt-evac engines for (chunk1, chunk2)
VT_ENG = "va"
# s-slots computed via a DVE products TT + 4 cheap TSP-accums
PROD_S = (2, 3)


def _ap(t, dims, offset_elems=0):
    a = t[:] if hasattr(t, "tile") or not isinstance(t, bass.AP) else t
    return bass.AP(tensor=a.tensor, offset=a.offset + offset_elems, ap=dims)


def build(nsub=8, ngroups=4, unroll=False, zu_bufs=1, vtps_bufs=2, mpd_bufs=2,
          h_bufs=3, wk_bufs=5, sm_bufs=8, zu_mode="half", hf_bufs=1):
    """One NeuronCore program processing nsub*ngroups*128 samples."""
    bpc = nsub * ngroups * P
    nc = bacc.Bacc("TRN2", target_bir_lowering=False, debug=False)

    x_d = nc.dram_tensor("x", [bpc, FEAT], BF, kind="ExternalInput")
    zu_d = nc.dram_tensor("zu_w", [P, 2, ZSLOT], BF, kind="ExternalInput")
    d1_d = nc.dram_tensor("dec1_w", [P, 8, FEAT], BF, kind="ExternalInput")
    d2_d = nc.dram_tensor("dec2_w", [P, 7, FEAT], BF, kind="ExternalInput")
    ow_d = nc.dram_tensor("out_w", [P, 7, 10], BF, kind="ExternalInput")
    out_d = nc.dram_tensor("out", [bpc, 10], F32, kind="ExternalOutput")

    with tile.TileContext(nc) as tc:
        consts = tc.alloc_tile_pool(name="consts", bufs=1)
        hp = tc.alloc_tile_pool(name="h", bufs=h_bufs)
        wk = tc.alloc_tile_pool(name="wk", bufs=wk_bufs)
        wkd = tc.alloc_tile_pool(name="wkd", bufs=1)
        sm = tc.alloc_tile_pool(name="small", bufs=sm_bufs)
        pp = tc.alloc_tile_pool(name="ps", bufs=zu_bufs, space="PSUM")
        pv = tc.alloc_tile_pool(name="pv", bufs=vtps_bufs, space="PSUM")

        ident_f = consts.tile([P, P], F32)
        make_identity(nc, ident_f)
        ident = consts.tile([P, P], FR)
        nc.vector.tensor_copy(ident, ident_f)
        ident_b = consts.tile([P, P], BF)
        nc.vector.tensor_copy(ident_b, ident_f)
        ones_c = consts.tile([P, 512], F32)
        nc.vector.memset(ones_c, 1.0)
        zu_w = consts.tile([P, 2, ZSLOT], BF)
        nc.sync.dma_start(out=zu_w, in_=zu_d[:, :, :])
        d1_w = consts.tile([P, 8, FEAT], BF)
        nc.sync.dma_start(out=d1_w, in_=d1_d[:, :, :])
        d2_w = consts.tile([P, 7, FEAT], BF)
        nc.sync.dma_start(out=d2_w, in_=d2_d[:, :, :])
        ow_w = consts.tile([P, 7, 10], BF)
        nc.sync.dma_start(out=ow_w, in_=ow_d[:, :, :])

        def eng(c):
            return {"v": nc.vector, "p": nc.gpsimd, "a": nc.scalar}[c]

        def capsule_iter(h_cur, h_nxt, j):
            """One capsule-attention iteration: h_nxt <- attn(h_cur)."""
            # PE transposes: batch-major h -> feature-major V.T chunks
            vt_ps = pv.tile([P, T, 2, P], BF, tag="vtps")
            for t in range(T):
                nc.tensor.transpose(vt_ps[:, t, 0, :], h_cur[:, t, 0:P], ident_b)
                # includes the ones column -> row 68 is 1.0
                nc.tensor.transpose(vt_ps[0:69, t, 1, :],
                                    h_cur[:, t, P : P + 69], ident_b)
            vt = wk.tile([P, T, 2, P], BF, tag="vt")
            for ci, (dst, src) in enumerate(
                [(vt[:, :, 0, :], vt_ps[:, :, 0, :]),
                 (vt[0:69, :, 1, :], vt_ps[0:69, :, 1, :])]
            ):
                c = VT_ENG[ci]
                if c == "a":
                    nc.scalar.copy(dst, src)
                elif c == "v":
                    nc.vector.tensor_copy(dst, src)
                else:
                    nc.gpsimd.tensor_copy(dst, src)

            # z|r|u fused matmuls + Act evacuation
            zu = wk.tile([P, T, ZSLOT], BF, tag="zu")
            if zu_mode == "half":
                for half in range(2):
                    # 512-wide slots keep each matmul within one PSUM bank
                    zu_ps = pp.tile([P, 2, 512], F32, tag=f"zu{half}")
                    for k in range(2):
                        s = 2 * half + k
                        nc.tensor.matmul(zu_ps[:, k, 0:ZSLOT], vt[:, s, 0, :],
                                         zu_w[:, 0, :], start=True, stop=False)
                        nc.tensor.matmul(zu_ps[:, k, 0:ZSLOT], vt[0:69, s, 1, :],
                                         zu_w[0:69, 1, :], start=False, stop=True)
                    nc.scalar.copy(zu[:, 2 * half : 2 * half + 2, :],
                                   zu_ps[:, :, 0:ZSLOT])
            else:  # per-slot PSUM tiles, finer cross-tile pipelining
                for s in range(T):
                    zu_ps = pp.tile([P, ZSLOT], F32, tag=f"zs{s}")
                    nc.tensor.matmul(zu_ps, vt[:, s, 0, :],
                                     zu_w[:, 0, :], start=True, stop=False)
                    nc.tensor.matmul(zu_ps, vt[0:69, s, 1, :],
                                     zu_w[0:69, 1, :], start=False, stop=True)
                    nc.scalar.copy(zu[:, s, :], zu_ps)

            # dots: per-half DVE products TT (starts right after that
            # half's evacuation) + 16 cheap TSP accums (4x mode) on DVE.
            dots = sm.tile([P, T, T], F32, tag="dots")
            scr = sm.tile([P, 3, SLOT], BF, tag="scr", bufs=8)
            for half in range(2):
                prod = sm.tile([P, 2, T, SLOT], BF, tag=f"prod{half}", bufs=2)
                hin = _ap(h_cur, [h_cur[:].ap[0], [0, 2], [SLOT, T], [1, SLOT]])
                zin = _ap(zu, [zu[:].ap[0], [ZSLOT, 2], [0, T], [1, SLOT]],
                          offset_elems=half * 2 * ZSLOT)
                nc.vector.tensor_tensor(out=prod, in0=hin, in1=zin, op=ALU.mult)
                for k in range(2):
                    s = 2 * half + k
                    for t in range(T):
                        nc.vector.tensor_scalar(
                            out=scr[:, 0, :],
                            in0=prod[:, k, t, :], scalar1=1.0,
                            scalar2=0.0, op0=ALU.mult, op1=ALU.add,
                            accum_out=dots[:, t, s : s + 1])

            # softmax over s (no max subtraction; |scores| < 30)
            e_t = sm.tile([P, T, T], F32, tag="e")
            nc.scalar.activation(e_t, dots, AF.Exp)
            sums = sm.tile([P, T], F32, tag="sums")
            nc.vector.reduce_sum(sums, e_t, axis=mybir.AxisListType.X)
            rec = sm.tile([P, T], F32, tag="rec")
            nc.vector.reciprocal(rec, sums)
            probs = sm.tile([P, T, T], F32, tag="probs")
            nc.vector.tensor_tensor(
                out=probs, in0=e_t,
                in1=_ap(rec, [rec[:].ap[0], [1, T], [0, T]]),
                op=ALU.mult,
            )

            # ones column for the next h
            nc.gpsimd.tensor_copy(h_nxt[:, :, FV:SLOT], ones_c[:, 0:T])
            # combine: pu[t,s] = P[t,s] * u_s on DVE/Act/Pool, then two
            # batched DVE adds: h'_t = (pu[t,0]+pu[t,1]) + (pu[t,2]+pu[t,3])
            pu = sm.tile([P, T, T, FV], BF, tag="pu", bufs=2)
            for t in range(T):
                for s in range(T):
                    c = MUL_ENG[t][s]
                    if c == "a":
                        nc.scalar.activation(
                            pu[:, t, s, :], zu[:, s, SLOT:ZSLOT], AF.Copy,
                            scale=probs[:, t, s : s + 1])
                    elif c == "v":
                        nc.vector.tensor_scalar_mul(
                            pu[:, t, s, :], zu[:, s, SLOT:ZSLOT],
                            probs[:, t, s : s + 1])
                    else:
                        nc.gpsimd.tensor_scalar_mul(
                            pu[:, t, s, :], zu[:, s, SLOT:ZSLOT],
                            probs[:, t, s : s + 1])
            q = sm.tile([P, T, 2, FV], BF, tag="q", bufs=2)
            ev = _ap(pu, [pu[:].ap[0], [T * FV, T], [2 * FV, 2], [1, FV]])
            od = _ap(pu, [pu[:].ap[0], [T * FV, T], [2 * FV, 2], [1, FV]],
                     offset_elems=FV)
            nc.vector.tensor_tensor(out=q, in0=ev, in1=od, op=ALU.add)
            nc.vector.tensor_tensor(out=h_nxt[:, :, 0:FV], in0=q[:, :, 0, :],
                                    in1=q[:, :, 1, :], op=ALU.add)

        def decoder(hs, g, goff=0):
            """Decoder over a chunk of <=4 tiles (N = len(hs)*128 wide)."""
            W = len(hs) * P
            # h.T chunks, slot-major: [128] x4 and [69] x4 (with ones row)
            ht1 = wkd.tile([P, T, W], BF, tag="ht1")
            ht2 = wkd.tile([69, T, W], BF, tag="ht2")
            for t in range(T):
                t1_ps = pv.tile([P, W], BF, tag="vtps")
                t2_ps = pv.tile([69, W], BF, tag="vtps")
                for j in range(len(hs)):
                    nc.tensor.transpose(
                        t1_ps[:, j * P : (j + 1) * P], hs[j][:, t, 0:P], ident_b
                    )
                    nc.tensor.transpose(
                        t2_ps[:, j * P : (j + 1) * P],
                        hs[j][:, t, P : P + 69], ident_b
                    )
                nc.scalar.copy(ht1[:, t, :], t1_ps)
                nc.vector.tensor_copy(ht2[:, t, :], t2_ps)

            # dec1 = relu(Wd1 @ h.T + bd1), feature-major, 7 M-chunks
            d1a = wkd.tile([P, 6, W], BF, tag="d1a")
            d1b = wkd.tile([17, W], BF, tag="d1b")
            nc.vector.tensor_copy(d1b, ones_c[0:17, 0:W])
            for m in range(7):
                mw = min(P, FEAT - m * P)
                mp = pv.tile([P, W], F32, tag="mpd", bufs=mpd_bufs)
                msl = slice(m * P, m * P + mw)
                for t in range(T):
                    nc.tensor.matmul(mp[0:mw, :], d1_w[:, t, msl], ht1[:, t, :],
                                     start=(t == 0), stop=False)
                for t in range(T):
                    nc.tensor.matmul(mp[0:mw, :], d1_w[0:69, 4 + t, msl],
                                     ht2[:, t, :], start=False, stop=(t == 3))
                if m < 6:
                    nc.scalar.activation(d1a[:, m, :], mp, AF.Relu)
                else:
                    nc.scalar.activation(d1b[0:16, :], mp[0:16, :], AF.Relu)

            # dec2 = Wd2 @ relu1 + bd2, feature-major
            d2a = wkd.tile([P, 6, W], BF, tag="d2a")
            d2b = wkd.tile([17, W], BF, tag="d2b")
            nc.vector.tensor_copy(d2b, ones_c[0:17, 0:W])
            for m in range(7):
                mw = min(P, FEAT - m * P)
                mp = pv.tile([P, W], F32, tag="mpd", bufs=mpd_bufs)
                msl = slice(m * P, m * P + mw)
                for c in range(6):
                    nc.tensor.matmul(mp[0:mw, :], d2_w[:, c, msl], d1a[:, c, :],
                                     start=(c == 0), stop=False)
                nc.tensor.matmul(mp[0:mw, :], d2_w[0:17, 6, msl], d1b,
                                 start=False, stop=True)
                if m < 6:
                    nc.vector.tensor_copy(d2a[:, m, :], mp)
                else:
                    nc.vector.tensor_copy(d2b[0:16, :], mp[0:16, :])

            # logits + softmax per subtile
            for j in range(len(hs)):
                jsl = slice(j * P, (j + 1) * P)
                lg = pv.tile([P, 10], F32, tag="mpd", bufs=mpd_bufs)
                for c in range(6):
                    nc.tensor.matmul(lg, d2a[:, c, jsl], ow_w[:, c, :],
                                     start=(c == 0), stop=False)
                nc.tensor.matmul(lg, d2b[:, jsl], ow_w[0:17, 6, :],
                                 start=False, stop=True)
                mx = sm.tile([P, 1], F32, tag="mx")
                nc.vector.reduce_max(mx, lg, axis=mybir.AxisListType.X)
                nmx = sm.tile([P, 1], F32, tag="nmx")
                nc.vector.tensor_scalar_mul(nmx, mx, -1.0)
                e10 = sm.tile([P, 10], F32, tag="e10")
                s10 = sm.tile([P, 1], F32, tag="s10")
                nc.scalar.activation(e10, lg, AF.Exp, bias=nmx, accum_out=s10)
                r10 = sm.tile([P, 1], F32, tag="r10")
                nc.vector.reciprocal(r10, s10)
                o10 = sm.tile([P, 10], F32, tag="o10")
                nc.vector.tensor_scalar_mul(o10, e10, r10)
                nc.sync.dma_start(
                    out=out_d[ds(g * (nsub * P) + (goff + j) * P, P), :],
                    in_=o10,
                )

        def body(g):
            hs = []
            for j in range(nsub):
                h0 = hp.tile([P, T, SLOT], BF, tag=f"h{j}")
                nc.sync.dma_start(
                    out=h0[:, :, 0:FV],
                    in_=x_d[ds(g * (nsub * P) + j * P, P), :].rearrange(
                        "p (t f) -> p t f", t=T
                    ),
                )
                nc.gpsimd.tensor_copy(h0[:, :, FV:SLOT], ones_c[:, 0:T])
                hs.append(h0)
            for it in range(8):
                for j in range(nsub):
                    h_nxt = hp.tile([P, T, SLOT], BF, tag=f"h{j}")
                    capsule_iter(hs[j], h_nxt, j)
                    hs[j] = h_nxt
            for d0 in range(0, nsub, 4):
                decoder(hs[d0 : d0 + 4], g, d0)

        if ngroups == 1:
            body(0)
        elif unroll:
            for g in range(ngroups):
                body(g)
        else:
            with tc.For_i(0, ngroups, 1) as g:
                body(g)
        for _pool in (pv, pp, sm, wkd, wk, hp, consts):
            _pool.release()

    nc.compile()
    return nc


def pack_weights(W1, b1, W2, b2, W3, b3, Wd1, bd1, Wd2, bd2, Wo, bo):
    f64 = np.float64
    W1, b1, W2, b2, W3, b3 = (np.asarray(t, f64) for t in (W1, b1, W2, b2, W3, b3))
    G = W1.T @ W2
    a = W2.T @ b1
    c = W1.T @ b2
    d = float(b1 @ b2)

    zu = np.zeros((P, 2, ZSLOT), np.float32)
    full = np.zeros((197, ZSLOT), f64)
    full[:196, :196] = G.T
    full[:196, 196] = a
    full[:196, SLOT:] = W3.T
    full[196, :196] = c
    full[196, 196] = d
    full[196, SLOT:] = b3
    zu[:, 0, :] = full[0:128]
    zu[0:69, 1, :] = full[128:197]

    d1 = np.zeros((P, 8, FEAT), np.float32)
    W1T = np.asarray(Wd1, f64).T  # [784 f_in, 784 j]
    for t in range(T):
        d1[:, t, :] = W1T[t * FV : t * FV + P, :]
        d1[0:68, 4 + t, :] = W1T[t * FV + P : (t + 1) * FV, :]
    d1[68, 4, :] = np.asarray(bd1, f64)

    d2 = np.zeros((P, 7, FEAT), np.float32)
    W2T = np.asarray(Wd2, f64).T
    for cidx in range(6):
        d2[:, cidx, :] = W2T[cidx * P : (cidx + 1) * P, :]
    d2[0:16, 6, :] = W2T[768:784, :]
    d2[16, 6, :] = np.asarray(bd2, f64)

    ow = np.zeros((P, 7, 10), np.float32)
    WoT = np.asarray(Wo, f64).T
    for cidx in range(6):
        ow[:, cidx, :] = WoT[cidx * P : (cidx + 1) * P, :]
    ow[0:16, 6, :] = WoT[768:784, :]
    ow[16, 6, :] = np.asarray(bo, f64)
    return (zu.astype(ml_dtypes.bfloat16), d1.astype(ml_dtypes.bfloat16),
            d2.astype(ml_dtypes.bfloat16), ow.astype(ml_dtypes.bfloat16))


_NC_CACHE = {}


def kernel(**inputs):
    x = np.ascontiguousarray(np.asarray(inputs["x"], np.float32)).astype(
        ml_dtypes.bfloat16
    )
    zu, d1, d2, ow = pack_weights(
        inputs["W1"], inputs["b1"], inputs["W2"], inputs["b2"], inputs["W3"],
        inputs["b3"], inputs["Wd1"], inputs["bd1"], inputs["Wd2"],
        inputs["bd2"], inputs["Wo"], inputs["bo"],
    )
    if "nc" not in _NC_CACHE:
        _NC_CACHE["nc"] = build(8, 4)
    nc = _NC_CACHE["nc"]
    bpc = B // NCORES
    in_maps = [
        {
            "x": x[c * bpc : (c + 1) * bpc],
            "zu_w": zu,
            "dec1_w": d1,
            "dec2_w": d2,
            "out_w": ow,
        }
        for c in range(NCORES)
    ]
    res = run_bass_kernel_spmd(nc, in_maps, core_ids=list(range(NCORES)))
    return np.concatenate([res.results[c]["out"] for c in range(NCORES)], axis=0)


# revision 3
# speedup vs baseline: 1.0049x; 1.0049x over previous
"""Trainium2 Bass kernel for nn_CapsuleNeuralNetworkV2 (8 cores, data-parallel).

Math (per sample, 8 capsule iterations then decoder):
  v = h.reshape(4, 196)
  q = v @ W1.T + b1 ; k = v @ W2.T + b2 ; u = v @ W3.T + b3
  scores[t,s] = q_t . k_s  ->  softmax over s -> h'_t = sum_s P[t,s] u_s
  dec = relu(h Wd1.T + bd1) Wd2.T + bd2 ; out = softmax(dec Wo.T + bo)

Host-side algebra:
  scores[t,s] = v_t . z_s + r_s,  z_s = G v_s + c, r_s = a.v_s + d,
  G = W1.T W2, a = W2.T b1, c = W1.T b2, d = b1.b2; biases fused via an
  augmented ones row so one matmul emits [z | r | u] per slot.

v3 layout/schedule changes vs v2:
  - h slot is 197 wide ([v(196) | 1]); zu slot is 393 ([z | r | u]); the
    dots run over 197 elems and pick up r via the ones column.
  - zu matmuls write two 2-slot PSUM tiles; each is evacuated by ONE Act
    copy (f32->bf16), replacing four per-slot copies.
  - vt transposes write one bf16 PSUM tile; two DVE 2x copies evacuate.
  - dots are 16 STT+accum ops, combine is 4 chains of (seed + 3 MACs);
    both are spread across DVE/Act/Pool by a static table tuned against
    the instruction cost model.
  - probs = e * (1/sum) via one broadcast tensor_tensor.
"""

import numpy as np
import ml_dtypes

import concourse.bass as bass
import concourse.tile as tile
from concourse import bacc, mybir
from concourse.bass import ds
from concourse.bass_utils import run_bass_kernel_spmd
from concourse.masks import make_identity

FR = mybir.dt.float32r
BF = mybir.dt.bfloat16
F32 = mybir.dt.float32
AF = mybir.ActivationFunctionType
ALU = mybir.AluOpType

B = 32768
NCORES = 8
P = 128
T = 4
FV = 196
FEAT = 784
SLOT = FV + 1  # 197: 196 data + ones col
ZSLOT = 2 * FV + 1  # 393: z(196) | r(1) | u(196)

# --- static engine tables (v=DVE, a=Act, p=Pool), tuned vs cost model ---
# dots[t][s] (Act cannot run STT)
DOTS_ENG = [
    "vp..",
    "pp..",
    "pp..",
    "pp..",
]
# combine: SEED_ENG[t] + MAC_ENG[t] (3 chained MACs; MACs only v or p)
# combine mul engine per (t, s); adds are two batched DVE tensor_tensor
MUL_ENG = [
    "aapp",
    "aapp",
    "aapp",
    "appp",
]
# vt-evac engines for (chunk1, chunk2)
VT_ENG = "va"
# s-slots computed via a DVE products TT + 4 cheap TSP-accums
PROD_S = (2, 3)


def _ap(t, dims, offset_elems=0):
    a = t[:] if hasattr(t, "tile") or not isinstance(t, bass.AP) else t
    return bass.AP(tensor=a.tensor, offset=a.offset + offset_elems, ap=dims)


def build(nsub=8, ngroups=4, unroll=False, zu_bufs=1, vtps_bufs=2, mpd_bufs=2,
          h_bufs=3, wk_bufs=5, sm_bufs=8, zu_mode="half", hf_bufs=1):
    """One NeuronCore program processing nsub*ngroups*128 samples."""
    bpc = nsub * ngroups * P
    nc = bacc.Bacc("TRN2", target_bir_lowering=False, debug=False)

    x_d = nc.dram_tensor("x", [bpc, FEAT], BF, kind="ExternalInput")
    zu_d = nc.dram_tensor("zu_w", [P, 2, ZSLOT], BF, kind="ExternalInput")
    d1_d = nc.dram_tensor("dec1_w", [P, 8, FEAT], BF, kind="ExternalInput")
    d2_d = nc.dram_tensor("dec2_w", [P, 7, FEAT], BF, kind="ExternalInput")
    ow_d = nc.dram_tensor("out_w", [P, 7, 10], BF, kind="ExternalInput")
    out_d = nc.dram_tensor("out", [bpc, 10], F32, kind="ExternalOutput")

    with tile.TileContext(nc) as tc:
        consts = tc.alloc_tile_pool(name="consts", bufs=1)
        hp = tc.alloc_tile_pool(name="h", bufs=h_bufs)
        wk = tc.alloc_tile_pool(name="wk", bufs=wk_bufs)
        wkd = tc.alloc_tile_pool(name="wkd", bufs=1)
        sm = tc.alloc_tile_pool(name="small", bufs=sm_bufs)
        pp = tc.alloc_tile_pool(name="ps", bufs=zu_bufs, space="PSUM")
        pv = tc.alloc_tile_pool(name="pv", bufs=vtps_bufs, space="PSUM")

        ident_f = consts.tile([P, P], F32)
        make_identity(nc, ident_f)
        ident = consts.tile([P, P], FR)
        nc.vector.tensor_copy(ident, ident_f)
        ident_b = consts.tile([P, P], BF)
        nc.vector.tensor_copy(ident_b, ident_f)
        ones_c = consts.tile([P, 512], F32)
        nc.vector.memset(ones_c, 1.0)
        zu_w = consts.tile([P, 2, ZSLOT], BF)
        nc.sync.dma_start(out=zu_w, in_=zu_d[:, :, :])
        d1_w = consts.tile([P, 8, FEAT], BF)
        nc.sync.dma_start(out=d1_w, in_=d1_d[:, :, :])
        d2_w = consts.tile([P, 7, FEAT], BF)
        nc.sync.dma_start(out=d2_w, in_=d2_d[:, :, :])
        ow_w = consts.tile([P, 7, 10], BF)
        nc.sync.dma_start(out=ow_w, in_=ow_d[:, :, :])

        def eng(c):
            return {"v": nc.vector, "p": nc.gpsimd, "a": nc.scalar}[c]

        def capsule_iter(h_cur, h_nxt, j):
            """One capsule-attention iteration: h_nxt <- attn(h_cur)."""
            # PE transposes: batch-major h -> feature-major V.T chunks
            vt_ps = pv.tile([P, T, 2, P], BF, tag="vtps")
            for t in range(T):
                nc.tensor.transpose(vt_ps[:, t, 0, :], h_cur[:, t, 0:P], ident_b)
                # includes the ones column -> row 68 is 1.0
                nc.tensor.transpose(vt_ps[0:69, t, 1, :],
                                    h_cur[:, t, P : P + 69], ident_b)
            vt = wk.tile([P, T, 2, P], BF, tag="vt")
            for ci, (dst, src) in enumerate(
                [(vt[:, :, 0, :], vt_ps[:, :, 0, :]),
                 (vt[0:69, :, 1, :], vt_ps[0:69, :, 1, :])]
            ):
                c = VT_ENG[ci]
                if c == "a":
                    nc.scalar.copy(dst, src)
                elif c == "v":
                    nc.vector.tensor_copy(dst, src)
                else:
                    nc.gpsimd.tensor_copy(dst, src)

            # z|r|u fused matmuls + Act evacuation
            zu = wk.tile([P, T, ZSLOT], BF, tag="zu")
            if zu_mode == "half":
                for half in range(2):
                    # 512-wide slots keep each matmul within one PSUM bank
                    zu_ps = pp.tile([P, 2, 512], F32, tag=f"zu{half}")
                    for k in range(2):
                        s = 2 * half + k
                        nc.tensor.matmul(zu_ps[:, k, 0:ZSLOT], vt[:, s, 0, :],
                                         zu_w[:, 0, :], start=True, stop=False)
                        nc.tensor.matmul(zu_ps[:, k, 0:ZSLOT], vt[0:69, s, 1, :],
                                         zu_w[0:69, 1, :], start=False, stop=True)
                    nc.scalar.copy(zu[:, 2 * half : 2 * half + 2, :],
                                   zu_ps[:, :, 0:ZSLOT])
            else:  # per-slot PSUM tiles, finer cross-tile pipelining
                for s in range(T):
                    zu_ps = pp.tile([P, ZSLOT], F32, tag=f"zs{s}")
                    nc.tensor.matmul(zu_ps, vt[:, s, 0, :],
                                     zu_w[:, 0, :], start=True, stop=False)
                    nc.tensor.matmul(zu_ps, vt[0:69, s, 1, :],
                                     zu_w[0:69, 1, :], start=False, stop=True)
                    nc.scalar.copy(zu[:, s, :], zu_ps)

            # dots: per-half DVE products TT (starts right after that
            # half's evacuation) + 16 cheap TSP accums (4x mode) on DVE.
            dots = sm.tile([P, T, T], F32, tag="dots")
            scr = sm.tile([P, 3, SLOT], BF, tag="scr", bufs=8)
            for half in range(2):
                prod = sm.tile([P, 2, T, SLOT], BF, tag=f"prod{half}", bufs=2)
                hin = _ap(h_cur, [h_cur[:].ap[0], [0, 2], [SLOT, T], [1, SLOT]])
                zin = _ap(zu, [zu[:].ap[0], [ZSLOT, 2], [0, T], [1, SLOT]],
                          offset_elems=half * 2 * ZSLOT)
                nc.vector.tensor_tensor(out=prod, in0=hin, in1=zin, op=ALU.mult)
                for k in range(2):
                    s = 2 * half + k
                    for t in range(T):
                        nc.vector.tensor_scalar(
                            out=scr[:, 0, :],
                            in0=prod[:, k, t, :], scalar1=1.0,
                            scalar2=0.0, op0=ALU.mult, op1=ALU.add,
                            accum_out=dots[:, t, s : s + 1])

            # softmax over s (no max subtraction; |scores| < 30); exp is
            # split per dots-half so it overlaps the second half's dots
            e_t = sm.tile([P, T, T], F32, tag="e")
            nc.scalar.activation(e_t[:, :, 0:2], dots[:, :, 0:2], AF.Exp)
            nc.scalar.activation(e_t[:, :, 2:4], dots[:, :, 2:4], AF.Exp)
            sums = sm.tile([P, T], F32, tag="sums")
            nc.vector.reduce_sum(sums, e_t, axis=mybir.AxisListType.X)
            rec = sm.tile([P, T], F32, tag="rec")
            nc.vector.reciprocal(rec, sums)
            probs = sm.tile([P, T, T], F32, tag="probs")
            nc.vector.tensor_tensor(
                out=probs, in0=e_t,
                in1=_ap(rec, [rec[:].ap[0], [1, T], [0, T]]),
                op=ALU.mult,
            )

            # ones column for the next h
            nc.gpsimd.tensor_copy(h_nxt[:, :, FV:SLOT], ones_c[:, 0:T])
            # combine: pu[t,s] = P[t,s] * u_s on DVE/Act/Pool, then two
            # batched DVE adds: h'_t = (pu[t,0]+pu[t,1]) + (pu[t,2]+pu[t,3])
            pu = sm.tile([P, T, T, FV], BF, tag="pu", bufs=2)
            for t in range(T):
                for s in range(T):
                    c = MUL_ENG[t][s]
                    if c == "a":
                        nc.scalar.activation(
                            pu[:, t, s, :], zu[:, s, SLOT:ZSLOT], AF.Copy,
                            scale=probs[:, t, s : s + 1])
                    elif c == "v":
                        nc.vector.tensor_scalar_mul(
                            pu[:, t, s, :], zu[:, s, SLOT:ZSLOT],
                            probs[:, t, s : s + 1])
                    else:
                        nc.gpsimd.tensor_scalar_mul(
                            pu[:, t, s, :], zu[:, s, SLOT:ZSLOT],
                            probs[:, t, s : s + 1])
            q = sm.tile([P, T, 2, FV], BF, tag="q", bufs=2)
            ev = _ap(pu, [pu[:].ap[0], [T * FV, T], [2 * FV, 2], [1, FV]])
            od = _ap(pu, [pu[:].ap[0], [T * FV, T], [2 * FV, 2], [1, FV]],
                     offset_elems=FV)
            nc.vector.tensor_tensor(out=q, in0=ev, in1=od, op=ALU.add)
            nc.vector.tensor_tensor(out=h_nxt[:, :, 0:FV], in0=q[:, :, 0, :],
                                    in1=q[:, :, 1, :], op=ALU.add)

        def decoder(hs, g, goff=0):
            """Decoder over a chunk of <=4 tiles (N = len(hs)*128 wide)."""
            W = len(hs) * P
            # h.T chunks, slot-major: [128] x4 and [69] x4 (with ones row)
            ht1 = wkd.tile([P, T, W], BF, tag="ht1")
            ht2 = wkd.tile([69, T, W], BF, tag="ht2")
            for t in range(T):
                t1_ps = pv.tile([P, W], BF, tag="vtps")
                t2_ps = pv.tile([69, W], BF, tag="vtps")
                for j in range(len(hs)):
                    nc.tensor.transpose(
                        t1_ps[:, j * P : (j + 1) * P], hs[j][:, t, 0:P], ident_b
                    )
                    nc.tensor.transpose(
                        t2_ps[:, j * P : (j + 1) * P],
                        hs[j][:, t, P : P + 69], ident_b
                    )
                nc.scalar.copy(ht1[:, t, :], t1_ps)
                nc.vector.tensor_copy(ht2[:, t, :], t2_ps)

            # dec1 = relu(Wd1 @ h.T + bd1), feature-major, 7 M-chunks
            d1a = wkd.tile([P, 6, W], BF, tag="d1a")
            d1b = wkd.tile([17, W], BF, tag="d1b")
            nc.vector.tensor_copy(d1b, ones_c[0:17, 0:W])
            for m in range(7):
                mw = min(P, FEAT - m * P)
                mp = pv.tile([P, W], F32, tag="mpd", bufs=mpd_bufs)
                msl = slice(m * P, m * P + mw)
                for t in range(T):
                    nc.tensor.matmul(mp[0:mw, :], d1_w[:, t, msl], ht1[:, t, :],
                                     start=(t == 0), stop=False)
                for t in range(T):
                    nc.tensor.matmul(mp[0:mw, :], d1_w[0:69, 4 + t, msl],
                                     ht2[:, t, :], start=False, stop=(t == 3))
                if m < 6:
                    nc.scalar.activation(d1a[:, m, :], mp, AF.Relu)
                else:
                    nc.scalar.activation(d1b[0:16, :], mp[0:16, :], AF.Relu)

            # dec2 = Wd2 @ relu1 + bd2, feature-major
            d2a = wkd.tile([P, 6, W], BF, tag="d2a")
            d2b = wkd.tile([17, W], BF, tag="d2b")
            nc.vector.tensor_copy(d2b, ones_c[0:17, 0:W])
            for m in range(7):
                mw = min(P, FEAT - m * P)
                mp = pv.tile([P, W], F32, tag="mpd", bufs=mpd_bufs)
                msl = slice(m * P, m * P + mw)
                for c in range(6):
                    nc.tensor.matmul(mp[0:mw, :], d2_w[:, c, msl], d1a[:, c, :],
                                     start=(c == 0), stop=False)
                nc.tensor.matmul(mp[0:mw, :], d2_w[0:17, 6, msl], d1b,
                                 start=False, stop=True)
                if m < 6:
                    nc.vector.tensor_copy(d2a[:, m, :], mp)
                else:
                    nc.vector.tensor_copy(d2b[0:16, :], mp[0:16, :])

            # logits + softmax per subtile
            for j in range(len(hs)):
                jsl = slice(j * P, (j + 1) * P)
                lg = pv.tile([P, 10], F32, tag="mpd", bufs=mpd_bufs)
                for c in range(6):
                    nc.tensor.matmul(lg, d2a[:, c, jsl], ow_w[:, c, :],
                                     start=(c == 0), stop=False)
                nc.tensor.matmul(lg, d2b[:, jsl], ow_w[0:17, 6, :],
                                 start=False, stop=True)
                mx = sm.tile([P, 1], F32, tag="mx")
                nc.vector.reduce_max(mx, lg, axis=mybir.AxisListType.X)
                nmx = sm.tile([P, 1], F32, tag="nmx")
                nc.vector.tensor_scalar_mul(nmx, mx, -1.0)
                e10 = sm.tile([P, 10], F32, tag="e10")
                s10 = sm.tile([P, 1], F32, tag="s10")
                nc.scalar.activation(e10, lg, AF.Exp, bias=nmx, accum_out=s10)
                r10 = sm.tile([P, 1], F32, tag="r10")
                nc.vector.reciprocal(r10, s10)
                o10 = sm.tile([P, 10], F32, tag="o10")
                nc.vector.tensor_scalar_mul(o10, e10, r10)
                nc.sync.dma_start(
                    out=out_d[ds(g * (nsub * P) + (goff + j) * P, P), :],
                    in_=o10,
                )

        def body(g):
            hs = []
            for j in range(nsub):
                h0 = hp.tile([P, T, SLOT], BF, tag=f"h{j}")
                nc.sync.dma_start(
                    out=h0[:, :, 0:FV],
                    in_=x_d[ds(g * (nsub * P) + j * P, P), :].rearrange(
                        "p (t f) -> p t f", t=T
                    ),
                )
                nc.gpsimd.tensor_copy(h0[:, :, FV:SLOT], ones_c[:, 0:T])
                hs.append(h0)
            for it in range(8):
                for j in range(nsub):
                    h_nxt = hp.tile([P, T, SLOT], BF, tag=f"h{j}")
                    capsule_iter(hs[j], h_nxt, j)
                    hs[j] = h_nxt
            for d0 in range(0, nsub, 4):
                decoder(hs[d0 : d0 + 4], g, d0)

        if ngroups == 1:
            body(0)
        elif unroll:
            for g in range(ngroups):
                body(g)
        else:
            with tc.For_i(0, ngroups, 1) as g:
                body(g)
        for _pool in (pv, pp, sm, wkd, wk, hp, consts):
            _pool.release()

    nc.compile()
    return nc


def pack_weights(W1, b1, W2, b2, W3, b3, Wd1, bd1, Wd2, bd2, Wo, bo):
    f64 = np.float64
    W1, b1, W2, b2, W3, b3 = (np.asarray(t, f64) for t in (W1, b1, W2, b2, W3, b3))
    G = W1.T @ W2
    a = W2.T @ b1
    c = W1.T @ b2
    d = float(b1 @ b2)

    zu = np.zeros((P, 2, ZSLOT), np.float32)
    full = np.zeros((197, ZSLOT), f64)
    full[:196, :196] = G.T
    full[:196, 196] = a
    full[:196, SLOT:] = W3.T
    full[196, :196] = c
    full[196, 196] = d
    full[196, SLOT:] = b3
    zu[:, 0, :] = full[0:128]
    zu[0:69, 1, :] = full[128:197]

    d1 = np.zeros((P, 8, FEAT), np.float32)
    W1T = np.asarray(Wd1, f64).T  # [784 f_in, 784 j]
    for t in range(T):
        d1[:, t, :] = W1T[t * FV : t * FV + P, :]
        d1[0:68, 4 + t, :] = W1T[t * FV + P : (t + 1) * FV, :]
    d1[68, 4, :] = np.asarray(bd1, f64)

    d2 = np.zeros((P, 7, FEAT), np.float32)
    W2T = np.asarray(Wd2, f64).T
    for cidx in range(6):
        d2[:, cidx, :] = W2T[cidx * P : (cidx + 1) * P, :]
    d2[0:16, 6, :] = W2T[768:784, :]
    d2[16, 6, :] = np.asarray(bd2, f64)

    ow = np.zeros((P, 7, 10), np.float32)
    WoT = np.asarray(Wo, f64).T
    for cidx in range(6):
        ow[:, cidx, :] = WoT[cidx * P : (cidx + 1) * P, :]
    ow[0:16, 6, :] = WoT[768:784, :]
    ow[16, 6, :] = np.asarray(bo, f64)
    return (zu.astype(ml_dtypes.bfloat16), d1.astype(ml_dtypes.bfloat16),
            d2.astype(ml_dtypes.bfloat16), ow.astype(ml_dtypes.bfloat16))


_NC_CACHE = {}


def kernel(**inputs):
    x = np.ascontiguousarray(np.asarray(inputs["x"], np.float32)).astype(
        ml_dtypes.bfloat16
    )
    zu, d1, d2, ow = pack_weights(
        inputs["W1"], inputs["b1"], inputs["W2"], inputs["b2"], inputs["W3"],
        inputs["b3"], inputs["Wd1"], inputs["bd1"], inputs["Wd2"],
        inputs["bd2"], inputs["Wo"], inputs["bo"],
    )
    if "nc" not in _NC_CACHE:
        _NC_CACHE["nc"] = build(8, 4)
    nc = _NC_CACHE["nc"]
    bpc = B // NCORES
    in_maps = [
        {
            "x": x[c * bpc : (c + 1) * bpc],
            "zu_w": zu,
            "dec1_w": d1,
            "dec2_w": d2,
            "out_w": ow,
        }
        for c in range(NCORES)
    ]
    res = run_bass_kernel_spmd(nc, in_maps, core_ids=list(range(NCORES)))
    return np.concatenate([res.results[c]["out"] for c in range(NCORES)], axis=0)


# revision 4
# speedup vs baseline: 1.0053x; 1.0004x over previous
"""Trainium2 Bass kernel for nn_CapsuleNeuralNetworkV2 (8 cores, data-parallel).

Math (per sample, 8 capsule iterations then decoder):
  v = h.reshape(4, 196)
  q = v @ W1.T + b1 ; k = v @ W2.T + b2 ; u = v @ W3.T + b3
  scores[t,s] = q_t . k_s  ->  softmax over s -> h'_t = sum_s P[t,s] u_s
  dec = relu(h Wd1.T + bd1) Wd2.T + bd2 ; out = softmax(dec Wo.T + bo)

Host-side algebra:
  scores[t,s] = v_t . z_s + r_s,  z_s = G v_s + c, r_s = a.v_s + d,
  G = W1.T W2, a = W2.T b1, c = W1.T b2, d = b1.b2; biases fused via an
  augmented ones row so one matmul emits [z | r | u] per slot.

v3 layout/schedule changes vs v2:
  - h slot is 197 wide ([v(196) | 1]); zu slot is 393 ([z | r | u]); the
    dots run over 197 elems and pick up r via the ones column.
  - zu matmuls write two 2-slot PSUM tiles; each is evacuated by ONE Act
    copy (f32->bf16), replacing four per-slot copies.
  - vt transposes write one bf16 PSUM tile; two DVE 2x copies evacuate.
  - dots are 16 STT+accum ops, combine is 4 chains of (seed + 3 MACs);
    both are spread across DVE/Act/Pool by a static table tuned against
    the instruction cost model.
  - probs = e * (1/sum) via one broadcast tensor_tensor.
"""

import numpy as np
import ml_dtypes

import concourse.bass as bass
import concourse.tile as tile
from concourse import bacc, mybir
from concourse.bass import ds
from concourse.bass_utils import run_bass_kernel_spmd
from concourse.masks import make_identity

FR = mybir.dt.float32r
BF = mybir.dt.bfloat16
F32 = mybir.dt.float32
AF = mybir.ActivationFunctionType
ALU = mybir.AluOpType

B = 32768
NCORES = 8
P = 128
T = 4
FV = 196
FEAT = 784
SLOT = FV + 1  # 197: 196 data + ones col
ZSLOT = 2 * FV + 1  # 393: z(196) | r(1) | u(196)

# --- static engine tables (v=DVE, a=Act, p=Pool), tuned vs cost model ---
# dots[t][s] (Act cannot run STT)
DOTS_ENG = [
    "vp..",
    "pp..",
    "pp..",
    "pp..",
]
# combine: SEED_ENG[t] + MAC_ENG[t] (3 chained MACs; MACs only v or p)
# combine mul engine per (t, s); adds are two batched DVE tensor_tensor
MUL_ENG = [
    "aapp",
    "aapp",
    "aapp",
    "appp",
]
# vt-evac engines for (chunk1, chunk2)
VT_ENG = "va"
# s-slots computed via a DVE products TT + 4 cheap TSP-accums
PROD_S = (2, 3)
ADDS_PER_T = False
ZU0_B2 = False


def _ap(t, dims, offset_elems=0):
    a = t[:] if hasattr(t, "tile") or not isinstance(t, bass.AP) else t
    return bass.AP(tensor=a.tensor, offset=a.offset + offset_elems, ap=dims)


def build(nsub=8, ngroups=4, unroll=False, zu_bufs=1, vtps_bufs=2, mpd_bufs=2,
          h_bufs=3, wk_bufs=5, sm_bufs=8, zu_mode="half", hf_bufs=1):
    """One NeuronCore program processing nsub*ngroups*128 samples."""
    bpc = nsub * ngroups * P
    nc = bacc.Bacc("TRN2", target_bir_lowering=False, debug=False)

    x_d = nc.dram_tensor("x", [bpc, FEAT], BF, kind="ExternalInput")
    zu_d = nc.dram_tensor("zu_w", [P, 2, ZSLOT], BF, kind="ExternalInput")
    d1_d = nc.dram_tensor("dec1_w", [P, 8, FEAT], BF, kind="ExternalInput")
    d2_d = nc.dram_tensor("dec2_w", [P, 7, FEAT], BF, kind="ExternalInput")
    ow_d = nc.dram_tensor("out_w", [P, 7, 10], BF, kind="ExternalInput")
    out_d = nc.dram_tensor("out", [bpc, 10], F32, kind="ExternalOutput")

    with tile.TileContext(nc) as tc:
        consts = tc.alloc_tile_pool(name="consts", bufs=1)
        hp = tc.alloc_tile_pool(name="h", bufs=h_bufs)
        wk = tc.alloc_tile_pool(name="wk", bufs=wk_bufs)
        wkd = tc.alloc_tile_pool(name="wkd", bufs=1)
        sm = tc.alloc_tile_pool(name="small", bufs=sm_bufs)
        pp = tc.alloc_tile_pool(name="ps", bufs=zu_bufs, space="PSUM")
        pv = tc.alloc_tile_pool(name="pv", bufs=vtps_bufs, space="PSUM")

        ident_f = consts.tile([P, P], F32)
        make_identity(nc, ident_f)
        ident = consts.tile([P, P], FR)
        nc.vector.tensor_copy(ident, ident_f)
        ident_b = consts.tile([P, P], BF)
        nc.vector.tensor_copy(ident_b, ident_f)
        ones_c = consts.tile([P, 512], F32)
        nc.vector.memset(ones_c, 1.0)
        zu_w = consts.tile([P, 2, ZSLOT], BF)
        nc.sync.dma_start(out=zu_w, in_=zu_d[:, :, :])
        d1_w = consts.tile([P, 8, FEAT], BF)
        nc.sync.dma_start(out=d1_w, in_=d1_d[:, :, :])
        d2_w = consts.tile([P, 7, FEAT], BF)
        nc.sync.dma_start(out=d2_w, in_=d2_d[:, :, :])
        ow_w = consts.tile([P, 7, 10], BF)
        nc.sync.dma_start(out=ow_w, in_=ow_d[:, :, :])

        def eng(c):
            return {"v": nc.vector, "p": nc.gpsimd, "a": nc.scalar}[c]

        def capsule_iter(h_cur, h_nxt, j):
            """One capsule-attention iteration: h_nxt <- attn(h_cur)."""
            # PE transposes: batch-major h -> feature-major V.T chunks
            vt_ps = pv.tile([P, T, 2, P], BF, tag="vtps")
            for t in range(T):
                nc.tensor.transpose(vt_ps[:, t, 0, :], h_cur[:, t, 0:P], ident_b)
                # includes the ones column -> row 68 is 1.0
                nc.tensor.transpose(vt_ps[0:69, t, 1, :],
                                    h_cur[:, t, P : P + 69], ident_b)
            vt = wk.tile([P, T, 2, P], BF, tag="vt")
            for ci, (dst, src) in enumerate(
                [(vt[:, :, 0, :], vt_ps[:, :, 0, :]),
                 (vt[0:69, :, 1, :], vt_ps[0:69, :, 1, :])]
            ):
                c = VT_ENG[ci]
                if c == "a":
                    nc.scalar.copy(dst, src)
                elif c == "v":
                    nc.vector.tensor_copy(dst, src)
                else:
                    nc.gpsimd.tensor_copy(dst, src)

            # z|r|u fused matmuls + Act evacuation
            zu = wk.tile([P, T, ZSLOT], BF, tag="zu")
            if zu_mode == "half":
                for half in range(2):
                    # 512-wide slots keep each matmul within one PSUM bank
                    zu_ps = pp.tile([P, 2, 512], F32, tag=f"zu{half}",
                                    bufs=(2 if (half == 0 and ZU0_B2) else 1))
                    for k in range(2):
                        s = 2 * half + k
                        nc.tensor.matmul(zu_ps[:, k, 0:ZSLOT], vt[:, s, 0, :],
                                         zu_w[:, 0, :], start=True, stop=False)
                        nc.tensor.matmul(zu_ps[:, k, 0:ZSLOT], vt[0:69, s, 1, :],
                                         zu_w[0:69, 1, :], start=False, stop=True)
                    nc.scalar.copy(zu[:, 2 * half : 2 * half + 2, :],
                                   zu_ps[:, :, 0:ZSLOT])
            else:  # per-slot PSUM tiles, finer cross-tile pipelining
                for s in range(T):
                    zu_ps = pp.tile([P, ZSLOT], F32, tag=f"zs{s}")
                    nc.tensor.matmul(zu_ps, vt[:, s, 0, :],
                                     zu_w[:, 0, :], start=True, stop=False)
                    nc.tensor.matmul(zu_ps, vt[0:69, s, 1, :],
                                     zu_w[0:69, 1, :], start=False, stop=True)
                    nc.scalar.copy(zu[:, s, :], zu_ps)

            # dots: per-half DVE products TT (starts right after that
            # half's evacuation) + 16 cheap TSP accums (4x mode) on DVE.
            dots = sm.tile([P, T, T], F32, tag="dots")
            scr = sm.tile([P, 3, SLOT], BF, tag="scr", bufs=8)
            for half in range(2):
                prod = sm.tile([P, 2, T, SLOT], BF, tag=f"prod{half}", bufs=2)
                hin = _ap(h_cur, [h_cur[:].ap[0], [0, 2], [SLOT, T], [1, SLOT]])
                zin = _ap(zu, [zu[:].ap[0], [ZSLOT, 2], [0, T], [1, SLOT]],
                          offset_elems=half * 2 * ZSLOT)
                nc.vector.tensor_tensor(out=prod, in0=hin, in1=zin, op=ALU.mult)
                for k in range(2):
                    s = 2 * half + k
                    for t in range(T):
                        nc.vector.tensor_scalar(
                            out=scr[:, 0, :],
                            in0=prod[:, k, t, :], scalar1=1.0,
                            scalar2=0.0, op0=ALU.mult, op1=ALU.add,
                            accum_out=dots[:, t, s : s + 1])

            # softmax over s (no max subtraction; |scores| < 30); exp is
            # split per dots-half so it overlaps the second half's dots
            e_t = sm.tile([P, T, T], F32, tag="e")
            nc.scalar.activation(e_t[:, :, 0:2], dots[:, :, 0:2], AF.Exp)
            nc.scalar.activation(e_t[:, :, 2:4], dots[:, :, 2:4], AF.Exp)
            sums = sm.tile([P, T], F32, tag="sums")
            nc.vector.reduce_sum(sums, e_t, axis=mybir.AxisListType.X)
            rec = sm.tile([P, T], F32, tag="rec")
            nc.vector.reciprocal(rec, sums)
            probs = sm.tile([P, T, T], F32, tag="probs")
            nc.vector.tensor_tensor(
                out=probs, in0=e_t,
                in1=_ap(rec, [rec[:].ap[0], [1, T], [0, T]]),
                op=ALU.mult,
            )

            # ones column for the next h
            nc.gpsimd.tensor_copy(h_nxt[:, :, FV:SLOT], ones_c[:, 0:T])
            # combine: pu[t,s] = P[t,s] * u_s on DVE/Act/Pool, then two
            # batched DVE adds: h'_t = (pu[t,0]+pu[t,1]) + (pu[t,2]+pu[t,3])
            pu = sm.tile([P, T, T, FV], BF, tag="pu", bufs=2)
            for t in range(T):
                for s in range(T):
                    c = MUL_ENG[t][s]
                    if c == "a":
                        nc.scalar.activation(
                            pu[:, t, s, :], zu[:, s, SLOT:ZSLOT], AF.Copy,
                            scale=probs[:, t, s : s + 1])
                    elif c == "v":
                        nc.vector.tensor_scalar_mul(
                            pu[:, t, s, :], zu[:, s, SLOT:ZSLOT],
                            probs[:, t, s : s + 1])
                    else:
                        nc.gpsimd.tensor_scalar_mul(
                            pu[:, t, s, :], zu[:, s, SLOT:ZSLOT],
                            probs[:, t, s : s + 1])
            if ADDS_PER_T:
                q = sm.tile([P, T, 2, FV], BF, tag="q", bufs=2)
                for t in range(T):
                    nc.vector.tensor_tensor(
                        out=q[:, t, :, :],
                        in0=_ap(pu, [pu[:].ap[0], [2 * FV, 2], [1, FV]],
                                offset_elems=t * T * FV),
                        in1=_ap(pu, [pu[:].ap[0], [2 * FV, 2], [1, FV]],
                                offset_elems=t * T * FV + FV),
                        op=ALU.add)
                    nc.vector.tensor_tensor(
                        out=h_nxt[:, t, 0:FV], in0=q[:, t, 0, :],
                        in1=q[:, t, 1, :], op=ALU.add)
            else:
                q = sm.tile([P, T, 2, FV], BF, tag="q", bufs=2)
                ev = _ap(pu, [pu[:].ap[0], [T * FV, T], [2 * FV, 2], [1, FV]])
                od = _ap(pu, [pu[:].ap[0], [T * FV, T], [2 * FV, 2], [1, FV]],
                         offset_elems=FV)
                nc.vector.tensor_tensor(out=q, in0=ev, in1=od, op=ALU.add)
                nc.vector.tensor_tensor(out=h_nxt[:, :, 0:FV], in0=q[:, :, 0, :],
                                        in1=q[:, :, 1, :], op=ALU.add)

        def decoder(hs, g, goff=0):
            """Decoder over a chunk of <=4 tiles (N = len(hs)*128 wide)."""
            W = len(hs) * P
            # h.T chunks, slot-major: [128] x4 and [69] x4 (with ones row)
            ht1 = wkd.tile([P, T, W], BF, tag="ht1")
            ht2 = wkd.tile([69, T, W], BF, tag="ht2")
            for t in range(T):
                t1_ps = pv.tile([P, W], BF, tag="vtps")
                t2_ps = pv.tile([69, W], BF, tag="vtps")
                for j in range(len(hs)):
                    nc.tensor.transpose(
                        t1_ps[:, j * P : (j + 1) * P], hs[j][:, t, 0:P], ident_b
                    )
                    nc.tensor.transpose(
                        t2_ps[:, j * P : (j + 1) * P],
                        hs[j][:, t, P : P + 69], ident_b
                    )
                nc.scalar.copy(ht1[:, t, :], t1_ps)
                nc.vector.tensor_copy(ht2[:, t, :], t2_ps)

            # dec1 = relu(Wd1 @ h.T + bd1), feature-major, 7 M-chunks
            d1a = wkd.tile([P, 6, W], BF, tag="d1a")
            d1b = wkd.tile([17, W], BF, tag="d1b")
            nc.vector.tensor_copy(d1b, ones_c[0:17, 0:W])
            for m in range(7):
                mw = min(P, FEAT - m * P)
                mp = pv.tile([P, W], F32, tag="mpd", bufs=mpd_bufs)
                msl = slice(m * P, m * P + mw)
                for t in range(T):
                    nc.tensor.matmul(mp[0:mw, :], d1_w[:, t, msl], ht1[:, t, :],
                                     start=(t == 0), stop=False)
                for t in range(T):
                    nc.tensor.matmul(mp[0:mw, :], d1_w[0:69, 4 + t, msl],
                                     ht2[:, t, :], start=False, stop=(t == 3))
                if m < 6:
                    nc.scalar.activation(d1a[:, m, :], mp, AF.Relu)
                else:
                    nc.scalar.activation(d1b[0:16, :], mp[0:16, :], AF.Relu)

            # dec2 = Wd2 @ relu1 + bd2, feature-major
            d2a = wkd.tile([P, 6, W], BF, tag="d2a")
            d2b = wkd.tile([17, W], BF, tag="d2b")
            nc.vector.tensor_copy(d2b, ones_c[0:17, 0:W])
            for m in range(7):
                mw = min(P, FEAT - m * P)
                mp = pv.tile([P, W], F32, tag="mpd", bufs=mpd_bufs)
                msl = slice(m * P, m * P + mw)
                for c in range(6):
                    nc.tensor.matmul(mp[0:mw, :], d2_w[:, c, msl], d1a[:, c, :],
                                     start=(c == 0), stop=False)
                nc.tensor.matmul(mp[0:mw, :], d2_w[0:17, 6, msl], d1b,
                                 start=False, stop=True)
                if m < 6:
                    nc.vector.tensor_copy(d2a[:, m, :], mp)
                else:
                    nc.vector.tensor_copy(d2b[0:16, :], mp[0:16, :])

            # logits + softmax per subtile
            for j in range(len(hs)):
                jsl = slice(j * P, (j + 1) * P)
                lg = pv.tile([P, 10], F32, tag="mpd", bufs=mpd_bufs)
                for c in range(6):
                    nc.tensor.matmul(lg, d2a[:, c, jsl], ow_w[:, c, :],
                                     start=(c == 0), stop=False)
                nc.tensor.matmul(lg, d2b[:, jsl], ow_w[0:17, 6, :],
                                 start=False, stop=True)
                mx = sm.tile([P, 1], F32, tag="mx")
                nc.vector.reduce_max(mx, lg, axis=mybir.AxisListType.X)
                nmx = sm.tile([P, 1], F32, tag="nmx")
                nc.vector.tensor_scalar_mul(nmx, mx, -1.0)
                e10 = sm.tile([P, 10], F32, tag="e10")
                s10 = sm.tile([P, 1], F32, tag="s10")
                nc.scalar.activation(e10, lg, AF.Exp, bias=nmx, accum_out=s10)
                r10 = sm.tile([P, 1], F32, tag="r10")
                nc.vector.reciprocal(r10, s10)
                o10 = sm.tile([P, 10], F32, tag="o10")
                nc.vector.tensor_scalar_mul(o10, e10, r10)
                nc.sync.dma_start(
                    out=out_d[ds(g * (nsub * P) + (goff + j) * P, P), :],
                    in_=o10,
                )

        def body_capsule(g):
            hs = []
            for j in range(nsub):
                h0 = hp.tile([P, T, SLOT], BF, tag=f"h{j}")
                nc.sync.dma_start(
                    out=h0[:, :, 0:FV],
                    in_=x_d[ds(g * (nsub * P) + j * P, P), :].rearrange(
                        "p (t f) -> p t f", t=T
                    ),
                )
                nc.gpsimd.tensor_copy(h0[:, :, FV:SLOT], ones_c[:, 0:T])
                hs.append(h0)
            for it in range(8):
                for j in range(nsub):
                    if it < 7:
                        h_nxt = hp.tile([P, T, SLOT], BF, tag=f"h{j}")
                    else:
                        h_nxt = hp.tile([P, T, SLOT], BF, tag=f"hold{j}",
                                        bufs=2)
                    capsule_iter(hs[j], h_nxt, j)
                    hs[j] = h_nxt
            return hs

        def body(g):
            hs = body_capsule(g)
            for d0 in range(0, nsub, 4):
                decoder(hs[d0 : d0 + 4], g, d0)

        if ngroups == 1:
            body(0)
        elif unroll:
            # software-pipelined: decode group g-1 while computing group g
            holds = [body_capsule(0)]
            for g in range(1, ngroups):
                holds.append(body_capsule(g))
                for d0 in range(0, nsub, 4):
                    decoder(holds[g - 1][d0 : d0 + 4], g - 1, d0)
            for d0 in range(0, nsub, 4):
                decoder(holds[-1][d0 : d0 + 4], ngroups - 1, d0)
        else:
            with tc.For_i(0, ngroups, 1) as g:
                body(g)
        for _pool in (pv, pp, sm, wkd, wk, hp, consts):
            _pool.release()

    nc.compile()
    return nc


def pack_weights(W1, b1, W2, b2, W3, b3, Wd1, bd1, Wd2, bd2, Wo, bo):
    f64 = np.float64
    W1, b1, W2, b2, W3, b3 = (np.asarray(t, f64) for t in (W1, b1, W2, b2, W3, b3))
    G = W1.T @ W2
    a = W2.T @ b1
    c = W1.T @ b2
    d = float(b1 @ b2)

    zu = np.zeros((P, 2, ZSLOT), np.float32)
    full = np.zeros((197, ZSLOT), f64)
    full[:196, :196] = G.T
    full[:196, 196] = a
    full[:196, SLOT:] = W3.T
    full[196, :196] = c
    full[196, 196] = d
    full[196, SLOT:] = b3
    zu[:, 0, :] = full[0:128]
    zu[0:69, 1, :] = full[128:197]

    d1 = np.zeros((P, 8, FEAT), np.float32)
    W1T = np.asarray(Wd1, f64).T  # [784 f_in, 784 j]
    for t in range(T):
        d1[:, t, :] = W1T[t * FV : t * FV + P, :]
        d1[0:68, 4 + t, :] = W1T[t * FV + P : (t + 1) * FV, :]
    d1[68, 4, :] = np.asarray(bd1, f64)

    d2 = np.zeros((P, 7, FEAT), np.float32)
    W2T = np.asarray(Wd2, f64).T
    for cidx in range(6):
        d2[:, cidx, :] = W2T[cidx * P : (cidx + 1) * P, :]
    d2[0:16, 6, :] = W2T[768:784, :]
    d2[16, 6, :] = np.asarray(bd2, f64)

    ow = np.zeros((P, 7, 10), np.float32)
    WoT = np.asarray(Wo, f64).T
    for cidx in range(6):
        ow[:, cidx, :] = WoT[cidx * P : (cidx + 1) * P, :]
    ow[0:16, 6, :] = WoT[768:784, :]
    ow[16, 6, :] = np.asarray(bo, f64)
    return (zu.astype(ml_dtypes.bfloat16), d1.astype(ml_dtypes.bfloat16),
            d2.astype(ml_dtypes.bfloat16), ow.astype(ml_dtypes.bfloat16))


_NC_CACHE = {}


def kernel(**inputs):
    x = np.ascontiguousarray(np.asarray(inputs["x"], np.float32)).astype(
        ml_dtypes.bfloat16
    )
    zu, d1, d2, ow = pack_weights(
        inputs["W1"], inputs["b1"], inputs["W2"], inputs["b2"], inputs["W3"],
        inputs["b3"], inputs["Wd1"], inputs["bd1"], inputs["Wd2"],
        inputs["bd2"], inputs["Wo"], inputs["bo"],
    )
    if "nc" not in _NC_CACHE:
        _NC_CACHE["nc"] = build(8, 4, unroll=True)
    nc = _NC_CACHE["nc"]
    bpc = B // NCORES
    in_maps = [
        {
            "x": x[c * bpc : (c + 1) * bpc],
            "zu_w": zu,
            "dec1_w": d1,
            "dec2_w": d2,
            "out_w": ow,
        }
        for c in range(NCORES)
    ]
    res = run_bass_kernel_spmd(nc, in_maps, core_ids=list(range(NCORES)))
    return np.concatenate([res.results[c]["out"] for c in range(NCORES)], axis=0)


# revision 5
# speedup vs baseline: 1.0090x; 1.0037x over previous
"""Trainium2 Bass kernel for nn_CapsuleNeuralNetworkV2 (8 cores, data-parallel).

Math (per sample, 8 capsule iterations then decoder):
  v = h.reshape(4, 196)
  q = v @ W1.T + b1 ; k = v @ W2.T + b2 ; u = v @ W3.T + b3
  scores[t,s] = q_t . k_s  ->  softmax over s -> h'_t = sum_s P[t,s] u_s
  dec = relu(h Wd1.T + bd1) Wd2.T + bd2 ; out = softmax(dec Wo.T + bo)

Host-side algebra:
  scores[t,s] = v_t . z_s + r_s,  z_s = G v_s + c, r_s = a.v_s + d,
  G = W1.T W2, a = W2.T b1, c = W1.T b2, d = b1.b2; biases fused via an
  augmented ones row so one matmul emits [z | r | u] per slot.

v3 layout/schedule changes vs v2:
  - h slot is 197 wide ([v(196) | 1]); zu slot is 393 ([z | r | u]); the
    dots run over 197 elems and pick up r via the ones column.
  - zu matmuls write two 2-slot PSUM tiles; each is evacuated by ONE Act
    copy (f32->bf16), replacing four per-slot copies.
  - vt transposes write one bf16 PSUM tile; two DVE 2x copies evacuate.
  - dots are 16 STT+accum ops, combine is 4 chains of (seed + 3 MACs);
    both are spread across DVE/Act/Pool by a static table tuned against
    the instruction cost model.
  - probs = e * (1/sum) via one broadcast tensor_tensor.
"""

import numpy as np
import ml_dtypes

import concourse.bass as bass
import concourse.tile as tile
from concourse import bacc, mybir
from concourse.bass import ds
from concourse.bass_utils import run_bass_kernel_spmd
from concourse.masks import make_identity

FR = mybir.dt.float32r
BF = mybir.dt.bfloat16
F32 = mybir.dt.float32
AF = mybir.ActivationFunctionType
ALU = mybir.AluOpType

B = 32768
NCORES = 8
P = 128
T = 4
FV = 196
FEAT = 784
SLOT = FV + 1  # 197: 196 data + ones col
ZSLOT = 2 * FV + 1  # 393: z(196) | r(1) | u(196)

# --- static engine tables (v=DVE, a=Act, p=Pool), tuned vs cost model ---
# dots[t][s] (Act cannot run STT)
DOTS_ENG = [
    "vp..",
    "pp..",
    "pp..",
    "pp..",
]
# combine: SEED_ENG[t] + MAC_ENG[t] (3 chained MACs; MACs only v or p)
# combine mul engine per (t, s); adds are two batched DVE tensor_tensor
MUL_ENG = [
    "aapp",
    "aapp",
    "aapp",
    "appp",
]
# vt-evac engines for (chunk1, chunk2)
VT_ENG = "va"
# s-slots computed via a DVE products TT + 4 cheap TSP-accums
PROD_S = (2, 3)
ADDS_PER_T = False
ZU0_B2 = False
PHASE_MAJOR = False
ZU_BUFS = 5
D2_ACT = True
EXP_ACCUM = False


def _ap(t, dims, offset_elems=0):
    a = t[:] if hasattr(t, "tile") or not isinstance(t, bass.AP) else t
    return bass.AP(tensor=a.tensor, offset=a.offset + offset_elems, ap=dims)


def build(nsub=8, ngroups=4, unroll=False, zu_bufs=1, vtps_bufs=2, mpd_bufs=2,
          h_bufs=3, wk_bufs=5, sm_bufs=8, zu_mode="half", hf_bufs=1):
    """One NeuronCore program processing nsub*ngroups*128 samples."""
    bpc = nsub * ngroups * P
    nc = bacc.Bacc("TRN2", target_bir_lowering=False, debug=False)

    x_d = nc.dram_tensor("x", [bpc, FEAT], BF, kind="ExternalInput")
    zu_d = nc.dram_tensor("zu_w", [P, 2, ZSLOT], BF, kind="ExternalInput")
    d1_d = nc.dram_tensor("dec1_w", [P, 8, FEAT], BF, kind="ExternalInput")
    d2_d = nc.dram_tensor("dec2_w", [P, 7, FEAT], BF, kind="ExternalInput")
    ow_d = nc.dram_tensor("out_w", [P, 7, 10], BF, kind="ExternalInput")
    out_d = nc.dram_tensor("out", [bpc, 10], F32, kind="ExternalOutput")

    with tile.TileContext(nc) as tc:
        consts = tc.alloc_tile_pool(name="consts", bufs=1)
        hp = tc.alloc_tile_pool(name="h", bufs=h_bufs)
        wk = tc.alloc_tile_pool(name="wk", bufs=wk_bufs)
        wkd = tc.alloc_tile_pool(name="wkd", bufs=1)
        sm = tc.alloc_tile_pool(name="small", bufs=sm_bufs)
        pp = tc.alloc_tile_pool(name="ps", bufs=zu_bufs, space="PSUM")
        pv = tc.alloc_tile_pool(name="pv", bufs=vtps_bufs, space="PSUM")

        ident_f = consts.tile([P, P], F32)
        make_identity(nc, ident_f)
        ident = consts.tile([P, P], FR)
        nc.vector.tensor_copy(ident, ident_f)
        ident_b = consts.tile([P, P], BF)
        nc.vector.tensor_copy(ident_b, ident_f)
        ones_c = consts.tile([P, 512], F32)
        nc.vector.memset(ones_c, 1.0)
        zu_w = consts.tile([P, 2, ZSLOT], BF)
        nc.sync.dma_start(out=zu_w, in_=zu_d[:, :, :])
        d1_w = consts.tile([P, 8, FEAT], BF)
        nc.sync.dma_start(out=d1_w, in_=d1_d[:, :, :])
        d2_w = consts.tile([P, 7, FEAT], BF)
        nc.sync.dma_start(out=d2_w, in_=d2_d[:, :, :])
        ow_w = consts.tile([P, 7, 10], BF)
        nc.sync.dma_start(out=ow_w, in_=ow_d[:, :, :])

        def eng(c):
            return {"v": nc.vector, "p": nc.gpsimd, "a": nc.scalar}[c]

        def capsule_psum(h_cur, j):
            """PE transposes + zu matmuls + evacuations -> zu SBUF tile."""
            vt_ps = pv.tile([P, T, 2, P], BF, tag="vtps")
            for t in range(T):
                nc.tensor.transpose(vt_ps[:, t, 0, :], h_cur[:, t, 0:P], ident_b)
                # includes the ones column -> row 68 is 1.0
                nc.tensor.transpose(vt_ps[0:69, t, 1, :],
                                    h_cur[:, t, P : P + 69], ident_b)
            vt = wk.tile([P, T, 2, P], BF, tag="vt")
            for ci, (dst, srcp) in enumerate(
                [(vt[:, :, 0, :], vt_ps[:, :, 0, :]),
                 (vt[0:69, :, 1, :], vt_ps[0:69, :, 1, :])]
            ):
                c = VT_ENG[ci]
                if c == "a":
                    nc.scalar.copy(dst, srcp)
                elif c == "v":
                    nc.vector.tensor_copy(dst, srcp)
                else:
                    nc.gpsimd.tensor_copy(dst, srcp)

            zu = wk.tile([P, T, ZSLOT], BF, tag="zu", bufs=ZU_BUFS)
            for half in range(2):
                # 512-wide slots keep each matmul within one PSUM bank
                zu_ps = pp.tile([P, 2, 512], F32, tag=f"zu{half}")
                for k in range(2):
                    s = 2 * half + k
                    nc.tensor.matmul(zu_ps[:, k, 0:ZSLOT], vt[:, s, 0, :],
                                     zu_w[:, 0, :], start=True, stop=False)
                    nc.tensor.matmul(zu_ps[:, k, 0:ZSLOT], vt[0:69, s, 1, :],
                                     zu_w[0:69, 1, :], start=False, stop=True)
                nc.scalar.copy(zu[:, 2 * half : 2 * half + 2, :],
                               zu_ps[:, :, 0:ZSLOT])
            return zu

        def capsule_vec(h_cur, h_nxt, zu, j):
            """SBUF-only: dots, softmax, combine -> h_nxt."""
            dots = sm.tile([P, T, T], F32, tag="dots")
            scr = sm.tile([P, 3, SLOT], BF, tag="scr", bufs=8)
            for half in range(2):
                prod = sm.tile([P, 2, T, SLOT], BF, tag=f"prod{half}", bufs=2)
                hin = _ap(h_cur, [h_cur[:].ap[0], [0, 2], [SLOT, T], [1, SLOT]])
                zin = _ap(zu, [zu[:].ap[0], [ZSLOT, 2], [0, T], [1, SLOT]],
                          offset_elems=half * 2 * ZSLOT)
                nc.vector.tensor_tensor(out=prod, in0=hin, in1=zin, op=ALU.mult)
                for k in range(2):
                    s = 2 * half + k
                    for t in range(T):
                        nc.vector.tensor_scalar(
                            out=scr[:, 0, :],
                            in0=prod[:, k, t, :], scalar1=1.0,
                            scalar2=0.0, op0=ALU.mult, op1=ALU.add,
                            accum_out=dots[:, t, s : s + 1])

            # softmax over s (no max subtraction; |scores| < 30)
            e_t = sm.tile([P, T, T], F32, tag="e")
            sums = sm.tile([P, T], F32, tag="sums")
            if EXP_ACCUM:
                for t in range(T):
                    nc.scalar.activation(e_t[:, t, :], dots[:, t, :], AF.Exp,
                                         accum_out=sums[:, t : t + 1])
            else:
                nc.scalar.activation(e_t[:, :, 0:2], dots[:, :, 0:2], AF.Exp)
                nc.scalar.activation(e_t[:, :, 2:4], dots[:, :, 2:4], AF.Exp)
                nc.vector.reduce_sum(sums, e_t, axis=mybir.AxisListType.X)
            rec = sm.tile([P, T], F32, tag="rec")
            nc.vector.reciprocal(rec, sums)
            probs = sm.tile([P, T, T], F32, tag="probs")
            nc.vector.tensor_tensor(
                out=probs, in0=e_t,
                in1=_ap(rec, [rec[:].ap[0], [1, T], [0, T]]),
                op=ALU.mult,
            )

            # ones column for the next h
            nc.gpsimd.tensor_copy(h_nxt[:, :, FV:SLOT], ones_c[:, 0:T])
            # combine: pu[t,s] = P[t,s] * u_s, then two batched DVE adds
            pu = sm.tile([P, T, T, FV], BF, tag="pu", bufs=2)
            for t in range(T):
                for s in range(T):
                    c = MUL_ENG[t][s]
                    if c == "a":
                        nc.scalar.activation(
                            pu[:, t, s, :], zu[:, s, SLOT:ZSLOT], AF.Copy,
                            scale=probs[:, t, s : s + 1])
                    elif c == "v":
                        nc.vector.tensor_scalar_mul(
                            pu[:, t, s, :], zu[:, s, SLOT:ZSLOT],
                            probs[:, t, s : s + 1])
                    else:
                        nc.gpsimd.tensor_scalar_mul(
                            pu[:, t, s, :], zu[:, s, SLOT:ZSLOT],
                            probs[:, t, s : s + 1])
            q = sm.tile([P, T, 2, FV], BF, tag="q", bufs=2)
            ev = _ap(pu, [pu[:].ap[0], [T * FV, T], [2 * FV, 2], [1, FV]])
            od = _ap(pu, [pu[:].ap[0], [T * FV, T], [2 * FV, 2], [1, FV]],
                     offset_elems=FV)
            nc.vector.tensor_tensor(out=q, in0=ev, in1=od, op=ALU.add)
            nc.vector.tensor_tensor(out=h_nxt[:, :, 0:FV], in0=q[:, :, 0, :],
                                    in1=q[:, :, 1, :], op=ALU.add)

        def capsule_iter(h_cur, h_nxt, j):
            zu = capsule_psum(h_cur, j)
            capsule_vec(h_cur, h_nxt, zu, j)

        def decoder(hs, g, goff=0):
            """Decoder over a chunk of <=4 tiles (N = len(hs)*128 wide)."""
            W = len(hs) * P
            # h.T chunks, slot-major: [128] x4 and [69] x4 (with ones row)
            ht1 = wkd.tile([P, T, W], BF, tag="ht1")
            ht2 = wkd.tile([69, T, W], BF, tag="ht2")
            for t in range(T):
                t1_ps = pv.tile([P, W], BF, tag="vtps")
                t2_ps = pv.tile([69, W], BF, tag="vtps")
                for j in range(len(hs)):
                    nc.tensor.transpose(
                        t1_ps[:, j * P : (j + 1) * P], hs[j][:, t, 0:P], ident_b
                    )
                    nc.tensor.transpose(
                        t2_ps[:, j * P : (j + 1) * P],
                        hs[j][:, t, P : P + 69], ident_b
                    )
                nc.scalar.copy(ht1[:, t, :], t1_ps)
                nc.vector.tensor_copy(ht2[:, t, :], t2_ps)

            # dec1 = relu(Wd1 @ h.T + bd1), feature-major, 7 M-chunks
            d1a = wkd.tile([P, 6, W], BF, tag="d1a")
            d1b = wkd.tile([17, W], BF, tag="d1b")
            nc.vector.tensor_copy(d1b, ones_c[0:17, 0:W])
            for m in range(7):
                mw = min(P, FEAT - m * P)
                mp = pv.tile([P, W], F32, tag="mpd", bufs=mpd_bufs)
                msl = slice(m * P, m * P + mw)
                for t in range(T):
                    nc.tensor.matmul(mp[0:mw, :], d1_w[:, t, msl], ht1[:, t, :],
                                     start=(t == 0), stop=False)
                for t in range(T):
                    nc.tensor.matmul(mp[0:mw, :], d1_w[0:69, 4 + t, msl],
                                     ht2[:, t, :], start=False, stop=(t == 3))
                if m < 6:
                    nc.scalar.activation(d1a[:, m, :], mp, AF.Relu)
                else:
                    nc.scalar.activation(d1b[0:16, :], mp[0:16, :], AF.Relu)

            # dec2 = Wd2 @ relu1 + bd2, feature-major
            d2a = wkd.tile([P, 6, W], BF, tag="d2a")
            d2b = wkd.tile([17, W], BF, tag="d2b")
            nc.vector.tensor_copy(d2b, ones_c[0:17, 0:W])
            for m in range(7):
                mw = min(P, FEAT - m * P)
                mp = pv.tile([P, W], F32, tag="mpd", bufs=mpd_bufs)
                msl = slice(m * P, m * P + mw)
                for c in range(6):
                    nc.tensor.matmul(mp[0:mw, :], d2_w[:, c, msl], d1a[:, c, :],
                                     start=(c == 0), stop=False)
                nc.tensor.matmul(mp[0:mw, :], d2_w[0:17, 6, msl], d1b,
                                 start=False, stop=True)
                if m < 6:
                    (nc.scalar.copy if D2_ACT else nc.vector.tensor_copy)(
                        d2a[:, m, :], mp)
                else:
                    (nc.scalar.copy if D2_ACT else nc.vector.tensor_copy)(
                        d2b[0:16, :], mp[0:16, :])

            # logits + softmax per subtile
            for j in range(len(hs)):
                jsl = slice(j * P, (j + 1) * P)
                lg = pv.tile([P, 10], F32, tag="mpd", bufs=mpd_bufs)
                for c in range(6):
                    nc.tensor.matmul(lg, d2a[:, c, jsl], ow_w[:, c, :],
                                     start=(c == 0), stop=False)
                nc.tensor.matmul(lg, d2b[:, jsl], ow_w[0:17, 6, :],
                                 start=False, stop=True)
                mx = sm.tile([P, 1], F32, tag="mx")
                nc.vector.reduce_max(mx, lg, axis=mybir.AxisListType.X)
                nmx = sm.tile([P, 1], F32, tag="nmx")
                nc.vector.tensor_scalar_mul(nmx, mx, -1.0)
                e10 = sm.tile([P, 10], F32, tag="e10")
                s10 = sm.tile([P, 1], F32, tag="s10")
                nc.scalar.activation(e10, lg, AF.Exp, bias=nmx, accum_out=s10)
                r10 = sm.tile([P, 1], F32, tag="r10")
                nc.vector.reciprocal(r10, s10)
                o10 = sm.tile([P, 10], F32, tag="o10")
                nc.vector.tensor_scalar_mul(o10, e10, r10)
                nc.sync.dma_start(
                    out=out_d[ds(g * (nsub * P) + (goff + j) * P, P), :],
                    in_=o10,
                )

        def body_capsule(g):
            hs = []
            for j in range(nsub):
                h0 = hp.tile([P, T, SLOT], BF, tag=f"h{j}")
                nc.sync.dma_start(
                    out=h0[:, :, 0:FV],
                    in_=x_d[ds(g * (nsub * P) + j * P, P), :].rearrange(
                        "p (t f) -> p t f", t=T
                    ),
                )
                nc.gpsimd.tensor_copy(h0[:, :, FV:SLOT], ones_c[:, 0:T])
                hs.append(h0)
            for it in range(8):
                nxts = []
                for j in range(nsub):
                    if it < 7:
                        h_nxt = hp.tile([P, T, SLOT], BF, tag=f"h{j}")
                    else:
                        h_nxt = hp.tile([P, T, SLOT], BF, tag=f"hold{j}",
                                        bufs=2)
                    nxts.append(h_nxt)
                if PHASE_MAJOR:
                    zus = [capsule_psum(hs[j], j) for j in range(nsub)]
                    for j in range(nsub):
                        capsule_vec(hs[j], nxts[j], zus[j], j)
                else:
                    for j in range(nsub):
                        capsule_iter(hs[j], nxts[j], j)
                hs = list(nxts)
            return hs

        def body(g):
            hs = body_capsule(g)
            for d0 in range(0, nsub, 4):
                decoder(hs[d0 : d0 + 4], g, d0)

        if ngroups == 1:
            body(0)
        elif unroll:
            # software-pipelined: decode group g-1 while computing group g
            holds = [body_capsule(0)]
            for g in range(1, ngroups):
                holds.append(body_capsule(g))
                for d0 in range(0, nsub, 4):
                    decoder(holds[g - 1][d0 : d0 + 4], g - 1, d0)
            for d0 in range(0, nsub, 4):
                decoder(holds[-1][d0 : d0 + 4], ngroups - 1, d0)
        else:
            with tc.For_i(0, ngroups, 1) as g:
                body(g)
        for _pool in (pv, pp, sm, wkd, wk, hp, consts):
            _pool.release()

    nc.compile()
    return nc


def pack_weights(W1, b1, W2, b2, W3, b3, Wd1, bd1, Wd2, bd2, Wo, bo):
    f64 = np.float64
    W1, b1, W2, b2, W3, b3 = (np.asarray(t, f64) for t in (W1, b1, W2, b2, W3, b3))
    G = W1.T @ W2
    a = W2.T @ b1
    c = W1.T @ b2
    d = float(b1 @ b2)

    zu = np.zeros((P, 2, ZSLOT), np.float32)
    full = np.zeros((197, ZSLOT), f64)
    full[:196, :196] = G.T
    full[:196, 196] = a
    full[:196, SLOT:] = W3.T
    full[196, :196] = c
    full[196, 196] = d
    full[196, SLOT:] = b3
    zu[:, 0, :] = full[0:128]
    zu[0:69, 1, :] = full[128:197]

    d1 = np.zeros((P, 8, FEAT), np.float32)
    W1T = np.asarray(Wd1, f64).T  # [784 f_in, 784 j]
    for t in range(T):
        d1[:, t, :] = W1T[t * FV : t * FV + P, :]
        d1[0:68, 4 + t, :] = W1T[t * FV + P : (t + 1) * FV, :]
    d1[68, 4, :] = np.asarray(bd1, f64)

    d2 = np.zeros((P, 7, FEAT), np.float32)
    W2T = np.asarray(Wd2, f64).T
    for cidx in range(6):
        d2[:, cidx, :] = W2T[cidx * P : (cidx + 1) * P, :]
    d2[0:16, 6, :] = W2T[768:784, :]
    d2[16, 6, :] = np.asarray(bd2, f64)

    ow = np.zeros((P, 7, 10), np.float32)
    WoT = np.asarray(Wo, f64).T
    for cidx in range(6):
        ow[:, cidx, :] = WoT[cidx * P : (cidx + 1) * P, :]
    ow[0:16, 6, :] = WoT[768:784, :]
    ow[16, 6, :] = np.asarray(bo, f64)
    return (zu.astype(ml_dtypes.bfloat16), d1.astype(ml_dtypes.bfloat16),
            d2.astype(ml_dtypes.bfloat16), ow.astype(ml_dtypes.bfloat16))


_NC_CACHE = {}


def kernel(**inputs):
    x = np.ascontiguousarray(np.asarray(inputs["x"], np.float32)).astype(
        ml_dtypes.bfloat16
    )
    zu, d1, d2, ow = pack_weights(
        inputs["W1"], inputs["b1"], inputs["W2"], inputs["b2"], inputs["W3"],
        inputs["b3"], inputs["Wd1"], inputs["bd1"], inputs["Wd2"],
        inputs["bd2"], inputs["Wo"], inputs["bo"],
    )
    if "nc" not in _NC_CACHE:
        _NC_CACHE["nc"] = build(8, 4, unroll=True)
    nc = _NC_CACHE["nc"]
    bpc = B // NCORES
    in_maps = [
        {
            "x": x[c * bpc : (c + 1) * bpc],
            "zu_w": zu,
            "dec1_w": d1,
            "dec2_w": d2,
            "out_w": ow,
        }
        for c in range(NCORES)
    ]
    res = run_bass_kernel_spmd(nc, in_maps, core_ids=list(range(NCORES)))
    return np.concatenate([res.results[c]["out"] for c in range(NCORES)], axis=0)


# revision 6
# speedup vs baseline: 1.0464x; 1.0370x over previous
"""Trainium2 Bass kernel for nn_CapsuleNeuralNetworkV2 (8 cores, data-parallel).

Math (per sample, 8 capsule iterations then decoder):
  v = h.reshape(4, 196)
  q = v @ W1.T + b1 ; k = v @ W2.T + b2 ; u = v @ W3.T + b3
  scores[t,s] = q_t . k_s  ->  softmax over s -> h'_t = sum_s P[t,s] u_s
  dec = relu(h Wd1.T + bd1) Wd2.T + bd2 ; out = softmax(dec Wo.T + bo)

Host-side algebra:
  scores[t,s] = v_t . z_s + r_s,  z_s = G v_s + c, r_s = a.v_s + d,
  G = W1.T W2, a = W2.T b1, c = W1.T b2, d = b1.b2; biases fused via an
  augmented ones row so one matmul emits [z | r | u] per slot.

v3 layout/schedule changes vs v2:
  - h slot is 197 wide ([v(196) | 1]); zu slot is 393 ([z | r | u]); the
    dots run over 197 elems and pick up r via the ones column.
  - zu matmuls write two 2-slot PSUM tiles; each is evacuated by ONE Act
    copy (f32->bf16), replacing four per-slot copies.
  - vt transposes write one bf16 PSUM tile; two DVE 2x copies evacuate.
  - dots are 16 STT+accum ops, combine is 4 chains of (seed + 3 MACs);
    both are spread across DVE/Act/Pool by a static table tuned against
    the instruction cost model.
  - probs = e * (1/sum) via one broadcast tensor_tensor.
"""

import numpy as np
import ml_dtypes

import concourse.bass as bass
import concourse.tile as tile
from concourse import bacc, mybir
from concourse.bass import ds
from concourse.bass_utils import run_bass_kernel_spmd
from concourse.masks import make_identity

FR = mybir.dt.float32r
BF = mybir.dt.bfloat16
F32 = mybir.dt.float32
AF = mybir.ActivationFunctionType
ALU = mybir.AluOpType

B = 32768
NCORES = 8
P = 128
T = 4
FV = 196
FEAT = 784
SLOT = FV + 1  # 197: 196 data + ones col
ZSLOT = 2 * FV + 1  # 393: z(196) | r(1) | u(196)

# --- static engine tables (v=DVE, a=Act, p=Pool), tuned vs cost model ---
# dots[t][s] (Act cannot run STT)
DOTS_ENG = [
    "vp..",
    "pp..",
    "pp..",
    "pp..",
]
# combine: SEED_ENG[t] + MAC_ENG[t] (3 chained MACs; MACs only v or p)
# combine mul engine per (t, s); adds are two batched DVE tensor_tensor
MUL_ENG = [
    "aapp",
    "aapp",
    "aapp",
    "appp",
]
# vt-evac engines for (chunk1, chunk2)
VT_ENG = "va"
# s-slots computed via a DVE products TT + 4 cheap TSP-accums
PROD_S = (2, 3)
ADDS_PER_T = False
ZU0_B2 = False
PHASE_MAJOR = False
ZU_BUFS = 5
D2_ACT = True
EXP_ACCUM = False


def _ap(t, dims, offset_elems=0):
    a = t[:] if hasattr(t, "tile") or not isinstance(t, bass.AP) else t
    return bass.AP(tensor=a.tensor, offset=a.offset + offset_elems, ap=dims)


def build(nsub=8, ngroups=4, unroll=False, zu_bufs=1, vtps_bufs=2, mpd_bufs=2,
          h_bufs=3, wk_bufs=5, sm_bufs=8, zu_mode="half", hf_bufs=1):
    """One NeuronCore program processing nsub*ngroups*128 samples."""
    bpc = nsub * ngroups * P
    nc = bacc.Bacc("TRN2", target_bir_lowering=False, debug=False)

    x_d = nc.dram_tensor("x", [bpc, FEAT], BF, kind="ExternalInput")
    zu_d = nc.dram_tensor("zu_w", [P, 2, ZSLOT], BF, kind="ExternalInput")
    d1_d = nc.dram_tensor("dec1_w", [P, 8, FEAT], BF, kind="ExternalInput")
    d2_d = nc.dram_tensor("dec2_w", [P, 7, FEAT], BF, kind="ExternalInput")
    ow_d = nc.dram_tensor("out_w", [P, 7, 10], BF, kind="ExternalInput")
    out_d = nc.dram_tensor("out", [bpc, 10], F32, kind="ExternalOutput")

    with tile.TileContext(nc) as tc:
        consts = tc.alloc_tile_pool(name="consts", bufs=1)
        hp = tc.alloc_tile_pool(name="h", bufs=h_bufs)
        wk = tc.alloc_tile_pool(name="wk", bufs=wk_bufs)
        wkd = tc.alloc_tile_pool(name="wkd", bufs=1)
        sm = tc.alloc_tile_pool(name="small", bufs=sm_bufs)
        pp = tc.alloc_tile_pool(name="ps", bufs=zu_bufs, space="PSUM")
        pv = tc.alloc_tile_pool(name="pv", bufs=vtps_bufs, space="PSUM")

        ident_f = consts.tile([P, P], F32)
        make_identity(nc, ident_f)
        ident = consts.tile([P, P], FR)
        nc.vector.tensor_copy(ident, ident_f)
        ident_b = consts.tile([P, P], BF)
        nc.vector.tensor_copy(ident_b, ident_f)
        ones_c = consts.tile([P, 512], F32)
        nc.vector.memset(ones_c, 1.0)
        zu_w = consts.tile([P, 2, ZSLOT], BF)
        nc.sync.dma_start(out=zu_w, in_=zu_d[:, :, :])
        d1_w = consts.tile([P, 8, FEAT], BF)
        nc.sync.dma_start(out=d1_w, in_=d1_d[:, :, :])
        d2_w = consts.tile([P, 7, FEAT], BF)
        nc.sync.dma_start(out=d2_w, in_=d2_d[:, :, :])
        ow_w = consts.tile([P, 7, 10], BF)
        nc.sync.dma_start(out=ow_w, in_=ow_d[:, :, :])

        def eng(c):
            return {"v": nc.vector, "p": nc.gpsimd, "a": nc.scalar}[c]

        def capsule_psum(h_cur, j):
            """PE transposes + zu matmuls + evacuations -> zu SBUF tile."""
            vt_ps = pv.tile([P, T, 2, P], BF, tag="vtps")
            for t in range(T):
                nc.tensor.transpose(vt_ps[:, t, 0, :], h_cur[:, t, 0:P], ident_b)
                # includes the ones column -> row 68 is 1.0
                nc.tensor.transpose(vt_ps[0:69, t, 1, :],
                                    h_cur[:, t, P : P + 69], ident_b)
            vt = wk.tile([P, T, 2, P], BF, tag="vt")
            for ci, (dst, srcp) in enumerate(
                [(vt[:, :, 0, :], vt_ps[:, :, 0, :]),
                 (vt[0:69, :, 1, :], vt_ps[0:69, :, 1, :])]
            ):
                c = VT_ENG[ci]
                if c == "a":
                    nc.scalar.copy(dst, srcp)
                elif c == "v":
                    nc.vector.tensor_copy(dst, srcp)
                else:
                    nc.gpsimd.tensor_copy(dst, srcp)

            zu = wk.tile([P, T, ZSLOT], BF, tag="zu", bufs=ZU_BUFS)
            for half in range(2):
                # 512-wide slots keep each matmul within one PSUM bank
                zu_ps = pp.tile([P, 2, 512], F32, tag=f"zu{half}")
                for k in range(2):
                    s = 2 * half + k
                    nc.tensor.matmul(zu_ps[:, k, 0:ZSLOT], vt[:, s, 0, :],
                                     zu_w[:, 0, :], start=True, stop=False)
                    nc.tensor.matmul(zu_ps[:, k, 0:ZSLOT], vt[0:69, s, 1, :],
                                     zu_w[0:69, 1, :], start=False, stop=True)
                nc.scalar.copy(zu[:, 2 * half : 2 * half + 2, :],
                               zu_ps[:, :, 0:ZSLOT])
            return zu

        def capsule_vec(h_cur, h_nxt, zu, j):
            """SBUF-only: dots, softmax, combine -> h_nxt."""
            dots = sm.tile([P, T, T], F32, tag="dots")
            scr = sm.tile([P, 3, SLOT], BF, tag="scr", bufs=8)
            for half in range(2):
                prod = sm.tile([P, 2, T, SLOT], BF, tag=f"prod{half}", bufs=2)
                hin = _ap(h_cur, [h_cur[:].ap[0], [0, 2], [SLOT, T], [1, SLOT]])
                zin = _ap(zu, [zu[:].ap[0], [ZSLOT, 2], [0, T], [1, SLOT]],
                          offset_elems=half * 2 * ZSLOT)
                nc.vector.tensor_tensor(out=prod, in0=hin, in1=zin, op=ALU.mult)
                for k in range(2):
                    s = 2 * half + k
                    for t in range(T):
                        nc.vector.tensor_scalar(
                            out=scr[:, 0, :],
                            in0=prod[:, k, t, :], scalar1=1.0,
                            scalar2=0.0, op0=ALU.mult, op1=ALU.add,
                            accum_out=dots[:, t, s : s + 1])

            # softmax over s (no max subtraction; |scores| < 30)
            e_t = sm.tile([P, T, T], F32, tag="e")
            sums = sm.tile([P, T], F32, tag="sums")
            if EXP_ACCUM:
                for t in range(T):
                    nc.scalar.activation(e_t[:, t, :], dots[:, t, :], AF.Exp,
                                         accum_out=sums[:, t : t + 1])
            else:
                nc.scalar.activation(e_t[:, :, 0:2], dots[:, :, 0:2], AF.Exp)
                nc.scalar.activation(e_t[:, :, 2:4], dots[:, :, 2:4], AF.Exp)
                nc.vector.reduce_sum(sums, e_t, axis=mybir.AxisListType.X)
            rec = sm.tile([P, T], F32, tag="rec")
            nc.vector.reciprocal(rec, sums)
            probs = sm.tile([P, T, T], F32, tag="probs")
            nc.vector.tensor_tensor(
                out=probs, in0=e_t,
                in1=_ap(rec, [rec[:].ap[0], [1, T], [0, T]]),
                op=ALU.mult,
            )

            # ones column for the next h
            nc.gpsimd.tensor_copy(h_nxt[:, :, FV:SLOT], ones_c[:, 0:T])
            # combine: pu[t,s] = P[t,s] * u_s, then two batched DVE adds
            pu = sm.tile([P, T, T, FV], BF, tag="pu", bufs=2)
            for t in range(T):
                for s in range(T):
                    c = MUL_ENG[t][s]
                    if c == "a":
                        nc.scalar.activation(
                            pu[:, t, s, :], zu[:, s, SLOT:ZSLOT], AF.Copy,
                            scale=probs[:, t, s : s + 1])
                    elif c == "v":
                        nc.vector.tensor_scalar_mul(
                            pu[:, t, s, :], zu[:, s, SLOT:ZSLOT],
                            probs[:, t, s : s + 1])
                    else:
                        nc.gpsimd.tensor_scalar_mul(
                            pu[:, t, s, :], zu[:, s, SLOT:ZSLOT],
                            probs[:, t, s : s + 1])
            q = sm.tile([P, T, 2, FV], BF, tag="q", bufs=2)
            ev = _ap(pu, [pu[:].ap[0], [T * FV, T], [2 * FV, 2], [1, FV]])
            od = _ap(pu, [pu[:].ap[0], [T * FV, T], [2 * FV, 2], [1, FV]],
                     offset_elems=FV)
            nc.vector.tensor_tensor(out=q, in0=ev, in1=od, op=ALU.add)
            nc.vector.tensor_tensor(out=h_nxt[:, :, 0:FV], in0=q[:, :, 0, :],
                                    in1=q[:, :, 1, :], op=ALU.add)

        def capsule_iter(h_cur, h_nxt, j):
            zu = capsule_psum(h_cur, j)
            capsule_vec(h_cur, h_nxt, zu, j)

        def decoder(hs, g, goff=0):
            """Decoder over a chunk of <=4 tiles (N = len(hs)*128 wide)."""
            W = len(hs) * P
            # h.T chunks, slot-major: [128] x4 and [69] x4 (with ones row)
            ht1 = wkd.tile([P, T, W], BF, tag="ht1")
            ht2 = wkd.tile([69, T, W], BF, tag="ht2")
            for t in range(T):
                t1_ps = pv.tile([P, W], BF, tag="vtps")
                t2_ps = pv.tile([69, W], BF, tag="vtps")
                for j in range(len(hs)):
                    nc.tensor.transpose(
                        t1_ps[:, j * P : (j + 1) * P], hs[j][:, t, 0:P], ident_b
                    )
                    nc.tensor.transpose(
                        t2_ps[:, j * P : (j + 1) * P],
                        hs[j][:, t, P : P + 69], ident_b
                    )
                nc.scalar.copy(ht1[:, t, :], t1_ps)
                nc.vector.tensor_copy(ht2[:, t, :], t2_ps)

            # dec1 = relu(Wd1 @ h.T + bd1), feature-major, 7 M-chunks
            d1a = wkd.tile([P, 6, W], BF, tag="d1a")
            d1b = wkd.tile([17, W], BF, tag="d1b")
            nc.vector.tensor_copy(d1b, ones_c[0:17, 0:W])
            for m in range(7):
                mw = min(P, FEAT - m * P)
                mp = pv.tile([P, W], F32, tag="mpd", bufs=mpd_bufs)
                msl = slice(m * P, m * P + mw)
                for t in range(T):
                    nc.tensor.matmul(mp[0:mw, :], d1_w[:, t, msl], ht1[:, t, :],
                                     start=(t == 0), stop=False)
                for t in range(T):
                    nc.tensor.matmul(mp[0:mw, :], d1_w[0:69, 4 + t, msl],
                                     ht2[:, t, :], start=False, stop=(t == 3))
                if m < 6:
                    nc.scalar.activation(d1a[:, m, :], mp, AF.Relu)
                else:
                    nc.scalar.activation(d1b[0:16, :], mp[0:16, :], AF.Relu)

            # dec2 = Wd2 @ relu1 + bd2, feature-major
            d2a = wkd.tile([P, 6, W], BF, tag="d2a")
            d2b = wkd.tile([17, W], BF, tag="d2b")
            nc.vector.tensor_copy(d2b, ones_c[0:17, 0:W])
            for m in range(7):
                mw = min(P, FEAT - m * P)
                mp = pv.tile([P, W], F32, tag="mpd", bufs=mpd_bufs)
                msl = slice(m * P, m * P + mw)
                for c in range(6):
                    nc.tensor.matmul(mp[0:mw, :], d2_w[:, c, msl], d1a[:, c, :],
                                     start=(c == 0), stop=False)
                nc.tensor.matmul(mp[0:mw, :], d2_w[0:17, 6, msl], d1b,
                                 start=False, stop=True)
                if m < 6:
                    (nc.scalar.copy if D2_ACT else nc.vector.tensor_copy)(
                        d2a[:, m, :], mp)
                else:
                    (nc.scalar.copy if D2_ACT else nc.vector.tensor_copy)(
                        d2b[0:16, :], mp[0:16, :])

            # logits for all subtiles into one PSUM tile, then one
            # batched softmax (no max subtraction; |logits| < 30) and a
            # single strided output DMA.
            nh = len(hs)
            lgs = pv.tile([P, nh, 10], F32, tag="mpd", bufs=mpd_bufs)
            for j in range(nh):
                jsl = slice(j * P, (j + 1) * P)
                for c in range(6):
                    nc.tensor.matmul(lgs[:, j, :], d2a[:, c, jsl], ow_w[:, c, :],
                                     start=(c == 0), stop=False)
                nc.tensor.matmul(lgs[:, j, :], d2b[:, jsl], ow_w[0:17, 6, :],
                                 start=False, stop=True)
            e10 = sm.tile([P, nh, 10], F32, tag="e10")
            nc.scalar.activation(e10, lgs, AF.Exp)
            s10 = sm.tile([P, nh], F32, tag="s10")
            nc.vector.reduce_sum(s10, e10, axis=mybir.AxisListType.X)
            r10 = sm.tile([P, nh], F32, tag="r10")
            nc.vector.reciprocal(r10, s10)
            o10 = sm.tile([P, nh, 10], F32, tag="o10")
            nc.vector.tensor_tensor(
                out=o10, in0=e10,
                in1=_ap(r10, [r10[:].ap[0], [1, nh], [0, 10]]),
                op=ALU.mult,
            )
            base = out_d[ds(g * (nsub * P) + goff * P, P), :]
            oap = bass.AP(tensor=base.tensor, offset=base.offset,
                          ap=[base.ap[0], [10 * P, nh], [1, 10]])
            nc.sync.dma_start(out=oap, in_=o10)

        def body_capsule(g):
            hs = []
            for j in range(nsub):
                h0 = hp.tile([P, T, SLOT], BF, tag=f"h{j}")
                nc.sync.dma_start(
                    out=h0[:, :, 0:FV],
                    in_=x_d[ds(g * (nsub * P) + j * P, P), :].rearrange(
                        "p (t f) -> p t f", t=T
                    ),
                )
                nc.gpsimd.tensor_copy(h0[:, :, FV:SLOT], ones_c[:, 0:T])
                hs.append(h0)
            for it in range(8):
                nxts = []
                for j in range(nsub):
                    if it < 7:
                        h_nxt = hp.tile([P, T, SLOT], BF, tag=f"h{j}")
                    else:
                        h_nxt = hp.tile([P, T, SLOT], BF, tag=f"hold{j}",
                                        bufs=2)
                    nxts.append(h_nxt)
                if PHASE_MAJOR:
                    zus = [capsule_psum(hs[j], j) for j in range(nsub)]
                    for j in range(nsub):
                        capsule_vec(hs[j], nxts[j], zus[j], j)
                else:
                    for j in range(nsub):
                        capsule_iter(hs[j], nxts[j], j)
                hs = list(nxts)
            return hs

        def body(g):
            hs = body_capsule(g)
            for d0 in range(0, nsub, 4):
                decoder(hs[d0 : d0 + 4], g, d0)

        if ngroups == 1:
            body(0)
        elif unroll:
            # software-pipelined: decode group g-1 while computing group g
            holds = [body_capsule(0)]
            for g in range(1, ngroups):
                holds.append(body_capsule(g))
                for d0 in range(0, nsub, 4):
                    decoder(holds[g - 1][d0 : d0 + 4], g - 1, d0)
            for d0 in range(0, nsub, 4):
                decoder(holds[-1][d0 : d0 + 4], ngroups - 1, d0)
        else:
            with tc.For_i(0, ngroups, 1) as g:
                body(g)
        for _pool in (pv, pp, sm, wkd, wk, hp, consts):
            _pool.release()

    nc.compile()
    return nc


def pack_weights(W1, b1, W2, b2, W3, b3, Wd1, bd1, Wd2, bd2, Wo, bo):
    f64 = np.float64
    W1, b1, W2, b2, W3, b3 = (np.asarray(t, f64) for t in (W1, b1, W2, b2, W3, b3))
    G = W1.T @ W2
    a = W2.T @ b1
    c = W1.T @ b2
    d = float(b1 @ b2)

    zu = np.zeros((P, 2, ZSLOT), np.float32)
    full = np.zeros((197, ZSLOT), f64)
    full[:196, :196] = G.T
    full[:196, 196] = a
    full[:196, SLOT:] = W3.T
    full[196, :196] = c
    full[196, 196] = d
    full[196, SLOT:] = b3
    zu[:, 0, :] = full[0:128]
    zu[0:69, 1, :] = full[128:197]

    d1 = np.zeros((P, 8, FEAT), np.float32)
    W1T = np.asarray(Wd1, f64).T  # [784 f_in, 784 j]
    for t in range(T):
        d1[:, t, :] = W1T[t * FV : t * FV + P, :]
        d1[0:68, 4 + t, :] = W1T[t * FV + P : (t + 1) * FV, :]
    d1[68, 4, :] = np.asarray(bd1, f64)

    d2 = np.zeros((P, 7, FEAT), np.float32)
    W2T = np.asarray(Wd2, f64).T
    for cidx in range(6):
        d2[:, cidx, :] = W2T[cidx * P : (cidx + 1) * P, :]
    d2[0:16, 6, :] = W2T[768:784, :]
    d2[16, 6, :] = np.asarray(bd2, f64)

    ow = np.zeros((P, 7, 10), np.float32)
    WoT = np.asarray(Wo, f64).T
    for cidx in range(6):
        ow[:, cidx, :] = WoT[cidx * P : (cidx + 1) * P, :]
    ow[0:16, 6, :] = WoT[768:784, :]
    ow[16, 6, :] = np.asarray(bo, f64)
    return (zu.astype(ml_dtypes.bfloat16), d1.astype(ml_dtypes.bfloat16),
            d2.astype(ml_dtypes.bfloat16), ow.astype(ml_dtypes.bfloat16))


_NC_CACHE = {}


def kernel(**inputs):
    x = np.ascontiguousarray(np.asarray(inputs["x"], np.float32)).astype(
        ml_dtypes.bfloat16
    )
    zu, d1, d2, ow = pack_weights(
        inputs["W1"], inputs["b1"], inputs["W2"], inputs["b2"], inputs["W3"],
        inputs["b3"], inputs["Wd1"], inputs["bd1"], inputs["Wd2"],
        inputs["bd2"], inputs["Wo"], inputs["bo"],
    )
    if "nc" not in _NC_CACHE:
        _NC_CACHE["nc"] = build(8, 4, unroll=True)
    nc = _NC_CACHE["nc"]
    bpc = B // NCORES
    in_maps = [
        {
            "x": x[c * bpc : (c + 1) * bpc],
            "zu_w": zu,
            "dec1_w": d1,
            "dec2_w": d2,
            "out_w": ow,
        }
        for c in range(NCORES)
    ]
    res = run_bass_kernel_spmd(nc, in_maps, core_ids=list(range(NCORES)))
    return np.concatenate([res.results[c]["out"] for c in range(NCORES)], axis=0)


# revision 7
# speedup vs baseline: 1.0935x; 1.0450x over previous
"""Trainium2 Bass kernel for nn_CapsuleNeuralNetworkV2 (8 cores, data-parallel).

Math (per sample, 8 capsule iterations then decoder):
  v = h.reshape(4, 196)
  q = v @ W1.T + b1 ; k = v @ W2.T + b2 ; u = v @ W3.T + b3
  scores[t,s] = q_t . k_s  ->  softmax over s -> h'_t = sum_s P[t,s] u_s
  dec = relu(h Wd1.T + bd1) Wd2.T + bd2 ; out = softmax(dec Wo.T + bo)

Host-side algebra:
  scores[t,s] = v_t . z_s + r_s,  z_s = G v_s + c, r_s = a.v_s + d,
  G = W1.T W2, a = W2.T b1, c = W1.T b2, d = b1.b2; biases fused via an
  augmented ones row so one matmul emits [z | r | u] per slot.

v3 layout/schedule changes vs v2:
  - h slot is 197 wide ([v(196) | 1]); zu slot is 393 ([z | r | u]); the
    dots run over 197 elems and pick up r via the ones column.
  - zu matmuls write two 2-slot PSUM tiles; each is evacuated by ONE Act
    copy (f32->bf16), replacing four per-slot copies.
  - vt transposes write one bf16 PSUM tile; two DVE 2x copies evacuate.
  - dots are 16 STT+accum ops, combine is 4 chains of (seed + 3 MACs);
    both are spread across DVE/Act/Pool by a static table tuned against
    the instruction cost model.
  - probs = e * (1/sum) via one broadcast tensor_tensor.
"""

import numpy as np
import ml_dtypes

import concourse.bass as bass
import concourse.tile as tile
from concourse import bacc, mybir
from concourse.bass import ds
from concourse.bass_utils import run_bass_kernel_spmd
from concourse.masks import make_identity

FR = mybir.dt.float32r
BF = mybir.dt.bfloat16
F32 = mybir.dt.float32
AF = mybir.ActivationFunctionType
ALU = mybir.AluOpType

B = 32768
NCORES = 8
P = 128
T = 4
FV = 196
FEAT = 784
SLOT = FV + 1  # 197: 196 data + ones col
ZSLOT = 2 * FV + 1  # 393: z(196) | r(1) | u(196)

# --- static engine tables (v=DVE, a=Act, p=Pool), tuned vs cost model ---
# dots[t][s] (Act cannot run STT)
DOTS_ENG = [
    "vp..",
    "pp..",
    "pp..",
    "pp..",
]
# combine: SEED_ENG[t] + MAC_ENG[t] (3 chained MACs; MACs only v or p)
# combine mul engine per (t, s); adds are two batched DVE tensor_tensor
MUL_ENG = [
    "aapp",
    "aapp",
    "aapp",
    "appp",
]
# vt-evac engines for (chunk1, chunk2)
VT_ENG = "a"
# s-slots computed via a DVE products TT + 4 cheap TSP-accums
PROD_S = (2, 3)
ADDS_PER_T = False
ZU0_B2 = False
PHASE_MAJOR = False
PAIR_VEC = False
ZU_BUFS = 5
D2_ACT = True
EXP_ACCUM = False
EXP_SPLIT = True


def _ap(t, dims, offset_elems=0):
    a = t[:] if hasattr(t, "tile") or not isinstance(t, bass.AP) else t
    return bass.AP(tensor=a.tensor, offset=a.offset + offset_elems, ap=dims)


def build(nsub=8, ngroups=4, unroll=False, zu_bufs=1, vtps_bufs=2, mpd_bufs=2,
          h_bufs=3, wk_bufs=5, sm_bufs=8, zu_mode="half", hf_bufs=1):
    """One NeuronCore program processing nsub*ngroups*128 samples."""
    bpc = nsub * ngroups * P
    nc = bacc.Bacc("TRN2", target_bir_lowering=False, debug=False)

    x_d = nc.dram_tensor("x", [bpc, FEAT], BF, kind="ExternalInput")
    zu_d = nc.dram_tensor("zu_w", [P, 2, ZSLOT], BF, kind="ExternalInput")
    d1_d = nc.dram_tensor("dec1_w", [P, 8, FEAT], BF, kind="ExternalInput")
    d2_d = nc.dram_tensor("dec2_w", [P, 7, FEAT], BF, kind="ExternalInput")
    ow_d = nc.dram_tensor("out_w", [P, 7, 10], BF, kind="ExternalInput")
    out_d = nc.dram_tensor("out", [bpc, 10], F32, kind="ExternalOutput")

    with tile.TileContext(nc) as tc:
        consts = tc.alloc_tile_pool(name="consts", bufs=1)
        hp = tc.alloc_tile_pool(name="h", bufs=h_bufs)
        wk = tc.alloc_tile_pool(name="wk", bufs=wk_bufs)
        wkd = tc.alloc_tile_pool(name="wkd", bufs=1)
        sm = tc.alloc_tile_pool(name="small", bufs=sm_bufs)
        pp = tc.alloc_tile_pool(name="ps", bufs=zu_bufs, space="PSUM")
        pv = tc.alloc_tile_pool(name="pv", bufs=vtps_bufs, space="PSUM")

        ident_f = consts.tile([P, P], F32)
        make_identity(nc, ident_f)
        ident = consts.tile([P, P], FR)
        nc.vector.tensor_copy(ident, ident_f)
        ident_b = consts.tile([P, P], BF)
        nc.vector.tensor_copy(ident_b, ident_f)
        ones_c = consts.tile([P, 512], F32)
        nc.vector.memset(ones_c, 1.0)
        zu_w = consts.tile([P, 2, ZSLOT], BF)
        nc.sync.dma_start(out=zu_w, in_=zu_d[:, :, :])
        d1_w = consts.tile([P, 8, FEAT], BF)
        nc.sync.dma_start(out=d1_w, in_=d1_d[:, :, :])
        d2_w = consts.tile([P, 7, FEAT], BF)
        nc.sync.dma_start(out=d2_w, in_=d2_d[:, :, :])
        ow_w = consts.tile([P, 7, 10], BF)
        nc.sync.dma_start(out=ow_w, in_=ow_d[:, :, :])

        def eng(c):
            return {"v": nc.vector, "p": nc.gpsimd, "a": nc.scalar}[c]

        def capsule_psum(h_cur, j):
            """PE transposes + zu matmuls + evacuations -> zu SBUF tile."""
            # chunk2 is a full 128-row transpose of features 69..196
            # (overlap rows 69..127 are zeroed in the chunk-1 weights), so
            # one evacuation op covers both chunks.
            vt_ps = pv.tile([P, T, 2, P], BF, tag="vtps")
            for t in range(T):
                nc.tensor.transpose(vt_ps[:, t, 0, :], h_cur[:, t, 0:P], ident_b)
                # includes the ones column -> row 127 is 1.0
                nc.tensor.transpose(vt_ps[:, t, 1, :],
                                    h_cur[:, t, 69 : 69 + P], ident_b)
            vt = wk.tile([P, T, 2, P], BF, tag="vt")
            c = VT_ENG[0]
            if c == "a":
                nc.scalar.copy(vt, vt_ps)
            elif c == "v":
                nc.vector.tensor_copy(vt, vt_ps)
            else:
                nc.gpsimd.tensor_copy(vt, vt_ps)

            zu = wk.tile([P, T, ZSLOT], BF, tag="zu", bufs=ZU_BUFS)
            for half in range(2):
                # 512-wide slots keep each matmul within one PSUM bank
                zu_ps = pp.tile([P, 2, 512], F32, tag=f"zu{half}")
                for k in range(2):
                    s = 2 * half + k
                    nc.tensor.matmul(zu_ps[:, k, 0:ZSLOT], vt[:, s, 0, :],
                                     zu_w[:, 0, :], start=True, stop=False)
                    nc.tensor.matmul(zu_ps[:, k, 0:ZSLOT], vt[:, s, 1, :],
                                     zu_w[:, 1, :], start=False, stop=True)
                nc.scalar.copy(zu[:, 2 * half : 2 * half + 2, :],
                               zu_ps[:, :, 0:ZSLOT])
            return zu

        def capsule_vec(h_cur, h_nxt, zu, j):
            """SBUF-only: dots, softmax, combine -> h_nxt."""
            dots = sm.tile([P, T, T], F32, tag="dots")
            scr = sm.tile([P, 3, SLOT], BF, tag="scr", bufs=8)
            for half in range(2):
                prod = sm.tile([P, 2, T, SLOT], BF, tag=f"prod{half}", bufs=2)
                hin = _ap(h_cur, [h_cur[:].ap[0], [0, 2], [SLOT, T], [1, SLOT]])
                zin = _ap(zu, [zu[:].ap[0], [ZSLOT, 2], [0, T], [1, SLOT]],
                          offset_elems=half * 2 * ZSLOT)
                nc.vector.tensor_tensor(out=prod, in0=hin, in1=zin, op=ALU.mult)
                for k in range(2):
                    s = 2 * half + k
                    for t in range(T):
                        nc.vector.tensor_scalar(
                            out=scr[:, 0, :],
                            in0=prod[:, k, t, :], scalar1=1.0,
                            scalar2=0.0, op0=ALU.mult, op1=ALU.add,
                            accum_out=dots[:, t, s : s + 1])

            # softmax over s (no max subtraction; |scores| < 30)
            e_t = sm.tile([P, T, T], F32, tag="e")
            sums = sm.tile([P, T], F32, tag="sums")
            if EXP_ACCUM:
                for t in range(T):
                    nc.scalar.activation(e_t[:, t, :], dots[:, t, :], AF.Exp,
                                         accum_out=sums[:, t : t + 1])
            else:
                if EXP_SPLIT:
                    nc.scalar.activation(e_t[:, :, 0:2], dots[:, :, 0:2],
                                         AF.Exp)
                    nc.scalar.activation(e_t[:, :, 2:4], dots[:, :, 2:4],
                                         AF.Exp)
                else:
                    nc.scalar.activation(e_t, dots, AF.Exp)
                nc.vector.reduce_sum(sums, e_t, axis=mybir.AxisListType.X)
            rec = sm.tile([P, T], F32, tag="rec")
            nc.vector.reciprocal(rec, sums)
            probs = sm.tile([P, T, T], F32, tag="probs")
            nc.vector.tensor_tensor(
                out=probs, in0=e_t,
                in1=_ap(rec, [rec[:].ap[0], [1, T], [0, T]]),
                op=ALU.mult,
            )

            # ones column for the next h
            nc.gpsimd.tensor_copy(h_nxt[:, :, FV:SLOT], ones_c[:, 0:T])
            # combine: pu[t,s] = P[t,s] * u_s, then two batched DVE adds
            pu = sm.tile([P, T, T, FV], BF, tag="pu", bufs=2)
            for t in range(T):
                for s in range(T):
                    c = MUL_ENG[t][s]
                    if c == "a":
                        nc.scalar.activation(
                            pu[:, t, s, :], zu[:, s, SLOT:ZSLOT], AF.Copy,
                            scale=probs[:, t, s : s + 1])
                    elif c == "v":
                        nc.vector.tensor_scalar_mul(
                            pu[:, t, s, :], zu[:, s, SLOT:ZSLOT],
                            probs[:, t, s : s + 1])
                    else:
                        nc.gpsimd.tensor_scalar_mul(
                            pu[:, t, s, :], zu[:, s, SLOT:ZSLOT],
                            probs[:, t, s : s + 1])
            q = sm.tile([P, T, 2, FV], BF, tag="q", bufs=2)
            ev = _ap(pu, [pu[:].ap[0], [T * FV, T], [2 * FV, 2], [1, FV]])
            od = _ap(pu, [pu[:].ap[0], [T * FV, T], [2 * FV, 2], [1, FV]],
                     offset_elems=FV)
            nc.vector.tensor_tensor(out=q, in0=ev, in1=od, op=ALU.add)
            nc.vector.tensor_tensor(out=h_nxt[:, :, 0:FV], in0=q[:, :, 0, :],
                                    in1=q[:, :, 1, :], op=ALU.add)

        def capsule_vec_pair(hc, hn, zus):
            """Two tiles' dots/softmax/combine with pair-batched softmax."""
            npair = len(hc)
            dots = sm.tile([P, 2, T, T], F32, tag="dots")
            scr = sm.tile([P, 3, SLOT], BF, tag="scr", bufs=8)
            for jj in range(npair):
                for half in range(2):
                    prod = sm.tile([P, 2, T, SLOT], BF, tag=f"prod{half}",
                                   bufs=2)
                    hin = _ap(hc[jj], [hc[jj][:].ap[0], [0, 2], [SLOT, T],
                                       [1, SLOT]])
                    zin = _ap(zus[jj], [zus[jj][:].ap[0], [ZSLOT, 2], [0, T],
                                        [1, SLOT]],
                              offset_elems=half * 2 * ZSLOT)
                    nc.vector.tensor_tensor(out=prod, in0=hin, in1=zin,
                                            op=ALU.mult)
                    for k in range(2):
                        s = 2 * half + k
                        for t in range(T):
                            nc.vector.tensor_scalar(
                                out=scr[:, 0, :],
                                in0=prod[:, k, t, :], scalar1=1.0,
                                scalar2=0.0, op0=ALU.mult, op1=ALU.add,
                                accum_out=dots[:, jj, t, s : s + 1])

            # pair-batched softmax (no max subtraction; |scores| < 30)
            e_t = sm.tile([P, 2, T, T], F32, tag="e")
            nc.scalar.activation(e_t[:, 0:npair], dots[:, 0:npair], AF.Exp)
            sums = sm.tile([P, 2, T], F32, tag="sums")
            nc.vector.reduce_sum(sums[:, 0:npair], e_t[:, 0:npair],
                                 axis=mybir.AxisListType.X)
            rec = sm.tile([P, 2, T], F32, tag="rec")
            nc.vector.reciprocal(rec[:, 0:npair], sums[:, 0:npair])
            probs = sm.tile([P, 2, T, T], F32, tag="probs")
            nc.vector.tensor_tensor(
                out=probs[:, 0:npair], in0=e_t[:, 0:npair],
                in1=_ap(rec, [rec[:].ap[0], [T, npair], [1, T], [0, T]]),
                op=ALU.mult,
            )

            for jj in range(npair):
                nc.gpsimd.tensor_copy(hn[jj][:, :, FV:SLOT], ones_c[:, 0:T])
                pu = sm.tile([P, T, T, FV], BF, tag="pu", bufs=2)
                for t in range(T):
                    for s in range(T):
                        c = MUL_ENG[t][s]
                        if c == "a":
                            nc.scalar.activation(
                                pu[:, t, s, :], zus[jj][:, s, SLOT:ZSLOT],
                                AF.Copy, scale=probs[:, jj, t, s : s + 1])
                        elif c == "v":
                            nc.vector.tensor_scalar_mul(
                                pu[:, t, s, :], zus[jj][:, s, SLOT:ZSLOT],
                                probs[:, jj, t, s : s + 1])
                        else:
                            nc.gpsimd.tensor_scalar_mul(
                                pu[:, t, s, :], zus[jj][:, s, SLOT:ZSLOT],
                                probs[:, jj, t, s : s + 1])
                q = sm.tile([P, T, 2, FV], BF, tag="q", bufs=2)
                ev = _ap(pu, [pu[:].ap[0], [T * FV, T], [2 * FV, 2], [1, FV]])
                od = _ap(pu, [pu[:].ap[0], [T * FV, T], [2 * FV, 2], [1, FV]],
                         offset_elems=FV)
                nc.vector.tensor_tensor(out=q, in0=ev, in1=od, op=ALU.add)
                nc.vector.tensor_tensor(out=hn[jj][:, :, 0:FV],
                                        in0=q[:, :, 0, :],
                                        in1=q[:, :, 1, :], op=ALU.add)

        def capsule_iter(h_cur, h_nxt, j):
            zu = capsule_psum(h_cur, j)
            capsule_vec(h_cur, h_nxt, zu, j)

        def decoder(hs, g, goff=0):
            """Decoder over a chunk of <=4 tiles (N = len(hs)*128 wide)."""
            W = len(hs) * P
            # h.T chunks, slot-major: [128] x4 and [69] x4 (with ones row)
            ht1 = wkd.tile([P, T, W], BF, tag="ht1")
            ht2 = wkd.tile([69, T, W], BF, tag="ht2")
            for t in range(T):
                t1_ps = pv.tile([P, W], BF, tag="vtps")
                t2_ps = pv.tile([69, W], BF, tag="vtps")
                for j in range(len(hs)):
                    nc.tensor.transpose(
                        t1_ps[:, j * P : (j + 1) * P], hs[j][:, t, 0:P], ident_b
                    )
                    nc.tensor.transpose(
                        t2_ps[:, j * P : (j + 1) * P],
                        hs[j][:, t, P : P + 69], ident_b
                    )
                nc.scalar.copy(ht1[:, t, :], t1_ps)
                nc.vector.tensor_copy(ht2[:, t, :], t2_ps)

            # dec1 = relu(Wd1 @ h.T + bd1), feature-major, 7 M-chunks
            d1a = wkd.tile([P, 6, W], BF, tag="d1a")
            d1b = wkd.tile([17, W], BF, tag="d1b")
            nc.vector.tensor_copy(d1b, ones_c[0:17, 0:W])
            for m in range(7):
                mw = min(P, FEAT - m * P)
                mp = pv.tile([P, W], F32, tag="mpd", bufs=mpd_bufs)
                msl = slice(m * P, m * P + mw)
                for t in range(T):
                    nc.tensor.matmul(mp[0:mw, :], d1_w[:, t, msl], ht1[:, t, :],
                                     start=(t == 0), stop=False)
                for t in range(T):
                    nc.tensor.matmul(mp[0:mw, :], d1_w[0:69, 4 + t, msl],
                                     ht2[:, t, :], start=False, stop=(t == 3))
                if m < 6:
                    nc.scalar.activation(d1a[:, m, :], mp, AF.Relu)
                else:
                    nc.scalar.activation(d1b[0:16, :], mp[0:16, :], AF.Relu)

            # dec2 = Wd2 @ relu1 + bd2, feature-major
            d2a = wkd.tile([P, 6, W], BF, tag="d2a")
            d2b = wkd.tile([17, W], BF, tag="d2b")
            nc.vector.tensor_copy(d2b, ones_c[0:17, 0:W])
            for m in range(7):
                mw = min(P, FEAT - m * P)
                mp = pv.tile([P, W], F32, tag="mpd", bufs=mpd_bufs)
                msl = slice(m * P, m * P + mw)
                for c in range(6):
                    nc.tensor.matmul(mp[0:mw, :], d2_w[:, c, msl], d1a[:, c, :],
                                     start=(c == 0), stop=False)
                nc.tensor.matmul(mp[0:mw, :], d2_w[0:17, 6, msl], d1b,
                                 start=False, stop=True)
                if m < 6:
                    (nc.scalar.copy if D2_ACT else nc.vector.tensor_copy)(
                        d2a[:, m, :], mp)
                else:
                    (nc.scalar.copy if D2_ACT else nc.vector.tensor_copy)(
                        d2b[0:16, :], mp[0:16, :])

            # logits for all subtiles into one PSUM tile, then one
            # batched softmax (no max subtraction; |logits| < 30) and a
            # single strided output DMA.
            nh = len(hs)
            lgs = pv.tile([P, nh, 10], F32, tag="mpd", bufs=mpd_bufs)
            for j in range(nh):
                jsl = slice(j * P, (j + 1) * P)
                for c in range(6):
                    nc.tensor.matmul(lgs[:, j, :], d2a[:, c, jsl], ow_w[:, c, :],
                                     start=(c == 0), stop=False)
                nc.tensor.matmul(lgs[:, j, :], d2b[:, jsl], ow_w[0:17, 6, :],
                                 start=False, stop=True)
            e10 = sm.tile([P, nh, 10], F32, tag="e10")
            nc.scalar.activation(e10, lgs, AF.Exp)
            s10 = sm.tile([P, nh], F32, tag="s10")
            nc.vector.reduce_sum(s10, e10, axis=mybir.AxisListType.X)
            r10 = sm.tile([P, nh], F32, tag="r10")
            nc.vector.reciprocal(r10, s10)
            o10 = sm.tile([P, nh, 10], F32, tag="o10")
            nc.vector.tensor_tensor(
                out=o10, in0=e10,
                in1=_ap(r10, [r10[:].ap[0], [1, nh], [0, 10]]),
                op=ALU.mult,
            )
            base = out_d[ds(g * (nsub * P) + goff * P, P), :]
            oap = bass.AP(tensor=base.tensor, offset=base.offset,
                          ap=[base.ap[0], [10 * P, nh], [1, 10]])
            nc.sync.dma_start(out=oap, in_=o10)

        def body_capsule(g):
            hs = []
            for j in range(nsub):
                h0 = hp.tile([P, T, SLOT], BF, tag=f"h{j}")
                nc.sync.dma_start(
                    out=h0[:, :, 0:FV],
                    in_=x_d[ds(g * (nsub * P) + j * P, P), :].rearrange(
                        "p (t f) -> p t f", t=T
                    ),
                )
                nc.gpsimd.tensor_copy(h0[:, :, FV:SLOT], ones_c[:, 0:T])
                hs.append(h0)
            for it in range(8):
                nxts = []
                for j in range(nsub):
                    if it < 7:
                        h_nxt = hp.tile([P, T, SLOT], BF, tag=f"h{j}")
                    else:
                        h_nxt = hp.tile([P, T, SLOT], BF, tag=f"hold{j}",
                                        bufs=2)
                    nxts.append(h_nxt)
                if PAIR_VEC:
                    for j0 in range(0, nsub, 2):
                        zus = [capsule_psum(hs[j], j)
                               for j in range(j0, min(j0 + 2, nsub))]
                        capsule_vec_pair(hs[j0 : j0 + 2], nxts[j0 : j0 + 2],
                                         zus)
                elif PHASE_MAJOR:
                    zus = [capsule_psum(hs[j], j) for j in range(nsub)]
                    for j in range(nsub):
                        capsule_vec(hs[j], nxts[j], zus[j], j)
                else:
                    for j in range(nsub):
                        capsule_iter(hs[j], nxts[j], j)
                hs = list(nxts)
            return hs

        def body(g):
            hs = body_capsule(g)
            for d0 in range(0, nsub, 4):
                decoder(hs[d0 : d0 + 4], g, d0)

        if ngroups == 1:
            body(0)
        elif unroll:
            # software-pipelined: decode group g-1 while computing group g
            holds = [body_capsule(0)]
            for g in range(1, ngroups):
                holds.append(body_capsule(g))
                for d0 in range(0, nsub, 4):
                    decoder(holds[g - 1][d0 : d0 + 4], g - 1, d0)
            for d0 in range(0, nsub, 4):
                decoder(holds[-1][d0 : d0 + 4], ngroups - 1, d0)
        else:
            with tc.For_i(0, ngroups, 1) as g:
                body(g)
        for _pool in (pv, pp, sm, wkd, wk, hp, consts):
            _pool.release()

    nc.compile()
    return nc


def pack_weights(W1, b1, W2, b2, W3, b3, Wd1, bd1, Wd2, bd2, Wo, bo):
    f64 = np.float64
    W1, b1, W2, b2, W3, b3 = (np.asarray(t, f64) for t in (W1, b1, W2, b2, W3, b3))
    G = W1.T @ W2
    a = W2.T @ b1
    c = W1.T @ b2
    d = float(b1 @ b2)

    zu = np.zeros((P, 2, ZSLOT), np.float32)
    full = np.zeros((197, ZSLOT), f64)
    full[:196, :196] = G.T
    full[:196, 196] = a
    full[:196, SLOT:] = W3.T
    full[196, :196] = c
    full[196, 196] = d
    full[196, SLOT:] = b3
    zu[:, 0, :] = full[0:128]
    zu[69:128, 0, :] = 0.0
    zu[:, 1, :] = full[69:197]

    d1 = np.zeros((P, 8, FEAT), np.float32)
    W1T = np.asarray(Wd1, f64).T  # [784 f_in, 784 j]
    for t in range(T):
        d1[:, t, :] = W1T[t * FV : t * FV + P, :]
        d1[0:68, 4 + t, :] = W1T[t * FV + P : (t + 1) * FV, :]
    d1[68, 4, :] = np.asarray(bd1, f64)

    d2 = np.zeros((P, 7, FEAT), np.float32)
    W2T = np.asarray(Wd2, f64).T
    for cidx in range(6):
        d2[:, cidx, :] = W2T[cidx * P : (cidx + 1) * P, :]
    d2[0:16, 6, :] = W2T[768:784, :]
    d2[16, 6, :] = np.asarray(bd2, f64)

    ow = np.zeros((P, 7, 10), np.float32)
    WoT = np.asarray(Wo, f64).T
    for cidx in range(6):
        ow[:, cidx, :] = WoT[cidx * P : (cidx + 1) * P, :]
    ow[0:16, 6, :] = WoT[768:784, :]
    ow[16, 6, :] = np.asarray(bo, f64)
    return (zu.astype(ml_dtypes.bfloat16), d1.astype(ml_dtypes.bfloat16),
            d2.astype(ml_dtypes.bfloat16), ow.astype(ml_dtypes.bfloat16))


_NC_CACHE = {}


def kernel(**inputs):
    x = np.ascontiguousarray(np.asarray(inputs["x"], np.float32)).astype(
        ml_dtypes.bfloat16
    )
    zu, d1, d2, ow = pack_weights(
        inputs["W1"], inputs["b1"], inputs["W2"], inputs["b2"], inputs["W3"],
        inputs["b3"], inputs["Wd1"], inputs["bd1"], inputs["Wd2"],
        inputs["bd2"], inputs["Wo"], inputs["bo"],
    )
    if "nc" not in _NC_CACHE:
        _NC_CACHE["nc"] = build(8, 4, unroll=True)
    nc = _NC_CACHE["nc"]
    bpc = B // NCORES
    in_maps = [
        {
            "x": x[c * bpc : (c + 1) * bpc],
            "zu_w": zu,
            "dec1_w": d1,
            "dec2_w": d2,
            "out_w": ow,
        }
        for c in range(NCORES)
    ]
    res = run_bass_kernel_spmd(nc, in_maps, core_ids=list(range(NCORES)))
    return np.concatenate([res.results[c]["out"] for c in range(NCORES)], axis=0)


# revision 8
# speedup vs baseline: 1.0953x; 1.0017x over previous
"""Trainium2 Bass kernel for nn_CapsuleNeuralNetworkV2 (8 cores, data-parallel).

Math (per sample, 8 capsule iterations then decoder):
  v = h.reshape(4, 196)
  q = v @ W1.T + b1 ; k = v @ W2.T + b2 ; u = v @ W3.T + b3
  scores[t,s] = q_t . k_s  ->  softmax over s -> h'_t = sum_s P[t,s] u_s
  dec = relu(h Wd1.T + bd1) Wd2.T + bd2 ; out = softmax(dec Wo.T + bo)

Host-side algebra:
  scores[t,s] = v_t . z_s + r_s,  z_s = G v_s + c, r_s = a.v_s + d,
  G = W1.T W2, a = W2.T b1, c = W1.T b2, d = b1.b2; biases fused via an
  augmented ones row so one matmul emits [z | r | u] per slot.

v3 layout/schedule changes vs v2:
  - h slot is 197 wide ([v(196) | 1]); zu slot is 393 ([z | r | u]); the
    dots run over 197 elems and pick up r via the ones column.
  - zu matmuls write two 2-slot PSUM tiles; each is evacuated by ONE Act
    copy (f32->bf16), replacing four per-slot copies.
  - vt transposes write one bf16 PSUM tile; two DVE 2x copies evacuate.
  - dots are 16 STT+accum ops, combine is 4 chains of (seed + 3 MACs);
    both are spread across DVE/Act/Pool by a static table tuned against
    the instruction cost model.
  - probs = e * (1/sum) via one broadcast tensor_tensor.
"""

import numpy as np
import ml_dtypes

import concourse.bass as bass
import concourse.tile as tile
from concourse import bacc, mybir
from concourse.bass import ds
from concourse.bass_utils import run_bass_kernel_spmd
from concourse.masks import make_identity

FR = mybir.dt.float32r
BF = mybir.dt.bfloat16
F32 = mybir.dt.float32
AF = mybir.ActivationFunctionType
ALU = mybir.AluOpType

B = 32768
NCORES = 8
P = 128
T = 4
FV = 196
FEAT = 784
SLOT = FV + 1  # 197: 196 data + ones col
ZSLOT = 2 * FV + 1  # 393: z(196) | r(1) | u(196)

# --- static engine tables (v=DVE, a=Act, p=Pool), tuned vs cost model ---
# dots[t][s] (Act cannot run STT)
DOTS_ENG = [
    "vp..",
    "pp..",
    "pp..",
    "pp..",
]
# combine: SEED_ENG[t] + MAC_ENG[t] (3 chained MACs; MACs only v or p)
# combine mul engine per (t, s); adds are two batched DVE tensor_tensor
MUL_ENG = [
    "vapp",
    "aapp",
    "aapp",
    "aapp",
]
# vt-evac engines for (chunk1, chunk2)
VT_ENG = "a"
# s-slots computed via a DVE products TT + 4 cheap TSP-accums
PROD_S = (2, 3)
ADDS_PER_T = False
ZU0_B2 = False
PHASE_MAJOR = False
PAIR_VEC = False
ZU_BUFS = 5
D2_ACT = True
EXP_ACCUM = False
EXP_SPLIT = True


def _ap(t, dims, offset_elems=0):
    a = t[:] if hasattr(t, "tile") or not isinstance(t, bass.AP) else t
    return bass.AP(tensor=a.tensor, offset=a.offset + offset_elems, ap=dims)


def build(nsub=8, ngroups=4, unroll=False, zu_bufs=1, vtps_bufs=2, mpd_bufs=2,
          h_bufs=3, wk_bufs=5, sm_bufs=8, zu_mode="half", hf_bufs=1):
    """One NeuronCore program processing nsub*ngroups*128 samples."""
    bpc = nsub * ngroups * P
    nc = bacc.Bacc("TRN2", target_bir_lowering=False, debug=False)

    x_d = nc.dram_tensor("x", [bpc, FEAT], BF, kind="ExternalInput")
    zu_d = nc.dram_tensor("zu_w", [P, 2, ZSLOT], BF, kind="ExternalInput")
    d1_d = nc.dram_tensor("dec1_w", [P, 8, FEAT], BF, kind="ExternalInput")
    d2_d = nc.dram_tensor("dec2_w", [P, 7, FEAT], BF, kind="ExternalInput")
    ow_d = nc.dram_tensor("out_w", [P, 7, 10], BF, kind="ExternalInput")
    out_d = nc.dram_tensor("out", [bpc, 10], F32, kind="ExternalOutput")

    with tile.TileContext(nc) as tc:
        consts = tc.alloc_tile_pool(name="consts", bufs=1)
        hp = tc.alloc_tile_pool(name="h", bufs=h_bufs)
        wk = tc.alloc_tile_pool(name="wk", bufs=wk_bufs)
        wkd = tc.alloc_tile_pool(name="wkd", bufs=1)
        sm = tc.alloc_tile_pool(name="small", bufs=sm_bufs)
        pp = tc.alloc_tile_pool(name="ps", bufs=zu_bufs, space="PSUM")
        pv = tc.alloc_tile_pool(name="pv", bufs=vtps_bufs, space="PSUM")

        ident_f = consts.tile([P, P], F32)
        make_identity(nc, ident_f)
        ident = consts.tile([P, P], FR)
        nc.vector.tensor_copy(ident, ident_f)
        ident_b = consts.tile([P, P], BF)
        nc.vector.tensor_copy(ident_b, ident_f)
        ones_c = consts.tile([P, 512], F32)
        nc.vector.memset(ones_c, 1.0)
        zu_w = consts.tile([P, 2, ZSLOT], BF)
        nc.sync.dma_start(out=zu_w, in_=zu_d[:, :, :])
        d1_w = consts.tile([P, 8, FEAT], BF)
        nc.sync.dma_start(out=d1_w, in_=d1_d[:, :, :])
        d2_w = consts.tile([P, 7, FEAT], BF)
        nc.sync.dma_start(out=d2_w, in_=d2_d[:, :, :])
        ow_w = consts.tile([P, 7, 10], BF)
        nc.sync.dma_start(out=ow_w, in_=ow_d[:, :, :])

        def eng(c):
            return {"v": nc.vector, "p": nc.gpsimd, "a": nc.scalar}[c]

        def capsule_psum(h_cur, j):
            """PE transposes + zu matmuls + evacuations -> zu SBUF tile."""
            # chunk2 is a full 128-row transpose of features 69..196
            # (overlap rows 69..127 are zeroed in the chunk-1 weights), so
            # one evacuation op covers both chunks.
            vt_ps = pv.tile([P, T, 2, P], BF, tag="vtps")
            for t in range(T):
                nc.tensor.transpose(vt_ps[:, t, 0, :], h_cur[:, t, 0:P], ident_b)
                # includes the ones column -> row 127 is 1.0
                nc.tensor.transpose(vt_ps[:, t, 1, :],
                                    h_cur[:, t, 69 : 69 + P], ident_b)
            vt = wk.tile([P, T, 2, P], BF, tag="vt")
            c = VT_ENG[0]
            if c == "a":
                nc.scalar.copy(vt, vt_ps)
            elif c == "v":
                nc.vector.tensor_copy(vt, vt_ps)
            else:
                nc.gpsimd.tensor_copy(vt, vt_ps)

            zu = wk.tile([P, T, ZSLOT], BF, tag="zu", bufs=ZU_BUFS)
            for half in range(2):
                # 512-wide slots keep each matmul within one PSUM bank
                zu_ps = pp.tile([P, 2, 512], F32, tag=f"zu{half}")
                for k in range(2):
                    s = 2 * half + k
                    nc.tensor.matmul(zu_ps[:, k, 0:ZSLOT], vt[:, s, 0, :],
                                     zu_w[:, 0, :], start=True, stop=False)
                    nc.tensor.matmul(zu_ps[:, k, 0:ZSLOT], vt[:, s, 1, :],
                                     zu_w[:, 1, :], start=False, stop=True)
                nc.scalar.copy(zu[:, 2 * half : 2 * half + 2, :],
                               zu_ps[:, :, 0:ZSLOT])
            return zu

        def capsule_vec(h_cur, h_nxt, zu, j):
            """SBUF-only: dots, softmax, combine -> h_nxt."""
            dots = sm.tile([P, T, T], F32, tag="dots")
            scr = sm.tile([P, 3, SLOT], BF, tag="scr", bufs=8)
            for half in range(2):
                prod = sm.tile([P, 2, T, SLOT], BF, tag=f"prod{half}", bufs=2)
                hin = _ap(h_cur, [h_cur[:].ap[0], [0, 2], [SLOT, T], [1, SLOT]])
                zin = _ap(zu, [zu[:].ap[0], [ZSLOT, 2], [0, T], [1, SLOT]],
                          offset_elems=half * 2 * ZSLOT)
                nc.vector.tensor_tensor(out=prod, in0=hin, in1=zin, op=ALU.mult)
                for k in range(2):
                    s = 2 * half + k
                    for t in range(T):
                        nc.vector.tensor_scalar(
                            out=scr[:, 0, :],
                            in0=prod[:, k, t, :], scalar1=1.0,
                            scalar2=0.0, op0=ALU.mult, op1=ALU.add,
                            accum_out=dots[:, t, s : s + 1])

            # softmax over s (no max subtraction; |scores| < 30)
            e_t = sm.tile([P, T, T], F32, tag="e")
            sums = sm.tile([P, T], F32, tag="sums")
            if EXP_ACCUM:
                for t in range(T):
                    nc.scalar.activation(e_t[:, t, :], dots[:, t, :], AF.Exp,
                                         accum_out=sums[:, t : t + 1])
            else:
                if EXP_SPLIT:
                    nc.scalar.activation(e_t[:, :, 0:2], dots[:, :, 0:2],
                                         AF.Exp)
                    nc.scalar.activation(e_t[:, :, 2:4], dots[:, :, 2:4],
                                         AF.Exp)
                else:
                    nc.scalar.activation(e_t, dots, AF.Exp)
                nc.vector.reduce_sum(sums, e_t, axis=mybir.AxisListType.X)
            rec = sm.tile([P, T], F32, tag="rec")
            nc.vector.reciprocal(rec, sums)
            probs = sm.tile([P, T, T], F32, tag="probs")
            nc.vector.tensor_tensor(
                out=probs, in0=e_t,
                in1=_ap(rec, [rec[:].ap[0], [1, T], [0, T]]),
                op=ALU.mult,
            )

            # ones column for the next h
            nc.gpsimd.tensor_copy(h_nxt[:, :, FV:SLOT], ones_c[:, 0:T])
            # combine: pu[t,s] = P[t,s] * u_s, then two batched DVE adds
            pu = sm.tile([P, T, T, FV], BF, tag="pu", bufs=2)
            for t in range(T):
                for s in range(T):
                    c = MUL_ENG[t][s]
                    if c == "a":
                        nc.scalar.activation(
                            pu[:, t, s, :], zu[:, s, SLOT:ZSLOT], AF.Copy,
                            scale=probs[:, t, s : s + 1])
                    elif c == "v":
                        nc.vector.tensor_scalar_mul(
                            pu[:, t, s, :], zu[:, s, SLOT:ZSLOT],
                            probs[:, t, s : s + 1])
                    else:
                        nc.gpsimd.tensor_scalar_mul(
                            pu[:, t, s, :], zu[:, s, SLOT:ZSLOT],
                            probs[:, t, s : s + 1])
            q = sm.tile([P, T, 2, FV], BF, tag="q", bufs=2)
            ev = _ap(pu, [pu[:].ap[0], [T * FV, T], [2 * FV, 2], [1, FV]])
            od = _ap(pu, [pu[:].ap[0], [T * FV, T], [2 * FV, 2], [1, FV]],
                     offset_elems=FV)
            nc.vector.tensor_tensor(out=q, in0=ev, in1=od, op=ALU.add)
            nc.vector.tensor_tensor(out=h_nxt[:, :, 0:FV], in0=q[:, :, 0, :],
                                    in1=q[:, :, 1, :], op=ALU.add)

        def capsule_vec_pair(hc, hn, zus):
            """Two tiles' dots/softmax/combine with pair-batched softmax."""
            npair = len(hc)
            dots = sm.tile([P, 2, T, T], F32, tag="dots")
            scr = sm.tile([P, 3, SLOT], BF, tag="scr", bufs=8)
            for jj in range(npair):
                for half in range(2):
                    prod = sm.tile([P, 2, T, SLOT], BF, tag=f"prod{half}",
                                   bufs=2)
                    hin = _ap(hc[jj], [hc[jj][:].ap[0], [0, 2], [SLOT, T],
                                       [1, SLOT]])
                    zin = _ap(zus[jj], [zus[jj][:].ap[0], [ZSLOT, 2], [0, T],
                                        [1, SLOT]],
                              offset_elems=half * 2 * ZSLOT)
                    nc.vector.tensor_tensor(out=prod, in0=hin, in1=zin,
                                            op=ALU.mult)
                    for k in range(2):
                        s = 2 * half + k
                        for t in range(T):
                            nc.vector.tensor_scalar(
                                out=scr[:, 0, :],
                                in0=prod[:, k, t, :], scalar1=1.0,
                                scalar2=0.0, op0=ALU.mult, op1=ALU.add,
                                accum_out=dots[:, jj, t, s : s + 1])

            # pair-batched softmax (no max subtraction; |scores| < 30)
            e_t = sm.tile([P, 2, T, T], F32, tag="e")
            nc.scalar.activation(e_t[:, 0:npair], dots[:, 0:npair], AF.Exp)
            sums = sm.tile([P, 2, T], F32, tag="sums")
            nc.vector.reduce_sum(sums[:, 0:npair], e_t[:, 0:npair],
                                 axis=mybir.AxisListType.X)
            rec = sm.tile([P, 2, T], F32, tag="rec")
            nc.vector.reciprocal(rec[:, 0:npair], sums[:, 0:npair])
            probs = sm.tile([P, 2, T, T], F32, tag="probs")
            nc.vector.tensor_tensor(
                out=probs[:, 0:npair], in0=e_t[:, 0:npair],
                in1=_ap(rec, [rec[:].ap[0], [T, npair], [1, T], [0, T]]),
                op=ALU.mult,
            )

            for jj in range(npair):
                nc.gpsimd.tensor_copy(hn[jj][:, :, FV:SLOT], ones_c[:, 0:T])
                pu = sm.tile([P, T, T, FV], BF, tag="pu", bufs=2)
                for t in range(T):
                    for s in range(T):
                        c = MUL_ENG[t][s]
                        if c == "a":
                            nc.scalar.activation(
                                pu[:, t, s, :], zus[jj][:, s, SLOT:ZSLOT],
                                AF.Copy, scale=probs[:, jj, t, s : s + 1])
                        elif c == "v":
                            nc.vector.tensor_scalar_mul(
                                pu[:, t, s, :], zus[jj][:, s, SLOT:ZSLOT],
                                probs[:, jj, t, s : s + 1])
                        else:
                            nc.gpsimd.tensor_scalar_mul(
                                pu[:, t, s, :], zus[jj][:, s, SLOT:ZSLOT],
                                probs[:, jj, t, s : s + 1])
                q = sm.tile([P, T, 2, FV], BF, tag="q", bufs=2)
                ev = _ap(pu, [pu[:].ap[0], [T * FV, T], [2 * FV, 2], [1, FV]])
                od = _ap(pu, [pu[:].ap[0], [T * FV, T], [2 * FV, 2], [1, FV]],
                         offset_elems=FV)
                nc.vector.tensor_tensor(out=q, in0=ev, in1=od, op=ALU.add)
                nc.vector.tensor_tensor(out=hn[jj][:, :, 0:FV],
                                        in0=q[:, :, 0, :],
                                        in1=q[:, :, 1, :], op=ALU.add)

        def capsule_iter(h_cur, h_nxt, j):
            zu = capsule_psum(h_cur, j)
            capsule_vec(h_cur, h_nxt, zu, j)

        def decoder(hs, g, goff=0):
            """Decoder over a chunk of <=4 tiles (N = len(hs)*128 wide)."""
            W = len(hs) * P
            # h.T chunks, slot-major: [128] x4 and [69] x4 (with ones row)
            ht1 = wkd.tile([P, T, W], BF, tag="ht1")
            ht2 = wkd.tile([69, T, W], BF, tag="ht2")
            for t in range(T):
                t1_ps = pv.tile([P, W], BF, tag="vtps")
                t2_ps = pv.tile([69, W], BF, tag="vtps")
                for j in range(len(hs)):
                    nc.tensor.transpose(
                        t1_ps[:, j * P : (j + 1) * P], hs[j][:, t, 0:P], ident_b
                    )
                    nc.tensor.transpose(
                        t2_ps[:, j * P : (j + 1) * P],
                        hs[j][:, t, P : P + 69], ident_b
                    )
                nc.scalar.copy(ht1[:, t, :], t1_ps)
                nc.vector.tensor_copy(ht2[:, t, :], t2_ps)

            # dec1 = relu(Wd1 @ h.T + bd1), feature-major, 7 M-chunks
            d1a = wkd.tile([P, 6, W], BF, tag="d1a")
            d1b = wkd.tile([17, W], BF, tag="d1b")
            nc.vector.tensor_copy(d1b, ones_c[0:17, 0:W])
            for m in range(7):
                mw = min(P, FEAT - m * P)
                mp = pv.tile([P, W], F32, tag="mpd", bufs=mpd_bufs)
                msl = slice(m * P, m * P + mw)
                for t in range(T):
                    nc.tensor.matmul(mp[0:mw, :], d1_w[:, t, msl], ht1[:, t, :],
                                     start=(t == 0), stop=False)
                for t in range(T):
                    nc.tensor.matmul(mp[0:mw, :], d1_w[0:69, 4 + t, msl],
                                     ht2[:, t, :], start=False, stop=(t == 3))
                if m < 6:
                    nc.scalar.activation(d1a[:, m, :], mp, AF.Relu)
                else:
                    nc.scalar.activation(d1b[0:16, :], mp[0:16, :], AF.Relu)

            # dec2 = Wd2 @ relu1 + bd2, feature-major
            d2a = wkd.tile([P, 6, W], BF, tag="d2a")
            d2b = wkd.tile([17, W], BF, tag="d2b")
            nc.vector.tensor_copy(d2b, ones_c[0:17, 0:W])
            for m in range(7):
                mw = min(P, FEAT - m * P)
                mp = pv.tile([P, W], F32, tag="mpd", bufs=mpd_bufs)
                msl = slice(m * P, m * P + mw)
                for c in range(6):
                    nc.tensor.matmul(mp[0:mw, :], d2_w[:, c, msl], d1a[:, c, :],
                                     start=(c == 0), stop=False)
                nc.tensor.matmul(mp[0:mw, :], d2_w[0:17, 6, msl], d1b,
                                 start=False, stop=True)
                if m < 6:
                    (nc.scalar.copy if D2_ACT else nc.vector.tensor_copy)(
                        d2a[:, m, :], mp)
                else:
                    (nc.scalar.copy if D2_ACT else nc.vector.tensor_copy)(
                        d2b[0:16, :], mp[0:16, :])

            # logits for all subtiles into one PSUM tile, then one
            # batched softmax (no max subtraction; |logits| < 30) and a
            # single strided output DMA.
            nh = len(hs)
            lgs = pv.tile([P, nh, 10], F32, tag="mpd", bufs=mpd_bufs)
            for j in range(nh):
                jsl = slice(j * P, (j + 1) * P)
                for c in range(6):
                    nc.tensor.matmul(lgs[:, j, :], d2a[:, c, jsl], ow_w[:, c, :],
                                     start=(c == 0), stop=False)
                nc.tensor.matmul(lgs[:, j, :], d2b[:, jsl], ow_w[0:17, 6, :],
                                 start=False, stop=True)
            e10 = sm.tile([P, nh, 10], F32, tag="e10")
            nc.scalar.activation(e10, lgs, AF.Exp)
            s10 = sm.tile([P, nh], F32, tag="s10")
            nc.vector.reduce_sum(s10, e10, axis=mybir.AxisListType.X)
            r10 = sm.tile([P, nh], F32, tag="r10")
            nc.vector.reciprocal(r10, s10)
            o10 = sm.tile([P, nh, 10], F32, tag="o10")
            nc.vector.tensor_tensor(
                out=o10, in0=e10,
                in1=_ap(r10, [r10[:].ap[0], [1, nh], [0, 10]]),
                op=ALU.mult,
            )
            base = out_d[ds(g * (nsub * P) + goff * P, P), :]
            oap = bass.AP(tensor=base.tensor, offset=base.offset,
                          ap=[base.ap[0], [10 * P, nh], [1, 10]])
            nc.sync.dma_start(out=oap, in_=o10)

        def body_capsule(g):
            hs = []
            for j in range(nsub):
                h0 = hp.tile([P, T, SLOT], BF, tag=f"h{j}")
                nc.sync.dma_start(
                    out=h0[:, :, 0:FV],
                    in_=x_d[ds(g * (nsub * P) + j * P, P), :].rearrange(
                        "p (t f) -> p t f", t=T
                    ),
                )
                nc.gpsimd.tensor_copy(h0[:, :, FV:SLOT], ones_c[:, 0:T])
                hs.append(h0)
            for it in range(8):
                nxts = []
                for j in range(nsub):
                    if it < 7:
                        h_nxt = hp.tile([P, T, SLOT], BF, tag=f"h{j}")
                    else:
                        h_nxt = hp.tile([P, T, SLOT], BF, tag=f"hold{j}",
                                        bufs=2)
                    nxts.append(h_nxt)
                if PAIR_VEC:
                    for j0 in range(0, nsub, 2):
                        zus = [capsule_psum(hs[j], j)
                               for j in range(j0, min(j0 + 2, nsub))]
                        capsule_vec_pair(hs[j0 : j0 + 2], nxts[j0 : j0 + 2],
                                         zus)
                elif PHASE_MAJOR:
                    zus = [capsule_psum(hs[j], j) for j in range(nsub)]
                    for j in range(nsub):
                        capsule_vec(hs[j], nxts[j], zus[j], j)
                else:
                    for j in range(nsub):
                        capsule_iter(hs[j], nxts[j], j)
                hs = list(nxts)
            return hs

        def body(g):
            hs = body_capsule(g)
            for d0 in range(0, nsub, 4):
                decoder(hs[d0 : d0 + 4], g, d0)

        if ngroups == 1:
            body(0)
        elif unroll:
            # software-pipelined: decode group g-1 while computing group g
            holds = [body_capsule(0)]
            for g in range(1, ngroups):
                holds.append(body_capsule(g))
                for d0 in range(0, nsub, 4):
                    decoder(holds[g - 1][d0 : d0 + 4], g - 1, d0)
            for d0 in range(0, nsub, 4):
                decoder(holds[-1][d0 : d0 + 4], ngroups - 1, d0)
        else:
            with tc.For_i(0, ngroups, 1) as g:
                body(g)
        for _pool in (pv, pp, sm, wkd, wk, hp, consts):
            _pool.release()

    nc.compile()
    return nc


def pack_weights(W1, b1, W2, b2, W3, b3, Wd1, bd1, Wd2, bd2, Wo, bo):
    f64 = np.float64
    W1, b1, W2, b2, W3, b3 = (np.asarray(t, f64) for t in (W1, b1, W2, b2, W3, b3))
    G = W1.T @ W2
    a = W2.T @ b1
    c = W1.T @ b2
    d = float(b1 @ b2)

    zu = np.zeros((P, 2, ZSLOT), np.float32)
    full = np.zeros((197, ZSLOT), f64)
    full[:196, :196] = G.T
    full[:196, 196] = a
    full[:196, SLOT:] = W3.T
    full[196, :196] = c
    full[196, 196] = d
    full[196, SLOT:] = b3
    zu[:, 0, :] = full[0:128]
    zu[69:128, 0, :] = 0.0
    zu[:, 1, :] = full[69:197]

    d1 = np.zeros((P, 8, FEAT), np.float32)
    W1T = np.asarray(Wd1, f64).T  # [784 f_in, 784 j]
    for t in range(T):
        d1[:, t, :] = W1T[t * FV : t * FV + P, :]
        d1[0:68, 4 + t, :] = W1T[t * FV + P : (t + 1) * FV, :]
    d1[68, 4, :] = np.asarray(bd1, f64)

    d2 = np.zeros((P, 7, FEAT), np.float32)
    W2T = np.asarray(Wd2, f64).T
    for cidx in range(6):
        d2[:, cidx, :] = W2T[cidx * P : (cidx + 1) * P, :]
    d2[0:16, 6, :] = W2T[768:784, :]
    d2[16, 6, :] = np.asarray(bd2, f64)

    ow = np.zeros((P, 7, 10), np.float32)
    WoT = np.asarray(Wo, f64).T
    for cidx in range(6):
        ow[:, cidx, :] = WoT[cidx * P : (cidx + 1) * P, :]
    ow[0:16, 6, :] = WoT[768:784, :]
    ow[16, 6, :] = np.asarray(bo, f64)
    return (zu.astype(ml_dtypes.bfloat16), d1.astype(ml_dtypes.bfloat16),
            d2.astype(ml_dtypes.bfloat16), ow.astype(ml_dtypes.bfloat16))


_NC_CACHE = {}


def kernel(**inputs):
    x = np.ascontiguousarray(np.asarray(inputs["x"], np.float32)).astype(
        ml_dtypes.bfloat16
    )
    zu, d1, d2, ow = pack_weights(
        inputs["W1"], inputs["b1"], inputs["W2"], inputs["b2"], inputs["W3"],
        inputs["b3"], inputs["Wd1"], inputs["bd1"], inputs["Wd2"],
        inputs["bd2"], inputs["Wo"], inputs["bo"],
    )
    if "nc" not in _NC_CACHE:
        _NC_CACHE["nc"] = build(8, 4, unroll=True)
    nc = _NC_CACHE["nc"]
    bpc = B // NCORES
    in_maps = [
        {
            "x": x[c * bpc : (c + 1) * bpc],
            "zu_w": zu,
            "dec1_w": d1,
            "dec2_w": d2,
            "out_w": ow,
        }
        for c in range(NCORES)
    ]
    res = run_bass_kernel_spmd(nc, in_maps, core_ids=list(range(NCORES)))
    return np.concatenate([res.results[c]["out"] for c in range(NCORES)], axis=0)


# revision 10
# speedup vs baseline: 1.1975x; 1.0933x over previous
"""Trainium2 Bass kernel for nn_CapsuleNeuralNetworkV2 (8 cores, data-parallel).

Math (per sample, 8 capsule iterations then decoder):
  v = h.reshape(4, 196)
  q = v @ W1.T + b1 ; k = v @ W2.T + b2 ; u = v @ W3.T + b3
  scores[t,s] = q_t . k_s  ->  softmax over s -> h'_t = sum_s P[t,s] u_s
  dec = relu(h Wd1.T + bd1) Wd2.T + bd2 ; out = softmax(dec Wo.T + bo)

Host-side algebra:
  scores[t,s] = v_t . z_s + r_s,  z_s = G v_s + c, r_s = a.v_s + d,
  G = W1.T W2, a = W2.T b1, c = W1.T b2, d = b1.b2; biases fused via an
  augmented ones row so one matmul emits [z | r | u] per slot.

v3 layout/schedule changes vs v2:
  - h slot is 197 wide ([v(196) | 1]); zu slot is 393 ([z | r | u]); the
    dots run over 197 elems and pick up r via the ones column.
  - zu matmuls write two 2-slot PSUM tiles; each is evacuated by ONE Act
    copy (f32->bf16), replacing four per-slot copies.
  - vt transposes write one bf16 PSUM tile; two DVE 2x copies evacuate.
  - dots are 16 STT+accum ops, combine is 4 chains of (seed + 3 MACs);
    both are spread across DVE/Act/Pool by a static table tuned against
    the instruction cost model.
  - probs = e * (1/sum) via one broadcast tensor_tensor.
"""

import numpy as np
import ml_dtypes

import concourse.bass as bass
import concourse.tile as tile
from concourse import bacc, mybir
from concourse.bass import ds
from concourse.bass_utils import run_bass_kernel_spmd
from concourse.masks import make_identity

FR = mybir.dt.float32r
BF = mybir.dt.bfloat16
F32 = mybir.dt.float32
AF = mybir.ActivationFunctionType
ALU = mybir.AluOpType

B = 32768
NCORES = 8
P = 128
T = 4
FV = 196
FEAT = 784
SLOT = FV + 1  # 197: 196 data + ones col
ZSLOT = 2 * FV + 1  # 393: z(196) | r(1) | u(196)

# --- static engine tables (v=DVE, a=Act, p=Pool), tuned vs cost model ---
# dots[t][s] (Act cannot run STT)
DOTS_ENG = [
    "vp..",
    "pp..",
    "pp..",
    "pp..",
]
# combine: SEED_ENG[t] + MAC_ENG[t] (3 chained MACs; MACs only v or p)
# combine mul engine per (t, s); adds are two batched DVE tensor_tensor
MUL_ENG = [
    "vapp",
    "aapp",
    "aapp",
    "aapp",
]
# vt-evac engines for (chunk1, chunk2)
VT_ENG = "a"
# s-slots computed via a DVE products TT + 4 cheap TSP-accums
PROD_S = (2, 3)
ADDS_PER_T = False
ZU0_B2 = False
PHASE_MAJOR = False
PAIR_VEC = False
ZU_BUFS = 5
D2_ACT = False
EXP_ACCUM = False
EXP_SPLIT = False
PROBS_POOL = False
HT_DVE = False
ZU_ONE = True


def _ap(t, dims, offset_elems=0):
    a = t[:] if hasattr(t, "tile") or not isinstance(t, bass.AP) else t
    return bass.AP(tensor=a.tensor, offset=a.offset + offset_elems, ap=dims)


def build(nsub=8, ngroups=4, unroll=False, zu_bufs=1, vtps_bufs=2, mpd_bufs=2,
          h_bufs=3, wk_bufs=5, sm_bufs=8, zu_mode="half", hf_bufs=1):
    """One NeuronCore program processing nsub*ngroups*128 samples."""
    bpc = nsub * ngroups * P
    nc = bacc.Bacc("TRN2", target_bir_lowering=False, debug=False)

    x_d = nc.dram_tensor("x", [bpc, FEAT], BF, kind="ExternalInput")
    zu_d = nc.dram_tensor("zu_w", [P, 2, ZSLOT], BF, kind="ExternalInput")
    d1_d = nc.dram_tensor("dec1_w", [P, 8, FEAT], BF, kind="ExternalInput")
    d2_d = nc.dram_tensor("dec2_w", [P, 7, FEAT], BF, kind="ExternalInput")
    ow_d = nc.dram_tensor("out_w", [P, 7, 10], BF, kind="ExternalInput")
    out_d = nc.dram_tensor("out", [bpc, 10], F32, kind="ExternalOutput")

    with tile.TileContext(nc) as tc:
        consts = tc.alloc_tile_pool(name="consts", bufs=1)
        hp = tc.alloc_tile_pool(name="h", bufs=h_bufs)
        wk = tc.alloc_tile_pool(name="wk", bufs=wk_bufs)
        wkd = tc.alloc_tile_pool(name="wkd", bufs=1)
        sm = tc.alloc_tile_pool(name="small", bufs=sm_bufs)
        pp = tc.alloc_tile_pool(name="ps", bufs=zu_bufs, space="PSUM")
        pv = tc.alloc_tile_pool(name="pv", bufs=vtps_bufs, space="PSUM")

        ident_f = consts.tile([P, P], F32)
        make_identity(nc, ident_f)
        ident = consts.tile([P, P], FR)
        nc.vector.tensor_copy(ident, ident_f)
        ident_b = consts.tile([P, P], BF)
        nc.vector.tensor_copy(ident_b, ident_f)
        ones_c = consts.tile([P, 512], F32)
        nc.vector.memset(ones_c, 1.0)
        zu_w = consts.tile([P, 2, ZSLOT], BF)
        nc.sync.dma_start(out=zu_w, in_=zu_d[:, :, :])
        d1_w = consts.tile([P, 8, FEAT], BF)
        nc.sync.dma_start(out=d1_w, in_=d1_d[:, :, :])
        d2_w = consts.tile([P, 7, FEAT], BF)
        nc.sync.dma_start(out=d2_w, in_=d2_d[:, :, :])
        ow_w = consts.tile([P, 7, 10], BF)
        nc.sync.dma_start(out=ow_w, in_=ow_d[:, :, :])

        def eng(c):
            return {"v": nc.vector, "p": nc.gpsimd, "a": nc.scalar}[c]

        def capsule_psum(h_cur, j):
            """PE transposes + zu matmuls + evacuations -> zu SBUF tile."""
            # chunk2 is a full 128-row transpose of features 69..196
            # (overlap rows 69..127 are zeroed in the chunk-1 weights), so
            # one evacuation op covers both chunks.
            vt_ps = pv.tile([P, T, 2, P], BF, tag="vtps")
            for t in range(T):
                nc.tensor.transpose(vt_ps[:, t, 0, :], h_cur[:, t, 0:P], ident_b)
                # includes the ones column -> row 127 is 1.0
                nc.tensor.transpose(vt_ps[:, t, 1, :],
                                    h_cur[:, t, 69 : 69 + P], ident_b)
            vt = wk.tile([P, T, 2, P], BF, tag="vt")
            c = VT_ENG[0]
            if c == "a":
                nc.scalar.copy(vt, vt_ps)
            elif c == "v":
                nc.vector.tensor_copy(vt, vt_ps)
            else:
                nc.gpsimd.tensor_copy(vt, vt_ps)

            zu = wk.tile([P, T, ZSLOT], BF, tag="zu", bufs=ZU_BUFS)
            if ZU_ONE:
                zu_ps = pp.tile([P, T, 512], F32, tag="zu0")
                for s in range(T):
                    nc.tensor.matmul(zu_ps[:, s, 0:ZSLOT], vt[:, s, 0, :],
                                     zu_w[:, 0, :], start=True, stop=False)
                    nc.tensor.matmul(zu_ps[:, s, 0:ZSLOT], vt[:, s, 1, :],
                                     zu_w[:, 1, :], start=False, stop=True)
                nc.scalar.copy(zu, zu_ps[:, :, 0:ZSLOT])
            else:
                for half in range(2):
                    # 512-wide slots keep each matmul within one PSUM bank
                    zu_ps = pp.tile([P, 2, 512], F32, tag=f"zu{half}")
                    for k in range(2):
                        s = 2 * half + k
                        nc.tensor.matmul(zu_ps[:, k, 0:ZSLOT], vt[:, s, 0, :],
                                         zu_w[:, 0, :], start=True, stop=False)
                        nc.tensor.matmul(zu_ps[:, k, 0:ZSLOT], vt[:, s, 1, :],
                                         zu_w[:, 1, :], start=False, stop=True)
                    nc.scalar.copy(zu[:, 2 * half : 2 * half + 2, :],
                                   zu_ps[:, :, 0:ZSLOT])
            return zu

        def capsule_vec(h_cur, h_nxt, zu, j):
            """SBUF-only: dots, softmax, combine -> h_nxt."""
            dots = sm.tile([P, T, T], F32, tag="dots")
            scr = sm.tile([P, 3, SLOT], BF, tag="scr", bufs=8)
            halves = [(0, 4)] if ZU_ONE else [(0, 2), (2, 2)]
            for h0, hn_ in halves:
                prod = sm.tile([P, hn_, T, SLOT], BF, tag=f"prod{h0}",
                               bufs=(2 if len(halves) > 1 else 3))
                hin = _ap(h_cur, [h_cur[:].ap[0], [0, hn_], [SLOT, T],
                                  [1, SLOT]])
                zin = _ap(zu, [zu[:].ap[0], [ZSLOT, hn_], [0, T], [1, SLOT]],
                          offset_elems=h0 * ZSLOT)
                nc.vector.tensor_tensor(out=prod, in0=hin, in1=zin, op=ALU.mult)
                for k in range(hn_):
                    s = h0 + k
                    for t in range(T):
                        nc.vector.tensor_scalar(
                            out=scr[:, 0, :],
                            in0=prod[:, k, t, :], scalar1=1.0,
                            scalar2=0.0, op0=ALU.mult, op1=ALU.add,
                            accum_out=dots[:, t, s : s + 1])

            # softmax over s (no max subtraction; |scores| < 30)
            e_t = sm.tile([P, T, T], F32, tag="e")
            sums = sm.tile([P, T], F32, tag="sums")
            if EXP_ACCUM:
                for t in range(T):
                    nc.scalar.activation(e_t[:, t, :], dots[:, t, :], AF.Exp,
                                         accum_out=sums[:, t : t + 1])
            else:
                if EXP_SPLIT:
                    nc.scalar.activation(e_t[:, :, 0:2], dots[:, :, 0:2],
                                         AF.Exp)
                    nc.scalar.activation(e_t[:, :, 2:4], dots[:, :, 2:4],
                                         AF.Exp)
                else:
                    nc.scalar.activation(e_t, dots, AF.Exp)
                nc.vector.reduce_sum(sums, e_t, axis=mybir.AxisListType.X)
            rec = sm.tile([P, T], F32, tag="rec")
            nc.vector.reciprocal(rec, sums)
            probs = sm.tile([P, T, T], F32, tag="probs")
            (nc.gpsimd if PROBS_POOL else nc.vector).tensor_tensor(
                out=probs, in0=e_t,
                in1=_ap(rec, [rec[:].ap[0], [1, T], [0, T]]),
                op=ALU.mult,
            )

            # ones column for the next h
            nc.gpsimd.tensor_copy(h_nxt[:, :, FV:SLOT], ones_c[:, 0:T])
            # combine: pu[t,s] = P[t,s] * u_s, then two batched DVE adds
            pu = sm.tile([P, T, T, FV], BF, tag="pu", bufs=2)
            for t in range(T):
                for s in range(T):
                    c = MUL_ENG[t][s]
                    if c == "a":
                        nc.scalar.activation(
                            pu[:, t, s, :], zu[:, s, SLOT:ZSLOT], AF.Copy,
                            scale=probs[:, t, s : s + 1])
                    elif c == "v":
                        nc.vector.tensor_scalar_mul(
                            pu[:, t, s, :], zu[:, s, SLOT:ZSLOT],
                            probs[:, t, s : s + 1])
                    else:
                        nc.gpsimd.tensor_scalar_mul(
                            pu[:, t, s, :], zu[:, s, SLOT:ZSLOT],
                            probs[:, t, s : s + 1])
            q = sm.tile([P, T, 2, FV], BF, tag="q", bufs=2)
            ev = _ap(pu, [pu[:].ap[0], [T * FV, T], [2 * FV, 2], [1, FV]])
            od = _ap(pu, [pu[:].ap[0], [T * FV, T], [2 * FV, 2], [1, FV]],
                     offset_elems=FV)
            nc.vector.tensor_tensor(out=q, in0=ev, in1=od, op=ALU.add)
            nc.vector.tensor_tensor(out=h_nxt[:, :, 0:FV], in0=q[:, :, 0, :],
                                    in1=q[:, :, 1, :], op=ALU.add)

        def capsule_vec_pair(hc, hn, zus):
            """Two tiles' dots/softmax/combine with pair-batched softmax."""
            npair = len(hc)
            dots = sm.tile([P, 2, T, T], F32, tag="dots")
            scr = sm.tile([P, 3, SLOT], BF, tag="scr", bufs=8)
            for jj in range(npair):
                for half in range(2):
                    prod = sm.tile([P, 2, T, SLOT], BF, tag=f"prod{half}",
                                   bufs=2)
                    hin = _ap(hc[jj], [hc[jj][:].ap[0], [0, 2], [SLOT, T],
                                       [1, SLOT]])
                    zin = _ap(zus[jj], [zus[jj][:].ap[0], [ZSLOT, 2], [0, T],
                                        [1, SLOT]],
                              offset_elems=half * 2 * ZSLOT)
                    nc.vector.tensor_tensor(out=prod, in0=hin, in1=zin,
                                            op=ALU.mult)
                    for k in range(2):
                        s = 2 * half + k
                        for t in range(T):
                            nc.vector.tensor_scalar(
                                out=scr[:, 0, :],
                                in0=prod[:, k, t, :], scalar1=1.0,
                                scalar2=0.0, op0=ALU.mult, op1=ALU.add,
                                accum_out=dots[:, jj, t, s : s + 1])

            # pair-batched softmax (no max subtraction; |scores| < 30)
            e_t = sm.tile([P, 2, T, T], F32, tag="e")
            nc.scalar.activation(e_t[:, 0:npair], dots[:, 0:npair], AF.Exp)
            sums = sm.tile([P, 2, T], F32, tag="sums")
            nc.vector.reduce_sum(sums[:, 0:npair], e_t[:, 0:npair],
                                 axis=mybir.AxisListType.X)
            rec = sm.tile([P, 2, T], F32, tag="rec")
            nc.vector.reciprocal(rec[:, 0:npair], sums[:, 0:npair])
            probs = sm.tile([P, 2, T, T], F32, tag="probs")
            nc.vector.tensor_tensor(
                out=probs[:, 0:npair], in0=e_t[:, 0:npair],
                in1=_ap(rec, [rec[:].ap[0], [T, npair], [1, T], [0, T]]),
                op=ALU.mult,
            )

            for jj in range(npair):
                nc.gpsimd.tensor_copy(hn[jj][:, :, FV:SLOT], ones_c[:, 0:T])
                pu = sm.tile([P, T, T, FV], BF, tag="pu", bufs=2)
                for t in range(T):
                    for s in range(T):
                        c = MUL_ENG[t][s]
                        if c == "a":
                            nc.scalar.activation(
                                pu[:, t, s, :], zus[jj][:, s, SLOT:ZSLOT],
                                AF.Copy, scale=probs[:, jj, t, s : s + 1])
                        elif c == "v":
                            nc.vector.tensor_scalar_mul(
                                pu[:, t, s, :], zus[jj][:, s, SLOT:ZSLOT],
                                probs[:, jj, t, s : s + 1])
                        else:
                            nc.gpsimd.tensor_scalar_mul(
                                pu[:, t, s, :], zus[jj][:, s, SLOT:ZSLOT],
                                probs[:, jj, t, s : s + 1])
                q = sm.tile([P, T, 2, FV], BF, tag="q", bufs=2)
                ev = _ap(pu, [pu[:].ap[0], [T * FV, T], [2 * FV, 2], [1, FV]])
                od = _ap(pu, [pu[:].ap[0], [T * FV, T], [2 * FV, 2], [1, FV]],
                         offset_elems=FV)
                nc.vector.tensor_tensor(out=q, in0=ev, in1=od, op=ALU.add)
                nc.vector.tensor_tensor(out=hn[jj][:, :, 0:FV],
                                        in0=q[:, :, 0, :],
                                        in1=q[:, :, 1, :], op=ALU.add)

        def capsule_iter(h_cur, h_nxt, j):
            zu = capsule_psum(h_cur, j)
            capsule_vec(h_cur, h_nxt, zu, j)

        def decoder(hs, g, goff=0):
            """Decoder over a chunk of <=4 tiles (N = len(hs)*128 wide)."""
            W = len(hs) * P
            # h.T chunks, slot-major: [128] x4 and [69] x4 (with ones row)
            # chunk2 is a full 128-row transpose of features 69..196 per t
            # (overlap rows zeroed in the chunk-1 weights): one evac per t.
            ht = wkd.tile([P, T, 2, W], BF, tag="ht1")
            for t in range(T):
                t_ps = pv.tile([P, 2, W], BF, tag="vtps")
                for j in range(len(hs)):
                    nc.tensor.transpose(
                        t_ps[:, 0, j * P : (j + 1) * P], hs[j][:, t, 0:P],
                        ident_b)
                    nc.tensor.transpose(
                        t_ps[:, 1, j * P : (j + 1) * P],
                        hs[j][:, t, 69 : 69 + P], ident_b)
                if HT_DVE:
                    nc.vector.tensor_copy(ht[:, t, :, :], t_ps)
                else:
                    nc.scalar.copy(ht[:, t, :, :], t_ps)

            # dec1 = relu(Wd1 @ h.T + bd1), feature-major, 7 M-chunks
            d1a = wkd.tile([P, 6, W], BF, tag="d1a")
            d1b = wkd.tile([17, W], BF, tag="d1b")
            nc.vector.tensor_copy(d1b, ones_c[0:17, 0:W])
            for m in range(7):
                mw = min(P, FEAT - m * P)
                mp = pv.tile([P, W], F32, tag="mpd", bufs=mpd_bufs)
                msl = slice(m * P, m * P + mw)
                for t in range(T):
                    nc.tensor.matmul(mp[0:mw, :], d1_w[:, t, msl],
                                     ht[:, t, 0, :], start=(t == 0), stop=False)
                for t in range(T):
                    nc.tensor.matmul(mp[0:mw, :], d1_w[:, 4 + t, msl],
                                     ht[:, t, 1, :], start=False, stop=(t == 3))
                if m < 6:
                    nc.scalar.activation(d1a[:, m, :], mp, AF.Relu)
                else:
                    nc.scalar.activation(d1b[0:16, :], mp[0:16, :], AF.Relu)

            # dec2 = Wd2 @ relu1 + bd2, feature-major
            d2a = wkd.tile([P, 6, W], BF, tag="d2a")
            d2b = wkd.tile([17, W], BF, tag="d2b")
            nc.vector.tensor_copy(d2b, ones_c[0:17, 0:W])
            for m in range(7):
                mw = min(P, FEAT - m * P)
                mp = pv.tile([P, W], F32, tag="mpd", bufs=mpd_bufs)
                msl = slice(m * P, m * P + mw)
                for c in range(6):
                    nc.tensor.matmul(mp[0:mw, :], d2_w[:, c, msl], d1a[:, c, :],
                                     start=(c == 0), stop=False)
                nc.tensor.matmul(mp[0:mw, :], d2_w[0:17, 6, msl], d1b,
                                 start=False, stop=True)
                if m < 6:
                    (nc.scalar.copy if D2_ACT else nc.vector.tensor_copy)(
                        d2a[:, m, :], mp)
                else:
                    (nc.scalar.copy if D2_ACT else nc.vector.tensor_copy)(
                        d2b[0:16, :], mp[0:16, :])

            # logits for all subtiles into one PSUM tile, then one
            # batched softmax (no max subtraction; |logits| < 30) and a
            # single strided output DMA.
            nh = len(hs)
            lgs = pv.tile([P, nh, 10], F32, tag="mpd", bufs=mpd_bufs)
            for j in range(nh):
                jsl = slice(j * P, (j + 1) * P)
                for c in range(6):
                    nc.tensor.matmul(lgs[:, j, :], d2a[:, c, jsl], ow_w[:, c, :],
                                     start=(c == 0), stop=False)
                nc.tensor.matmul(lgs[:, j, :], d2b[:, jsl], ow_w[0:17, 6, :],
                                 start=False, stop=True)
            e10 = sm.tile([P, nh, 10], F32, tag="e10")
            nc.scalar.activation(e10, lgs, AF.Exp)
            s10 = sm.tile([P, nh], F32, tag="s10")
            nc.vector.reduce_sum(s10, e10, axis=mybir.AxisListType.X)
            r10 = sm.tile([P, nh], F32, tag="r10")
            nc.vector.reciprocal(r10, s10)
            o10 = sm.tile([P, nh, 10], F32, tag="o10")
            nc.vector.tensor_tensor(
                out=o10, in0=e10,
                in1=_ap(r10, [r10[:].ap[0], [1, nh], [0, 10]]),
                op=ALU.mult,
            )
            base = out_d[ds(g * (nsub * P) + goff * P, P), :]
            oap = bass.AP(tensor=base.tensor, offset=base.offset,
                          ap=[base.ap[0], [10 * P, nh], [1, 10]])
            nc.sync.dma_start(out=oap, in_=o10)

        def body_capsule(g):
            hs = []
            for j in range(nsub):
                h0 = hp.tile([P, T, SLOT], BF, tag=f"h{j}")
                nc.sync.dma_start(
                    out=h0[:, :, 0:FV],
                    in_=x_d[ds(g * (nsub * P) + j * P, P), :].rearrange(
                        "p (t f) -> p t f", t=T
                    ),
                )
                nc.gpsimd.tensor_copy(h0[:, :, FV:SLOT], ones_c[:, 0:T])
                hs.append(h0)
            for it in range(8):
                nxts = []
                for j in range(nsub):
                    if it < 7:
                        h_nxt = hp.tile([P, T, SLOT], BF, tag=f"h{j}")
                    else:
                        h_nxt = hp.tile([P, T, SLOT], BF, tag=f"hold{j}",
                                        bufs=2)
                    nxts.append(h_nxt)
                if PAIR_VEC:
                    for j0 in range(0, nsub, 2):
                        zus = [capsule_psum(hs[j], j)
                               for j in range(j0, min(j0 + 2, nsub))]
                        capsule_vec_pair(hs[j0 : j0 + 2], nxts[j0 : j0 + 2],
                                         zus)
                elif PHASE_MAJOR:
                    zus = [capsule_psum(hs[j], j) for j in range(nsub)]
                    for j in range(nsub):
                        capsule_vec(hs[j], nxts[j], zus[j], j)
                else:
                    for j in range(nsub):
                        capsule_iter(hs[j], nxts[j], j)
                hs = list(nxts)
            return hs

        def body(g):
            hs = body_capsule(g)
            for d0 in range(0, nsub, 4):
                decoder(hs[d0 : d0 + 4], g, d0)

        if ngroups == 1:
            body(0)
        elif unroll:
            # software-pipelined: decode group g-1 while computing group g
            holds = [body_capsule(0)]
            for g in range(1, ngroups):
                holds.append(body_capsule(g))
                for d0 in range(0, nsub, 4):
                    decoder(holds[g - 1][d0 : d0 + 4], g - 1, d0)
            for d0 in range(0, nsub, 4):
                decoder(holds[-1][d0 : d0 + 4], ngroups - 1, d0)
        else:
            with tc.For_i(0, ngroups, 1) as g:
                body(g)
        for _pool in (pv, pp, sm, wkd, wk, hp, consts):
            _pool.release()

    nc.compile()
    return nc


def pack_weights(W1, b1, W2, b2, W3, b3, Wd1, bd1, Wd2, bd2, Wo, bo):
    f64 = np.float64
    W1, b1, W2, b2, W3, b3 = (np.asarray(t, f64) for t in (W1, b1, W2, b2, W3, b3))
    G = W1.T @ W2
    a = W2.T @ b1
    c = W1.T @ b2
    d = float(b1 @ b2)

    zu = np.zeros((P, 2, ZSLOT), np.float32)
    full = np.zeros((197, ZSLOT), f64)
    full[:196, :196] = G.T
    full[:196, 196] = a
    full[:196, SLOT:] = W3.T
    full[196, :196] = c
    full[196, 196] = d
    full[196, SLOT:] = b3
    zu[:, 0, :] = full[0:128]
    zu[69:128, 0, :] = 0.0
    zu[:, 1, :] = full[69:197]

    d1 = np.zeros((P, 8, FEAT), np.float32)
    W1T = np.asarray(Wd1, f64).T  # [784 f_in, 784 j]
    for t in range(T):
        d1[:, t, :] = W1T[t * FV : t * FV + P, :]
        d1[69:128, t, :] = 0.0
        d1[0:127, 4 + t, :] = W1T[t * FV + 69 : t * FV + FV, :]
    # row 127 of every chunk-2 transpose is the ones column; add bd1 once
    d1[127, 4, :] = np.asarray(bd1, f64)

    d2 = np.zeros((P, 7, FEAT), np.float32)
    W2T = np.asarray(Wd2, f64).T
    for cidx in range(6):
        d2[:, cidx, :] = W2T[cidx * P : (cidx + 1) * P, :]
    d2[0:16, 6, :] = W2T[768:784, :]
    d2[16, 6, :] = np.asarray(bd2, f64)

    ow = np.zeros((P, 7, 10), np.float32)
    WoT = np.asarray(Wo, f64).T
    for cidx in range(6):
        ow[:, cidx, :] = WoT[cidx * P : (cidx + 1) * P, :]
    ow[0:16, 6, :] = WoT[768:784, :]
    ow[16, 6, :] = np.asarray(bo, f64)
    return (zu.astype(ml_dtypes.bfloat16), d1.astype(ml_dtypes.bfloat16),
            d2.astype(ml_dtypes.bfloat16), ow.astype(ml_dtypes.bfloat16))


_NC_CACHE = {}


def kernel(**inputs):
    x = np.ascontiguousarray(np.asarray(inputs["x"], np.float32)).astype(
        ml_dtypes.bfloat16
    )
    zu, d1, d2, ow = pack_weights(
        inputs["W1"], inputs["b1"], inputs["W2"], inputs["b2"], inputs["W3"],
        inputs["b3"], inputs["Wd1"], inputs["bd1"], inputs["Wd2"],
        inputs["bd2"], inputs["Wo"], inputs["bo"],
    )
    if "nc" not in _NC_CACHE:
        _NC_CACHE["nc"] = build(8, 4, unroll=True)
    nc = _NC_CACHE["nc"]
    bpc = B // NCORES
    in_maps = [
        {
            "x": x[c * bpc : (c + 1) * bpc],
            "zu_w": zu,
            "dec1_w": d1,
            "dec2_w": d2,
            "out_w": ow,
        }
        for c in range(NCORES)
    ]
    res = run_bass_kernel_spmd(nc, in_maps, core_ids=list(range(NCORES)))
    return np.concatenate([res.results[c]["out"] for c in range(NCORES)], axis=0)


# revision 11
# speedup vs baseline: 1.2114x; 1.0117x over previous
"""Trainium2 Bass kernel for nn_CapsuleNeuralNetworkV2 (8 cores, data-parallel).

Math (per sample, 8 capsule iterations then decoder):
  v = h.reshape(4, 196)
  q = v @ W1.T + b1 ; k = v @ W2.T + b2 ; u = v @ W3.T + b3
  scores[t,s] = q_t . k_s  ->  softmax over s -> h'_t = sum_s P[t,s] u_s
  dec = relu(h Wd1.T + bd1) Wd2.T + bd2 ; out = softmax(dec Wo.T + bo)

Host-side algebra:
  scores[t,s] = v_t . z_s + r_s,  z_s = G v_s + c, r_s = a.v_s + d,
  G = W1.T W2, a = W2.T b1, c = W1.T b2, d = b1.b2; biases fused via an
  augmented ones row so one matmul emits [z | r | u] per slot.

v3 layout/schedule changes vs v2:
  - h slot is 197 wide ([v(196) | 1]); zu slot is 393 ([z | r | u]); the
    dots run over 197 elems and pick up r via the ones column.
  - zu matmuls write two 2-slot PSUM tiles; each is evacuated by ONE Act
    copy (f32->bf16), replacing four per-slot copies.
  - vt transposes write one bf16 PSUM tile; two DVE 2x copies evacuate.
  - dots are 16 STT+accum ops, combine is 4 chains of (seed + 3 MACs);
    both are spread across DVE/Act/Pool by a static table tuned against
    the instruction cost model.
  - probs = e * (1/sum) via one broadcast tensor_tensor.
"""

import numpy as np
import ml_dtypes

import concourse.bass as bass
import concourse.tile as tile
from concourse import bacc, mybir
from concourse.bass import ds
from concourse.bass_utils import run_bass_kernel_spmd
from concourse.masks import make_identity

FR = mybir.dt.float32r
BF = mybir.dt.bfloat16
F32 = mybir.dt.float32
AF = mybir.ActivationFunctionType
ALU = mybir.AluOpType

B = 32768
NCORES = 8
P = 128
T = 4
FV = 196
FEAT = 784
SLOT = FV + 1  # 197: 196 data + ones col
ZSLOT = 2 * FV + 1  # 393: z(196) | r(1) | u(196)

# --- static engine tables (v=DVE, a=Act, p=Pool), tuned vs cost model ---
# dots[t][s] (Act cannot run STT)
DOTS_ENG = [
    "vp..",
    "pp..",
    "pp..",
    "pp..",
]
# combine: SEED_ENG[t] + MAC_ENG[t] (3 chained MACs; MACs only v or p)
# combine mul engine per (t, s); adds are two batched DVE tensor_tensor
MUL_ENG = [
    "vapp",
    "vapp",
    "aapp",
    "aapp",
]
# vt-evac engines for (chunk1, chunk2)
VT_ENG = "a"
# s-slots computed via a DVE products TT + 4 cheap TSP-accums
PROD_S = (2, 3)
ADDS_PER_T = False
ZU0_B2 = False
PHASE_MAJOR = False
PAIR_VEC = False
ZU_BUFS = 5
D2_ACT = False
EXP_ACCUM = False
EXP_SPLIT = False
PROBS_POOL = False
HT_DVE = False
ZU_ONE = True


def _ap(t, dims, offset_elems=0):
    a = t[:] if hasattr(t, "tile") or not isinstance(t, bass.AP) else t
    return bass.AP(tensor=a.tensor, offset=a.offset + offset_elems, ap=dims)


def build(nsub=8, ngroups=4, unroll=False, zu_bufs=1, vtps_bufs=2, mpd_bufs=2,
          h_bufs=3, wk_bufs=5, sm_bufs=8, zu_mode="half", hf_bufs=1):
    """One NeuronCore program processing nsub*ngroups*128 samples."""
    bpc = nsub * ngroups * P
    nc = bacc.Bacc("TRN2", target_bir_lowering=False, debug=False)

    x_d = nc.dram_tensor("x", [bpc, FEAT], BF, kind="ExternalInput")
    zu_d = nc.dram_tensor("zu_w", [P, 2, ZSLOT], BF, kind="ExternalInput")
    d1_d = nc.dram_tensor("dec1_w", [P, 8, FEAT], BF, kind="ExternalInput")
    d2_d = nc.dram_tensor("dec2_w", [P, 7, FEAT], BF, kind="ExternalInput")
    ow_d = nc.dram_tensor("out_w", [P, 7, 10], BF, kind="ExternalInput")
    out_d = nc.dram_tensor("out", [bpc, 10], F32, kind="ExternalOutput")

    with tile.TileContext(nc) as tc:
        consts = tc.alloc_tile_pool(name="consts", bufs=1)
        hp = tc.alloc_tile_pool(name="h", bufs=h_bufs)
        wk = tc.alloc_tile_pool(name="wk", bufs=wk_bufs)
        wkd = tc.alloc_tile_pool(name="wkd", bufs=1)
        sm = tc.alloc_tile_pool(name="small", bufs=sm_bufs)
        pp = tc.alloc_tile_pool(name="ps", bufs=zu_bufs, space="PSUM")
        pv = tc.alloc_tile_pool(name="pv", bufs=vtps_bufs, space="PSUM")

        ident_f = consts.tile([P, P], F32)
        make_identity(nc, ident_f)
        ident = consts.tile([P, P], FR)
        nc.vector.tensor_copy(ident, ident_f)
        ident_b = consts.tile([P, P], BF)
        nc.vector.tensor_copy(ident_b, ident_f)
        ones_c = consts.tile([P, 512], F32)
        nc.vector.memset(ones_c, 1.0)
        zu_w = consts.tile([P, 2, ZSLOT], BF)
        nc.sync.dma_start(out=zu_w, in_=zu_d[:, :, :])
        d1_w = consts.tile([P, 8, FEAT], BF)
        nc.sync.dma_start(out=d1_w, in_=d1_d[:, :, :])
        d2_w = consts.tile([P, 7, FEAT], BF)
        nc.sync.dma_start(out=d2_w, in_=d2_d[:, :, :])
        ow_w = consts.tile([P, 7, 10], BF)
        nc.sync.dma_start(out=ow_w, in_=ow_d[:, :, :])

        def eng(c):
            return {"v": nc.vector, "p": nc.gpsimd, "a": nc.scalar}[c]

        def capsule_psum(h_cur, j):
            """PE transposes + zu matmuls + evacuations -> zu SBUF tile."""
            # chunk2 is a full 128-row transpose of features 69..196
            # (overlap rows 69..127 are zeroed in the chunk-1 weights), so
            # one evacuation op covers both chunks.
            vt_ps = pv.tile([P, T, 2, P], BF, tag="vtps")
            for t in range(T):
                nc.tensor.transpose(vt_ps[:, t, 0, :], h_cur[:, t, 0:P], ident_b)
                # includes the ones column -> row 127 is 1.0
                nc.tensor.transpose(vt_ps[:, t, 1, :],
                                    h_cur[:, t, 69 : 69 + P], ident_b)
            vt = wk.tile([P, T, 2, P], BF, tag="vt")
            c = VT_ENG[0]
            if c == "a":
                nc.scalar.copy(vt, vt_ps)
            elif c == "v":
                nc.vector.tensor_copy(vt, vt_ps)
            else:
                nc.gpsimd.tensor_copy(vt, vt_ps)

            zu = wk.tile([P, T, ZSLOT], BF, tag="zu", bufs=ZU_BUFS)
            if ZU_ONE:
                zu_ps = pp.tile([P, T, 512], F32, tag="zu0")
                for s in range(T):
                    nc.tensor.matmul(zu_ps[:, s, 0:ZSLOT], vt[:, s, 0, :],
                                     zu_w[:, 0, :], start=True, stop=False)
                    nc.tensor.matmul(zu_ps[:, s, 0:ZSLOT], vt[:, s, 1, :],
                                     zu_w[:, 1, :], start=False, stop=True)
                nc.scalar.copy(zu, zu_ps[:, :, 0:ZSLOT])
            else:
                for half in range(2):
                    # 512-wide slots keep each matmul within one PSUM bank
                    zu_ps = pp.tile([P, 2, 512], F32, tag=f"zu{half}")
                    for k in range(2):
                        s = 2 * half + k
                        nc.tensor.matmul(zu_ps[:, k, 0:ZSLOT], vt[:, s, 0, :],
                                         zu_w[:, 0, :], start=True, stop=False)
                        nc.tensor.matmul(zu_ps[:, k, 0:ZSLOT], vt[:, s, 1, :],
                                         zu_w[:, 1, :], start=False, stop=True)
                    nc.scalar.copy(zu[:, 2 * half : 2 * half + 2, :],
                                   zu_ps[:, :, 0:ZSLOT])
            return zu

        def capsule_vec(h_cur, h_nxt, zu, j):
            """SBUF-only: dots, softmax, combine -> h_nxt."""
            dots = sm.tile([P, T, T], F32, tag="dots")
            scr = sm.tile([P, 3, SLOT], BF, tag="scr", bufs=8)
            halves = [(0, 4)] if ZU_ONE else [(0, 2), (2, 2)]
            for h0, hn_ in halves:
                prod = sm.tile([P, hn_, T, SLOT], BF, tag=f"prod{h0}",
                               bufs=(2 if len(halves) > 1 else 3))
                hin = _ap(h_cur, [h_cur[:].ap[0], [0, hn_], [SLOT, T],
                                  [1, SLOT]])
                zin = _ap(zu, [zu[:].ap[0], [ZSLOT, hn_], [0, T], [1, SLOT]],
                          offset_elems=h0 * ZSLOT)
                nc.vector.tensor_tensor(out=prod, in0=hin, in1=zin, op=ALU.mult)
                for k in range(hn_):
                    s = h0 + k
                    for t in range(T):
                        nc.vector.tensor_scalar(
                            out=scr[:, 0, :],
                            in0=prod[:, k, t, :], scalar1=1.0,
                            scalar2=0.0, op0=ALU.mult, op1=ALU.add,
                            accum_out=dots[:, t, s : s + 1])

            # softmax over s (no max subtraction; |scores| < 30)
            e_t = sm.tile([P, T, T], F32, tag="e")
            sums = sm.tile([P, T], F32, tag="sums")
            if EXP_ACCUM:
                for t in range(T):
                    nc.scalar.activation(e_t[:, t, :], dots[:, t, :], AF.Exp,
                                         accum_out=sums[:, t : t + 1])
            else:
                if EXP_SPLIT:
                    nc.scalar.activation(e_t[:, :, 0:2], dots[:, :, 0:2],
                                         AF.Exp)
                    nc.scalar.activation(e_t[:, :, 2:4], dots[:, :, 2:4],
                                         AF.Exp)
                else:
                    nc.scalar.activation(e_t, dots, AF.Exp)
                nc.vector.reduce_sum(sums, e_t, axis=mybir.AxisListType.X)
            rec = sm.tile([P, T], F32, tag="rec")
            nc.vector.reciprocal(rec, sums)
            probs = sm.tile([P, T, T], F32, tag="probs")
            (nc.gpsimd if PROBS_POOL else nc.vector).tensor_tensor(
                out=probs, in0=e_t,
                in1=_ap(rec, [rec[:].ap[0], [1, T], [0, T]]),
                op=ALU.mult,
            )

            # ones column for the next h
            nc.gpsimd.tensor_copy(h_nxt[:, :, FV:SLOT], ones_c[:, 0:T])
            # combine: pu[t,s] = P[t,s] * u_s, then two batched DVE adds
            pu = sm.tile([P, T, T, FV], BF, tag="pu", bufs=2)
            for t in range(T):
                for s in range(T):
                    c = MUL_ENG[t][s]
                    if c == "a":
                        nc.scalar.activation(
                            pu[:, t, s, :], zu[:, s, SLOT:ZSLOT], AF.Copy,
                            scale=probs[:, t, s : s + 1])
                    elif c == "v":
                        nc.vector.tensor_scalar_mul(
                            pu[:, t, s, :], zu[:, s, SLOT:ZSLOT],
                            probs[:, t, s : s + 1])
                    else:
                        nc.gpsimd.tensor_scalar_mul(
                            pu[:, t, s, :], zu[:, s, SLOT:ZSLOT],
                            probs[:, t, s : s + 1])
            q = sm.tile([P, T, 2, FV], BF, tag="q", bufs=2)
            ev = _ap(pu, [pu[:].ap[0], [T * FV, T], [2 * FV, 2], [1, FV]])
            od = _ap(pu, [pu[:].ap[0], [T * FV, T], [2 * FV, 2], [1, FV]],
                     offset_elems=FV)
            nc.vector.tensor_tensor(out=q, in0=ev, in1=od, op=ALU.add)
            nc.vector.tensor_tensor(out=h_nxt[:, :, 0:FV], in0=q[:, :, 0, :],
                                    in1=q[:, :, 1, :], op=ALU.add)

        def capsule_vec_pair(hc, hn, zus):
            """Two tiles' dots/softmax/combine with pair-batched softmax."""
            npair = len(hc)
            dots = sm.tile([P, 2, T, T], F32, tag="dots")
            scr = sm.tile([P, 3, SLOT], BF, tag="scr", bufs=8)
            for jj in range(npair):
                for half in range(2):
                    prod = sm.tile([P, 2, T, SLOT], BF, tag=f"prod{half}",
                                   bufs=2)
                    hin = _ap(hc[jj], [hc[jj][:].ap[0], [0, 2], [SLOT, T],
                                       [1, SLOT]])
                    zin = _ap(zus[jj], [zus[jj][:].ap[0], [ZSLOT, 2], [0, T],
                                        [1, SLOT]],
                              offset_elems=half * 2 * ZSLOT)
                    nc.vector.tensor_tensor(out=prod, in0=hin, in1=zin,
                                            op=ALU.mult)
                    for k in range(2):
                        s = 2 * half + k
                        for t in range(T):
                            nc.vector.tensor_scalar(
                                out=scr[:, 0, :],
                                in0=prod[:, k, t, :], scalar1=1.0,
                                scalar2=0.0, op0=ALU.mult, op1=ALU.add,
                                accum_out=dots[:, jj, t, s : s + 1])

            # pair-batched softmax (no max subtraction; |scores| < 30)
            e_t = sm.tile([P, 2, T, T], F32, tag="e")
            nc.scalar.activation(e_t[:, 0:npair], dots[:, 0:npair], AF.Exp)
            sums = sm.tile([P, 2, T], F32, tag="sums")
            nc.vector.reduce_sum(sums[:, 0:npair], e_t[:, 0:npair],
                                 axis=mybir.AxisListType.X)
            rec = sm.tile([P, 2, T], F32, tag="rec")
            nc.vector.reciprocal(rec[:, 0:npair], sums[:, 0:npair])
            probs = sm.tile([P, 2, T, T], F32, tag="probs")
            nc.vector.tensor_tensor(
                out=probs[:, 0:npair], in0=e_t[:, 0:npair],
                in1=_ap(rec, [rec[:].ap[0], [T, npair], [1, T], [0, T]]),
                op=ALU.mult,
            )

            for jj in range(npair):
                nc.gpsimd.tensor_copy(hn[jj][:, :, FV:SLOT], ones_c[:, 0:T])
                pu = sm.tile([P, T, T, FV], BF, tag="pu", bufs=2)
                for t in range(T):
                    for s in range(T):
                        c = MUL_ENG[t][s]
                        if c == "a":
                            nc.scalar.activation(
                                pu[:, t, s, :], zus[jj][:, s, SLOT:ZSLOT],
                                AF.Copy, scale=probs[:, jj, t, s : s + 1])
                        elif c == "v":
                            nc.vector.tensor_scalar_mul(
                                pu[:, t, s, :], zus[jj][:, s, SLOT:ZSLOT],
                                probs[:, jj, t, s : s + 1])
                        else:
                            nc.gpsimd.tensor_scalar_mul(
                                pu[:, t, s, :], zus[jj][:, s, SLOT:ZSLOT],
                                probs[:, jj, t, s : s + 1])
                q = sm.tile([P, T, 2, FV], BF, tag="q", bufs=2)
                ev = _ap(pu, [pu[:].ap[0], [T * FV, T], [2 * FV, 2], [1, FV]])
                od = _ap(pu, [pu[:].ap[0], [T * FV, T], [2 * FV, 2], [1, FV]],
                         offset_elems=FV)
                nc.vector.tensor_tensor(out=q, in0=ev, in1=od, op=ALU.add)
                nc.vector.tensor_tensor(out=hn[jj][:, :, 0:FV],
                                        in0=q[:, :, 0, :],
                                        in1=q[:, :, 1, :], op=ALU.add)

        def capsule_iter(h_cur, h_nxt, j):
            zu = capsule_psum(h_cur, j)
            capsule_vec(h_cur, h_nxt, zu, j)

        def decoder(hs, g, goff=0):
            """Decoder over a chunk of <=4 tiles (N = len(hs)*128 wide)."""
            W = len(hs) * P
            # h.T chunks, slot-major: [128] x4 and [69] x4 (with ones row)
            # chunk2 is a full 128-row transpose of features 69..196 per t
            # (overlap rows zeroed in the chunk-1 weights): one evac per t.
            ht = wkd.tile([P, T, 2, W], BF, tag="ht1")
            for t in range(T):
                t_ps = pv.tile([P, 2, W], BF, tag="vtps")
                for j in range(len(hs)):
                    nc.tensor.transpose(
                        t_ps[:, 0, j * P : (j + 1) * P], hs[j][:, t, 0:P],
                        ident_b)
                    nc.tensor.transpose(
                        t_ps[:, 1, j * P : (j + 1) * P],
                        hs[j][:, t, 69 : 69 + P], ident_b)
                if HT_DVE:
                    nc.vector.tensor_copy(ht[:, t, :, :], t_ps)
                else:
                    nc.scalar.copy(ht[:, t, :, :], t_ps)

            # dec1 = relu(Wd1 @ h.T + bd1), feature-major, 7 M-chunks
            d1a = wkd.tile([P, 6, W], BF, tag="d1a")
            d1b = wkd.tile([17, W], BF, tag="d1b")
            nc.vector.tensor_copy(d1b, ones_c[0:17, 0:W])
            for m in range(7):
                mw = min(P, FEAT - m * P)
                mp = pv.tile([P, W], F32, tag="mpd", bufs=mpd_bufs)
                msl = slice(m * P, m * P + mw)
                for t in range(T):
                    nc.tensor.matmul(mp[0:mw, :], d1_w[:, t, msl],
                                     ht[:, t, 0, :], start=(t == 0), stop=False)
                for t in range(T):
                    nc.tensor.matmul(mp[0:mw, :], d1_w[:, 4 + t, msl],
                                     ht[:, t, 1, :], start=False, stop=(t == 3))
                if m < 6:
                    nc.scalar.activation(d1a[:, m, :], mp, AF.Relu)
                else:
                    nc.scalar.activation(d1b[0:16, :], mp[0:16, :], AF.Relu)

            # dec2 = Wd2 @ relu1 + bd2, feature-major
            d2a = wkd.tile([P, 6, W], BF, tag="d2a")
            d2b = wkd.tile([17, W], BF, tag="d2b")
            nc.vector.tensor_copy(d2b, ones_c[0:17, 0:W])
            for m in range(7):
                mw = min(P, FEAT - m * P)
                mp = pv.tile([P, W], F32, tag="mpd", bufs=mpd_bufs)
                msl = slice(m * P, m * P + mw)
                for c in range(6):
                    nc.tensor.matmul(mp[0:mw, :], d2_w[:, c, msl], d1a[:, c, :],
                                     start=(c == 0), stop=False)
                nc.tensor.matmul(mp[0:mw, :], d2_w[0:17, 6, msl], d1b,
                                 start=False, stop=True)
                if m < 6:
                    (nc.scalar.copy if D2_ACT else nc.vector.tensor_copy)(
                        d2a[:, m, :], mp)
                else:
                    (nc.scalar.copy if D2_ACT else nc.vector.tensor_copy)(
                        d2b[0:16, :], mp[0:16, :])

            # logits for all subtiles into one PSUM tile, then one
            # batched softmax (no max subtraction; |logits| < 30) and a
            # single strided output DMA.
            nh = len(hs)
            lgs = pv.tile([P, nh, 10], F32, tag="mpd", bufs=mpd_bufs)
            for j in range(nh):
                jsl = slice(j * P, (j + 1) * P)
                for c in range(6):
                    nc.tensor.matmul(lgs[:, j, :], d2a[:, c, jsl], ow_w[:, c, :],
                                     start=(c == 0), stop=False)
                nc.tensor.matmul(lgs[:, j, :], d2b[:, jsl], ow_w[0:17, 6, :],
                                 start=False, stop=True)
            e10 = sm.tile([P, nh, 10], F32, tag="e10")
            nc.scalar.activation(e10, lgs, AF.Exp)
            s10 = sm.tile([P, nh], F32, tag="s10")
            nc.vector.reduce_sum(s10, e10, axis=mybir.AxisListType.X)
            r10 = sm.tile([P, nh], F32, tag="r10")
            nc.vector.reciprocal(r10, s10)
            o10 = sm.tile([P, nh, 10], F32, tag="o10")
            nc.vector.tensor_tensor(
                out=o10, in0=e10,
                in1=_ap(r10, [r10[:].ap[0], [1, nh], [0, 10]]),
                op=ALU.mult,
            )
            base = out_d[ds(g * (nsub * P) + goff * P, P), :]
            oap = bass.AP(tensor=base.tensor, offset=base.offset,
                          ap=[base.ap[0], [10 * P, nh], [1, 10]])
            nc.sync.dma_start(out=oap, in_=o10)

        def body_capsule(g):
            hs = []
            for j in range(nsub):
                h0 = hp.tile([P, T, SLOT], BF, tag=f"h{j}")
                nc.sync.dma_start(
                    out=h0[:, :, 0:FV],
                    in_=x_d[ds(g * (nsub * P) + j * P, P), :].rearrange(
                        "p (t f) -> p t f", t=T
                    ),
                )
                nc.gpsimd.tensor_copy(h0[:, :, FV:SLOT], ones_c[:, 0:T])
                hs.append(h0)
            for it in range(8):
                nxts = []
                for j in range(nsub):
                    if it < 7:
                        h_nxt = hp.tile([P, T, SLOT], BF, tag=f"h{j}")
                    else:
                        h_nxt = hp.tile([P, T, SLOT], BF, tag=f"hold{j}",
                                        bufs=2)
                    nxts.append(h_nxt)
                if PAIR_VEC:
                    for j0 in range(0, nsub, 2):
                        zus = [capsule_psum(hs[j], j)
                               for j in range(j0, min(j0 + 2, nsub))]
                        capsule_vec_pair(hs[j0 : j0 + 2], nxts[j0 : j0 + 2],
                                         zus)
                elif PHASE_MAJOR:
                    zus = [capsule_psum(hs[j], j) for j in range(nsub)]
                    for j in range(nsub):
                        capsule_vec(hs[j], nxts[j], zus[j], j)
                else:
                    for j in range(nsub):
                        capsule_iter(hs[j], nxts[j], j)
                hs = list(nxts)
            return hs

        def body(g):
            hs = body_capsule(g)
            for d0 in range(0, nsub, 4):
                decoder(hs[d0 : d0 + 4], g, d0)

        if ngroups == 1:
            body(0)
        elif unroll:
            # software-pipelined: decode group g-1 while computing group g
            holds = [body_capsule(0)]
            for g in range(1, ngroups):
                holds.append(body_capsule(g))
                for d0 in range(0, nsub, 4):
                    decoder(holds[g - 1][d0 : d0 + 4], g - 1, d0)
            for d0 in range(0, nsub, 4):
                decoder(holds[-1][d0 : d0 + 4], ngroups - 1, d0)
        else:
            with tc.For_i(0, ngroups, 1) as g:
                body(g)
        for _pool in (pv, pp, sm, wkd, wk, hp, consts):
            _pool.release()

    nc.compile()
    return nc


def pack_weights(W1, b1, W2, b2, W3, b3, Wd1, bd1, Wd2, bd2, Wo, bo):
    f64 = np.float64
    W1, b1, W2, b2, W3, b3 = (np.asarray(t, f64) for t in (W1, b1, W2, b2, W3, b3))
    G = W1.T @ W2
    a = W2.T @ b1
    c = W1.T @ b2
    d = float(b1 @ b2)

    zu = np.zeros((P, 2, ZSLOT), np.float32)
    full = np.zeros((197, ZSLOT), f64)
    full[:196, :196] = G.T
    full[:196, 196] = a
    full[:196, SLOT:] = W3.T
    full[196, :196] = c
    full[196, 196] = d
    full[196, SLOT:] = b3
    zu[:, 0, :] = full[0:128]
    zu[69:128, 0, :] = 0.0
    zu[:, 1, :] = full[69:197]

    d1 = np.zeros((P, 8, FEAT), np.float32)
    W1T = np.asarray(Wd1, f64).T  # [784 f_in, 784 j]
    for t in range(T):
        d1[:, t, :] = W1T[t * FV : t * FV + P, :]
        d1[69:128, t, :] = 0.0
        d1[0:127, 4 + t, :] = W1T[t * FV + 69 : t * FV + FV, :]
    # row 127 of every chunk-2 transpose is the ones column; add bd1 once
    d1[127, 4, :] = np.asarray(bd1, f64)

    d2 = np.zeros((P, 7, FEAT), np.float32)
    W2T = np.asarray(Wd2, f64).T
    for cidx in range(6):
        d2[:, cidx, :] = W2T[cidx * P : (cidx + 1) * P, :]
    d2[0:16, 6, :] = W2T[768:784, :]
    d2[16, 6, :] = np.asarray(bd2, f64)

    ow = np.zeros((P, 7, 10), np.float32)
    WoT = np.asarray(Wo, f64).T
    for cidx in range(6):
        ow[:, cidx, :] = WoT[cidx * P : (cidx + 1) * P, :]
    ow[0:16, 6, :] = WoT[768:784, :]
    ow[16, 6, :] = np.asarray(bo, f64)
    return (zu.astype(ml_dtypes.bfloat16), d1.astype(ml_dtypes.bfloat16),
            d2.astype(ml_dtypes.bfloat16), ow.astype(ml_dtypes.bfloat16))


_NC_CACHE = {}


def kernel(**inputs):
    x = np.ascontiguousarray(np.asarray(inputs["x"], np.float32)).astype(
        ml_dtypes.bfloat16
    )
    zu, d1, d2, ow = pack_weights(
        inputs["W1"], inputs["b1"], inputs["W2"], inputs["b2"], inputs["W3"],
        inputs["b3"], inputs["Wd1"], inputs["bd1"], inputs["Wd2"],
        inputs["bd2"], inputs["Wo"], inputs["bo"],
    )
    if "nc" not in _NC_CACHE:
        _NC_CACHE["nc"] = build(8, 4, unroll=True)
    nc = _NC_CACHE["nc"]
    bpc = B // NCORES
    in_maps = [
        {
            "x": x[c * bpc : (c + 1) * bpc],
            "zu_w": zu,
            "dec1_w": d1,
            "dec2_w": d2,
            "out_w": ow,
        }
        for c in range(NCORES)
    ]
    res = run_bass_kernel_spmd(nc, in_maps, core_ids=list(range(NCORES)))
    return np.concatenate([res.results[c]["out"] for c in range(NCORES)], axis=0)


# revision 12
# speedup vs baseline: 1.2120x; 1.0004x over previous
"""Trainium2 Bass kernel for nn_CapsuleNeuralNetworkV2 (8 cores, data-parallel).

Math (per sample, 8 capsule iterations then decoder):
  v = h.reshape(4, 196)
  q = v @ W1.T + b1 ; k = v @ W2.T + b2 ; u = v @ W3.T + b3
  scores[t,s] = q_t . k_s  ->  softmax over s -> h'_t = sum_s P[t,s] u_s
  dec = relu(h Wd1.T + bd1) Wd2.T + bd2 ; out = softmax(dec Wo.T + bo)

Host-side algebra:
  scores[t,s] = v_t . z_s + r_s,  z_s = G v_s + c, r_s = a.v_s + d,
  G = W1.T W2, a = W2.T b1, c = W1.T b2, d = b1.b2; biases fused via an
  augmented ones row so one matmul emits [z | r | u] per slot.

v3 layout/schedule changes vs v2:
  - h slot is 197 wide ([v(196) | 1]); zu slot is 393 ([z | r | u]); the
    dots run over 197 elems and pick up r via the ones column.
  - zu matmuls write two 2-slot PSUM tiles; each is evacuated by ONE Act
    copy (f32->bf16), replacing four per-slot copies.
  - vt transposes write one bf16 PSUM tile; two DVE 2x copies evacuate.
  - dots are 16 STT+accum ops, combine is 4 chains of (seed + 3 MACs);
    both are spread across DVE/Act/Pool by a static table tuned against
    the instruction cost model.
  - probs = e * (1/sum) via one broadcast tensor_tensor.
"""

import numpy as np
import ml_dtypes

import concourse.bass as bass
import concourse.tile as tile
from concourse import bacc, mybir
from concourse.bass import ds
from concourse.bass_utils import run_bass_kernel_spmd
from concourse.masks import make_identity

FR = mybir.dt.float32r
BF = mybir.dt.bfloat16
F32 = mybir.dt.float32
AF = mybir.ActivationFunctionType
ALU = mybir.AluOpType

B = 32768
NCORES = 8
P = 128
T = 4
FV = 196
FEAT = 784
SLOT = FV + 1  # 197: 196 data + ones col
ZSLOT = 2 * FV + 1  # 393: z(196) | r(1) | u(196)

# --- static engine tables (v=DVE, a=Act, p=Pool), tuned vs cost model ---
# dots[t][s] (Act cannot run STT)
DOTS_ENG = [
    "vp..",
    "pp..",
    "pp..",
    "pp..",
]
# combine: SEED_ENG[t] + MAC_ENG[t] (3 chained MACs; MACs only v or p)
# combine mul engine per (t, s); adds are two batched DVE tensor_tensor
MUL_ENG = [
    "vapp",
    "vapp",
    "aapp",
    "aapp",
]
# vt-evac engines for (chunk1, chunk2)
VT_ENG = "a"
# s-slots computed via a DVE products TT + 4 cheap TSP-accums
PROD_S = (2, 3)
ADDS_PER_T = False
ZU0_B2 = False
PHASE_MAJOR = False
PAIR_VEC = False
ZU_BUFS = 5
D2_ACT = False
EXP_ACCUM = False
EXP_SPLIT = False
PROBS_POOL = False
HT_DVE = False
ZU_ONE = True
SCR_ROT = True


def _ap(t, dims, offset_elems=0):
    a = t[:] if hasattr(t, "tile") or not isinstance(t, bass.AP) else t
    return bass.AP(tensor=a.tensor, offset=a.offset + offset_elems, ap=dims)


def build(nsub=8, ngroups=4, unroll=False, zu_bufs=1, vtps_bufs=2, mpd_bufs=2,
          h_bufs=3, wk_bufs=5, sm_bufs=8, zu_mode="half", hf_bufs=1):
    """One NeuronCore program processing nsub*ngroups*128 samples."""
    bpc = nsub * ngroups * P
    nc = bacc.Bacc("TRN2", target_bir_lowering=False, debug=False)

    x_d = nc.dram_tensor("x", [bpc, FEAT], BF, kind="ExternalInput")
    zu_d = nc.dram_tensor("zu_w", [P, 2, ZSLOT], BF, kind="ExternalInput")
    d1_d = nc.dram_tensor("dec1_w", [P, 8, FEAT], BF, kind="ExternalInput")
    d2_d = nc.dram_tensor("dec2_w", [P, 7, FEAT], BF, kind="ExternalInput")
    ow_d = nc.dram_tensor("out_w", [P, 7, 10], BF, kind="ExternalInput")
    out_d = nc.dram_tensor("out", [bpc, 10], F32, kind="ExternalOutput")

    with tile.TileContext(nc) as tc:
        consts = tc.alloc_tile_pool(name="consts", bufs=1)
        hp = tc.alloc_tile_pool(name="h", bufs=h_bufs)
        wk = tc.alloc_tile_pool(name="wk", bufs=wk_bufs)
        wkd = tc.alloc_tile_pool(name="wkd", bufs=1)
        sm = tc.alloc_tile_pool(name="small", bufs=sm_bufs)
        pp = tc.alloc_tile_pool(name="ps", bufs=zu_bufs, space="PSUM")
        pv = tc.alloc_tile_pool(name="pv", bufs=vtps_bufs, space="PSUM")

        ident_f = consts.tile([P, P], F32)
        make_identity(nc, ident_f)
        ident = consts.tile([P, P], FR)
        nc.vector.tensor_copy(ident, ident_f)
        ident_b = consts.tile([P, P], BF)
        nc.vector.tensor_copy(ident_b, ident_f)
        ones_c = consts.tile([P, 512], F32)
        nc.vector.memset(ones_c, 1.0)
        zu_w = consts.tile([P, 2, ZSLOT], BF)
        nc.sync.dma_start(out=zu_w, in_=zu_d[:, :, :])
        d1_w = consts.tile([P, 8, FEAT], BF)
        nc.sync.dma_start(out=d1_w, in_=d1_d[:, :, :])
        d2_w = consts.tile([P, 7, FEAT], BF)
        nc.sync.dma_start(out=d2_w, in_=d2_d[:, :, :])
        ow_w = consts.tile([P, 7, 10], BF)
        nc.sync.dma_start(out=ow_w, in_=ow_d[:, :, :])

        def eng(c):
            return {"v": nc.vector, "p": nc.gpsimd, "a": nc.scalar}[c]

        def capsule_psum(h_cur, j):
            """PE transposes + zu matmuls + evacuations -> zu SBUF tile."""
            # chunk2 is a full 128-row transpose of features 69..196
            # (overlap rows 69..127 are zeroed in the chunk-1 weights), so
            # one evacuation op covers both chunks.
            vt_ps = pv.tile([P, T, 2, P], BF, tag="vtps")
            for t in range(T):
                nc.tensor.transpose(vt_ps[:, t, 0, :], h_cur[:, t, 0:P], ident_b)
                # includes the ones column -> row 127 is 1.0
                nc.tensor.transpose(vt_ps[:, t, 1, :],
                                    h_cur[:, t, 69 : 69 + P], ident_b)
            vt = wk.tile([P, T, 2, P], BF, tag="vt")
            c = VT_ENG[0]
            if c == "a":
                nc.scalar.copy(vt, vt_ps)
            elif c == "v":
                nc.vector.tensor_copy(vt, vt_ps)
            else:
                nc.gpsimd.tensor_copy(vt, vt_ps)

            zu = wk.tile([P, T, ZSLOT], BF, tag="zu", bufs=ZU_BUFS)
            if ZU_ONE:
                zu_ps = pp.tile([P, T, 512], F32, tag="zu0")
                for s in range(T):
                    nc.tensor.matmul(zu_ps[:, s, 0:ZSLOT], vt[:, s, 0, :],
                                     zu_w[:, 0, :], start=True, stop=False)
                    nc.tensor.matmul(zu_ps[:, s, 0:ZSLOT], vt[:, s, 1, :],
                                     zu_w[:, 1, :], start=False, stop=True)
                nc.scalar.copy(zu, zu_ps[:, :, 0:ZSLOT])
            else:
                for half in range(2):
                    # 512-wide slots keep each matmul within one PSUM bank
                    zu_ps = pp.tile([P, 2, 512], F32, tag=f"zu{half}")
                    for k in range(2):
                        s = 2 * half + k
                        nc.tensor.matmul(zu_ps[:, k, 0:ZSLOT], vt[:, s, 0, :],
                                         zu_w[:, 0, :], start=True, stop=False)
                        nc.tensor.matmul(zu_ps[:, k, 0:ZSLOT], vt[:, s, 1, :],
                                         zu_w[:, 1, :], start=False, stop=True)
                    nc.scalar.copy(zu[:, 2 * half : 2 * half + 2, :],
                                   zu_ps[:, :, 0:ZSLOT])
            return zu

        def capsule_vec(h_cur, h_nxt, zu, j):
            """SBUF-only: dots, softmax, combine -> h_nxt."""
            dots = sm.tile([P, T, T], F32, tag="dots")
            scr = sm.tile([P, 3, SLOT], BF, tag="scr", bufs=8)
            halves = [(0, 4)] if ZU_ONE else [(0, 2), (2, 2)]
            for h0, hn_ in halves:
                prod = sm.tile([P, hn_, T, SLOT], BF, tag=f"prod{h0}",
                               bufs=(2 if len(halves) > 1 else 3))
                hin = _ap(h_cur, [h_cur[:].ap[0], [0, hn_], [SLOT, T],
                                  [1, SLOT]])
                zin = _ap(zu, [zu[:].ap[0], [ZSLOT, hn_], [0, T], [1, SLOT]],
                          offset_elems=h0 * ZSLOT)
                nc.vector.tensor_tensor(out=prod, in0=hin, in1=zin, op=ALU.mult)
                for k in range(hn_):
                    s = h0 + k
                    for t in range(T):
                        nc.vector.tensor_scalar(
                            out=scr[:, (k + t) % 3 if SCR_ROT else 0, :],
                            in0=prod[:, k, t, :], scalar1=1.0,
                            scalar2=0.0, op0=ALU.mult, op1=ALU.add,
                            accum_out=dots[:, t, s : s + 1])

            # softmax over s (no max subtraction; |scores| < 30)
            e_t = sm.tile([P, T, T], F32, tag="e")
            sums = sm.tile([P, T], F32, tag="sums")
            if EXP_ACCUM:
                for t in range(T):
                    nc.scalar.activation(e_t[:, t, :], dots[:, t, :], AF.Exp,
                                         accum_out=sums[:, t : t + 1])
            else:
                if EXP_SPLIT:
                    nc.scalar.activation(e_t[:, :, 0:2], dots[:, :, 0:2],
                                         AF.Exp)
                    nc.scalar.activation(e_t[:, :, 2:4], dots[:, :, 2:4],
                                         AF.Exp)
                else:
                    nc.scalar.activation(e_t, dots, AF.Exp)
                nc.vector.reduce_sum(sums, e_t, axis=mybir.AxisListType.X)
            rec = sm.tile([P, T], F32, tag="rec")
            nc.vector.reciprocal(rec, sums)
            probs = sm.tile([P, T, T], F32, tag="probs")
            (nc.gpsimd if PROBS_POOL else nc.vector).tensor_tensor(
                out=probs, in0=e_t,
                in1=_ap(rec, [rec[:].ap[0], [1, T], [0, T]]),
                op=ALU.mult,
            )

            # ones column for the next h
            nc.gpsimd.tensor_copy(h_nxt[:, :, FV:SLOT], ones_c[:, 0:T])
            # combine: pu[t,s] = P[t,s] * u_s, then two batched DVE adds
            pu = sm.tile([P, T, T, FV], BF, tag="pu", bufs=2)
            for t in range(T):
                for s in range(T):
                    c = MUL_ENG[t][s]
                    if c == "a":
                        nc.scalar.activation(
                            pu[:, t, s, :], zu[:, s, SLOT:ZSLOT], AF.Copy,
                            scale=probs[:, t, s : s + 1])
                    elif c == "v":
                        nc.vector.tensor_scalar_mul(
                            pu[:, t, s, :], zu[:, s, SLOT:ZSLOT],
                            probs[:, t, s : s + 1])
                    else:
                        nc.gpsimd.tensor_scalar_mul(
                            pu[:, t, s, :], zu[:, s, SLOT:ZSLOT],
                            probs[:, t, s : s + 1])
            q = sm.tile([P, T, 2, FV], BF, tag="q", bufs=2)
            ev = _ap(pu, [pu[:].ap[0], [T * FV, T], [2 * FV, 2], [1, FV]])
            od = _ap(pu, [pu[:].ap[0], [T * FV, T], [2 * FV, 2], [1, FV]],
                     offset_elems=FV)
            nc.vector.tensor_tensor(out=q, in0=ev, in1=od, op=ALU.add)
            nc.vector.tensor_tensor(out=h_nxt[:, :, 0:FV], in0=q[:, :, 0, :],
                                    in1=q[:, :, 1, :], op=ALU.add)

        def capsule_vec_pair(hc, hn, zus):
            """Two tiles' dots/softmax/combine with pair-batched softmax."""
            npair = len(hc)
            dots = sm.tile([P, 2, T, T], F32, tag="dots")
            scr = sm.tile([P, 3, SLOT], BF, tag="scr", bufs=8)
            for jj in range(npair):
                for half in range(2):
                    prod = sm.tile([P, 2, T, SLOT], BF, tag=f"prod{half}",
                                   bufs=2)
                    hin = _ap(hc[jj], [hc[jj][:].ap[0], [0, 2], [SLOT, T],
                                       [1, SLOT]])
                    zin = _ap(zus[jj], [zus[jj][:].ap[0], [ZSLOT, 2], [0, T],
                                        [1, SLOT]],
                              offset_elems=half * 2 * ZSLOT)
                    nc.vector.tensor_tensor(out=prod, in0=hin, in1=zin,
                                            op=ALU.mult)
                    for k in range(2):
                        s = 2 * half + k
                        for t in range(T):
                            nc.vector.tensor_scalar(
                                out=scr[:, 0, :],
                                in0=prod[:, k, t, :], scalar1=1.0,
                                scalar2=0.0, op0=ALU.mult, op1=ALU.add,
                                accum_out=dots[:, jj, t, s : s + 1])

            # pair-batched softmax (no max subtraction; |scores| < 30)
            e_t = sm.tile([P, 2, T, T], F32, tag="e")
            nc.scalar.activation(e_t[:, 0:npair], dots[:, 0:npair], AF.Exp)
            sums = sm.tile([P, 2, T], F32, tag="sums")
            nc.vector.reduce_sum(sums[:, 0:npair], e_t[:, 0:npair],
                                 axis=mybir.AxisListType.X)
            rec = sm.tile([P, 2, T], F32, tag="rec")
            nc.vector.reciprocal(rec[:, 0:npair], sums[:, 0:npair])
            probs = sm.tile([P, 2, T, T], F32, tag="probs")
            nc.vector.tensor_tensor(
                out=probs[:, 0:npair], in0=e_t[:, 0:npair],
                in1=_ap(rec, [rec[:].ap[0], [T, npair], [1, T], [0, T]]),
                op=ALU.mult,
            )

            for jj in range(npair):
                nc.gpsimd.tensor_copy(hn[jj][:, :, FV:SLOT], ones_c[:, 0:T])
                pu = sm.tile([P, T, T, FV], BF, tag="pu", bufs=2)
                for t in range(T):
                    for s in range(T):
                        c = MUL_ENG[t][s]
                        if c == "a":
                            nc.scalar.activation(
                                pu[:, t, s, :], zus[jj][:, s, SLOT:ZSLOT],
                                AF.Copy, scale=probs[:, jj, t, s : s + 1])
                        elif c == "v":
                            nc.vector.tensor_scalar_mul(
                                pu[:, t, s, :], zus[jj][:, s, SLOT:ZSLOT],
                                probs[:, jj, t, s : s + 1])
                        else:
                            nc.gpsimd.tensor_scalar_mul(
                                pu[:, t, s, :], zus[jj][:, s, SLOT:ZSLOT],
                                probs[:, jj, t, s : s + 1])
                q = sm.tile([P, T, 2, FV], BF, tag="q", bufs=2)
                ev = _ap(pu, [pu[:].ap[0], [T * FV, T], [2 * FV, 2], [1, FV]])
                od = _ap(pu, [pu[:].ap[0], [T * FV, T], [2 * FV, 2], [1, FV]],
                         offset_elems=FV)
                nc.vector.tensor_tensor(out=q, in0=ev, in1=od, op=ALU.add)
                nc.vector.tensor_tensor(out=hn[jj][:, :, 0:FV],
                                        in0=q[:, :, 0, :],
                                        in1=q[:, :, 1, :], op=ALU.add)

        def capsule_iter(h_cur, h_nxt, j):
            zu = capsule_psum(h_cur, j)
            capsule_vec(h_cur, h_nxt, zu, j)

        def decoder(hs, g, goff=0):
            """Decoder over a chunk of <=4 tiles (N = len(hs)*128 wide)."""
            W = len(hs) * P
            # h.T chunks, slot-major: [128] x4 and [69] x4 (with ones row)
            # chunk2 is a full 128-row transpose of features 69..196 per t
            # (overlap rows zeroed in the chunk-1 weights): one evac per t.
            ht = wkd.tile([P, T, 2, W], BF, tag="ht1")
            for t in range(T):
                t_ps = pv.tile([P, 2, W], BF, tag="vtps")
                for j in range(len(hs)):
                    nc.tensor.transpose(
                        t_ps[:, 0, j * P : (j + 1) * P], hs[j][:, t, 0:P],
                        ident_b)
                    nc.tensor.transpose(
                        t_ps[:, 1, j * P : (j + 1) * P],
                        hs[j][:, t, 69 : 69 + P], ident_b)
                if HT_DVE:
                    nc.vector.tensor_copy(ht[:, t, :, :], t_ps)
                else:
                    nc.scalar.copy(ht[:, t, :, :], t_ps)

            # dec1 = relu(Wd1 @ h.T + bd1), feature-major, 7 M-chunks
            d1a = wkd.tile([P, 6, W], BF, tag="d1a")
            d1b = wkd.tile([17, W], BF, tag="d1b")
            nc.vector.tensor_copy(d1b, ones_c[0:17, 0:W])
            for m in range(7):
                mw = min(P, FEAT - m * P)
                mp = pv.tile([P, W], F32, tag="mpd", bufs=mpd_bufs)
                msl = slice(m * P, m * P + mw)
                for t in range(T):
                    nc.tensor.matmul(mp[0:mw, :], d1_w[:, t, msl],
                                     ht[:, t, 0, :], start=(t == 0), stop=False)
                for t in range(T):
                    nc.tensor.matmul(mp[0:mw, :], d1_w[:, 4 + t, msl],
                                     ht[:, t, 1, :], start=False, stop=(t == 3))
                if m < 6:
                    nc.scalar.activation(d1a[:, m, :], mp, AF.Relu)
                else:
                    nc.scalar.activation(d1b[0:16, :], mp[0:16, :], AF.Relu)

            # dec2 = Wd2 @ relu1 + bd2, feature-major
            d2a = wkd.tile([P, 6, W], BF, tag="d2a")
            d2b = wkd.tile([17, W], BF, tag="d2b")
            nc.vector.tensor_copy(d2b, ones_c[0:17, 0:W])
            for m in range(7):
                mw = min(P, FEAT - m * P)
                mp = pv.tile([P, W], F32, tag="mpd", bufs=mpd_bufs)
                msl = slice(m * P, m * P + mw)
                for c in range(6):
                    nc.tensor.matmul(mp[0:mw, :], d2_w[:, c, msl], d1a[:, c, :],
                                     start=(c == 0), stop=False)
                nc.tensor.matmul(mp[0:mw, :], d2_w[0:17, 6, msl], d1b,
                                 start=False, stop=True)
                if m < 6:
                    (nc.scalar.copy if D2_ACT else nc.vector.tensor_copy)(
                        d2a[:, m, :], mp)
                else:
                    (nc.scalar.copy if D2_ACT else nc.vector.tensor_copy)(
                        d2b[0:16, :], mp[0:16, :])

            # logits for all subtiles into one PSUM tile, then one
            # batched softmax (no max subtraction; |logits| < 30) and a
            # single strided output DMA.
            nh = len(hs)
            lgs = pv.tile([P, nh, 10], F32, tag="mpd", bufs=mpd_bufs)
            for j in range(nh):
                jsl = slice(j * P, (j + 1) * P)
                for c in range(6):
                    nc.tensor.matmul(lgs[:, j, :], d2a[:, c, jsl], ow_w[:, c, :],
                                     start=(c == 0), stop=False)
                nc.tensor.matmul(lgs[:, j, :], d2b[:, jsl], ow_w[0:17, 6, :],
                                 start=False, stop=True)
            e10 = sm.tile([P, nh, 10], F32, tag="e10")
            nc.scalar.activation(e10, lgs, AF.Exp)
            s10 = sm.tile([P, nh], F32, tag="s10")
            nc.vector.reduce_sum(s10, e10, axis=mybir.AxisListType.X)
            r10 = sm.tile([P, nh], F32, tag="r10")
            nc.vector.reciprocal(r10, s10)
            o10 = sm.tile([P, nh, 10], F32, tag="o10")
            nc.vector.tensor_tensor(
                out=o10, in0=e10,
                in1=_ap(r10, [r10[:].ap[0], [1, nh], [0, 10]]),
                op=ALU.mult,
            )
            base = out_d[ds(g * (nsub * P) + goff * P, P), :]
            oap = bass.AP(tensor=base.tensor, offset=base.offset,
                          ap=[base.ap[0], [10 * P, nh], [1, 10]])
            nc.sync.dma_start(out=oap, in_=o10)

        def body_capsule(g):
            hs = []
            for j in range(nsub):
                h0 = hp.tile([P, T, SLOT], BF, tag=f"h{j}")
                nc.sync.dma_start(
                    out=h0[:, :, 0:FV],
                    in_=x_d[ds(g * (nsub * P) + j * P, P), :].rearrange(
                        "p (t f) -> p t f", t=T
                    ),
                )
                nc.gpsimd.tensor_copy(h0[:, :, FV:SLOT], ones_c[:, 0:T])
                hs.append(h0)
            for it in range(8):
                nxts = []
                for j in range(nsub):
                    if it < 7:
                        h_nxt = hp.tile([P, T, SLOT], BF, tag=f"h{j}")
                    else:
                        h_nxt = hp.tile([P, T, SLOT], BF, tag=f"hold{j}",
                                        bufs=2)
                    nxts.append(h_nxt)
                if PAIR_VEC:
                    for j0 in range(0, nsub, 2):
                        zus = [capsule_psum(hs[j], j)
                               for j in range(j0, min(j0 + 2, nsub))]
                        capsule_vec_pair(hs[j0 : j0 + 2], nxts[j0 : j0 + 2],
                                         zus)
                elif PHASE_MAJOR:
                    zus = [capsule_psum(hs[j], j) for j in range(nsub)]
                    for j in range(nsub):
                        capsule_vec(hs[j], nxts[j], zus[j], j)
                else:
                    for j in range(nsub):
                        capsule_iter(hs[j], nxts[j], j)
                hs = list(nxts)
            return hs

        def body(g):
            hs = body_capsule(g)
            for d0 in range(0, nsub, 4):
                decoder(hs[d0 : d0 + 4], g, d0)

        if ngroups == 1:
            body(0)
        elif unroll:
            # software-pipelined: decode group g-1 while computing group g
            holds = [body_capsule(0)]
            for g in range(1, ngroups):
                holds.append(body_capsule(g))
                for d0 in range(0, nsub, 4):
                    decoder(holds[g - 1][d0 : d0 + 4], g - 1, d0)
            for d0 in range(0, nsub, 4):
                decoder(holds[-1][d0 : d0 + 4], ngroups - 1, d0)
        else:
            with tc.For_i(0, ngroups, 1) as g:
                body(g)
        for _pool in (pv, pp, sm, wkd, wk, hp, consts):
            _pool.release()

    nc.compile()
    return nc


def pack_weights(W1, b1, W2, b2, W3, b3, Wd1, bd1, Wd2, bd2, Wo, bo):
    f64 = np.float64
    W1, b1, W2, b2, W3, b3 = (np.asarray(t, f64) for t in (W1, b1, W2, b2, W3, b3))
    G = W1.T @ W2
    a = W2.T @ b1
    c = W1.T @ b2
    d = float(b1 @ b2)

    zu = np.zeros((P, 2, ZSLOT), np.float32)
    full = np.zeros((197, ZSLOT), f64)
    full[:196, :196] = G.T
    full[:196, 196] = a
    full[:196, SLOT:] = W3.T
    full[196, :196] = c
    full[196, 196] = d
    full[196, SLOT:] = b3
    zu[:, 0, :] = full[0:128]
    zu[69:128, 0, :] = 0.0
    zu[:, 1, :] = full[69:197]

    d1 = np.zeros((P, 8, FEAT), np.float32)
    W1T = np.asarray(Wd1, f64).T  # [784 f_in, 784 j]
    for t in range(T):
        d1[:, t, :] = W1T[t * FV : t * FV + P, :]
        d1[69:128, t, :] = 0.0
        d1[0:127, 4 + t, :] = W1T[t * FV + 69 : t * FV + FV, :]
    # row 127 of every chunk-2 transpose is the ones column; add bd1 once
    d1[127, 4, :] = np.asarray(bd1, f64)

    d2 = np.zeros((P, 7, FEAT), np.float32)
    W2T = np.asarray(Wd2, f64).T
    for cidx in range(6):
        d2[:, cidx, :] = W2T[cidx * P : (cidx + 1) * P, :]
    d2[0:16, 6, :] = W2T[768:784, :]
    d2[16, 6, :] = np.asarray(bd2, f64)

    ow = np.zeros((P, 7, 10), np.float32)
    WoT = np.asarray(Wo, f64).T
    for cidx in range(6):
        ow[:, cidx, :] = WoT[cidx * P : (cidx + 1) * P, :]
    ow[0:16, 6, :] = WoT[768:784, :]
    ow[16, 6, :] = np.asarray(bo, f64)
    return (zu.astype(ml_dtypes.bfloat16), d1.astype(ml_dtypes.bfloat16),
            d2.astype(ml_dtypes.bfloat16), ow.astype(ml_dtypes.bfloat16))


_NC_CACHE = {}


def kernel(**inputs):
    x = np.ascontiguousarray(np.asarray(inputs["x"], np.float32)).astype(
        ml_dtypes.bfloat16
    )
    zu, d1, d2, ow = pack_weights(
        inputs["W1"], inputs["b1"], inputs["W2"], inputs["b2"], inputs["W3"],
        inputs["b3"], inputs["Wd1"], inputs["bd1"], inputs["Wd2"],
        inputs["bd2"], inputs["Wo"], inputs["bo"],
    )
    if "nc" not in _NC_CACHE:
        _NC_CACHE["nc"] = build(8, 4, unroll=True)
    nc = _NC_CACHE["nc"]
    bpc = B // NCORES
    in_maps = [
        {
            "x": x[c * bpc : (c + 1) * bpc],
            "zu_w": zu,
            "dec1_w": d1,
            "dec2_w": d2,
            "out_w": ow,
        }
        for c in range(NCORES)
    ]
    res = run_bass_kernel_spmd(nc, in_maps, core_ids=list(range(NCORES)))
    return np.concatenate([res.results[c]["out"] for c in range(NCORES)], axis=0)
